# revision 3
# baseline (speedup 1.0000x reference)
"""Trainium2 Bass kernel for nn_AutoregressiveDecoder (LSTM decoder w/ greedy sampling).

Strategy (8 NeuronCores, SPMD):
  - Vocab-shard the fc projection: core j holds Wfc rows [4000j, 4000(j+1)) padded to
    4096 (pad bias = -1e30), resident in SBUF.
  - LSTM weights replicated per core, SBUF-resident, gate columns permuted so that
    PE column-group q computes [i|f|g|o] for hidden quarter q -> full-partition
    elementwise state updates.
  - All matmuls bf16x3 (hi*hi + hi*lo + lo*hi, fp32 psum accumulate) with 4-way PE
    column tiling; fc bias folded into the matmul via a 2-row ones lhsT so the
    argmax reads finished logits straight out of PSUM.
  - Greedy token: per-half (512-wide) max8/max_index pipelined under the second
    half's matmul, 4 candidates/core AllGather'd ([16,32] payload), single
    multi-axis-reduce fold after the exchange, indirect-DMA embedding gather.
  - Filler matmuls (zero operands) span the AllGather window so the PE's HAM
    clock gate stays at full rate across the per-step collective stall.
  - Logits stream to DRAM as bf16 [T, 128, 1024] per core; host reassembles
    [B, T, V] in fp32 (output tolerance is 2e-2; bf16 staging halves the
    copy+DMA cost while the on-device argmax stays fp32-exact).
"""
import sys

sys.path.insert(0, "/opt/trn_rl_repo")

import numpy as np

import concourse.bass as bass
import concourse.bacc as bacc
import concourse.tile as tile
import concourse.mybir as mybir
from concourse.bass_utils import run_bass_kernel_spmd

FP32 = mybir.dt.float32
BF16 = mybir.dt.bfloat16
I32 = mybir.dt.int32
U32 = mybir.dt.uint32

N_CORES = 8
B, L, H, E, V, T = 32, 256, 512, 512, 32000, 64
VS = V // N_CORES          # 4000 true shard
VSP = 4096                 # padded shard
BIG = 65536.0
N_FILL = 28                # filler MMs spanning the AllGather window

Sigmoid = mybir.ActivationFunctionType.Sigmoid
Tanh = mybir.ActivationFunctionType.Tanh
Alu = mybir.AluOpType


def build_decoder(nc, io, n_steps):
    """Emit the full unrolled decoder. io: dict name -> DRAM AP."""
    with tile.TileContext(nc) as tc:
        sb = tc.alloc_tile_pool(name="sb", bufs=1)
        sb2 = tc.alloc_tile_pool(name="sb2", bufs=3)
        ps_g = tc.alloc_tile_pool(name="ps_g", bufs=2, space="PSUM")
        ps_v = tc.alloc_tile_pool(name="ps_v", bufs=2, space="PSUM")
        ps_s = tc.alloc_tile_pool(name="ps_s", bufs=2, space="PSUM")
        dr = tc.alloc_tile_pool(name="dr", bufs=4, space="DRAM")
        pools = [sb, sb2, ps_g, ps_v, ps_s, dr]

        # ---- persistent SBUF state & weights ----
        wfh = [sb.tile([128, VSP], BF16, name=f"wfh{q}") for q in range(4)]
        wfl = [sb.tile([128, VSP], BF16, name=f"wfl{q}") for q in range(4)]
        wgh = [sb.tile([128, 2048], BF16, name=f"wgh{r}") for r in range(8)]
        wgl = [sb.tile([128, 2048], BF16, name=f"wgl{r}") for r in range(8)]
        bias_g2 = sb.tile([2, 2048], BF16, name="bias_g2")
        bias_fc2 = sb.tile([2, VSP], BF16, name="bias_fc2")
        gkey = sb.tile([128, 2], FP32, name="gkey")   # BIG - (VS*j + 1024*g + 512*nt)
        ident = sb.tile([128, 128], FP32, name="ident")
        ones2 = sb.tile([2, 32], BF16, name="ones2")
        zeros = sb.tile([128, 512], BF16, name="zeros")
        xT = sb.tile([128, 128], FP32, name="xT")
        hT = sb.tile([128, 128], FP32, name="hT")
        xTh = sb.tile([128, 128], BF16, name="xTh")
        xTl = sb.tile([128, 128], BF16, name="xTl")
        hTh = sb.tile([128, 128], BF16, name="hTh")
        hTl = sb.tile([128, 128], BF16, name="hTl")
        c_t = sb.tile([128, 128], FP32, name="c_t")

        for q in range(4):
            nc.sync.dma_start(wfh[q][:], io["wfc_hi"][128 * q:128 * (q + 1), :])
            nc.sync.dma_start(wfl[q][:], io["wfc_lo"][128 * q:128 * (q + 1), :])
        for r in range(8):
            nc.sync.dma_start(wgh[r][:], io["wgates_hi"][128 * r:128 * (r + 1), :])
            nc.sync.dma_start(wgl[r][:], io["wgates_lo"][128 * r:128 * (r + 1), :])
        nc.sync.dma_start(bias_g2[:], io["bias_g2"][:])
        nc.sync.dma_start(bias_fc2[:], io["bias_fc2"][:])
        nc.sync.dma_start(gkey[:], io["gkey"][:])
        nc.sync.dma_start(ident[:], io["ident"][:])
        nc.sync.dma_start(ones2[:], io["ones2"][:])
        nc.sync.dma_start(xT[:], io["h0t"][:])
        nc.sync.dma_start(hT[:], io["h0t"][:])
        nc.sync.dma_start(c_t[:], io["c0"][:])
        nc.vector.memset(zeros[:], 0.0)
        # initial hi/lo splits of the (identical) x0 = h0 state
        nc.vector.tensor_copy(hTh[:], hT[:])
        nc.vector.tensor_tensor(hTl[:], hT[:], hTh[:], op=Alu.subtract)
        nc.vector.tensor_copy(xTh[:], hTh[:])
        nc.vector.tensor_copy(xTl[:], hTl[:])

        emb = io["emb"]
        out_logits = io["logits"]  # [T, 128, 1024] bf16

        # ---- gates matmul emission helpers ----
        # psum layout: partition 32q+b, free = gate*128+hw (cols permuted on host)
        def emit_gates_bias_h(pg):
            for g in range(4):
                nc.tensor.matmul(
                    pg[32 * g:32 * (g + 1), :], lhsT=ones2[:, :],
                    rhs=bias_g2[:, 512 * g:512 * (g + 1)],
                    start=True, stop=False, tile_position=(0, 32 * g),
                    skip_group_check=True,
                )
            emit_gates_rounds(pg, [4, 5, 6, 7], stop=False)

        def emit_gates_rounds(pg, rounds, stop):
            for r in rounds:
                hi, lo = (xTh, xTl) if r < 4 else (hTh, hTl)
                q = r % 4
                cs = slice(32 * q, 32 * (q + 1))
                for g in range(4):
                    gs = slice(512 * g, 512 * (g + 1))
                    out = pg[32 * g:32 * (g + 1), :]
                    passes = ((hi[:, cs], wgh[r][:, gs]),
                              (lo[:, cs], wgh[r][:, gs]),
                              (hi[:, cs], wgl[r][:, gs]))
                    for pi, (lhsT, rhs) in enumerate(passes):
                        nc.tensor.matmul(
                            out, lhsT=lhsT, rhs=rhs,
                            start=False,
                            stop=(stop and r == rounds[-1] and pi == 2),
                            tile_position=(0, 32 * g),
                            skip_group_check=True,
                        )

        # step-0 gates: bias + h-rounds up front (x == h0 so all 8 rounds)
        pg = ps_g.tile([128, 512], FP32, name="pg", tag="pg")
        emit_gates_bias_h(pg)

        for t in range(n_steps):
            last = t == n_steps - 1
            # ================= gates matmul: x-rounds =================
            emit_gates_rounds(pg, [0, 1, 2, 3], stop=True)

            # ================= activations / state =================
            # gate slots after host permutation: [i | f | o | g(tanh)]
            acts = sb2.tile([128, 512], FP32, name="acts", tag="acts")
            nc.scalar.activation(acts[:, 0:384], pg[:, 0:384], Sigmoid)
            nc.scalar.activation(acts[:, 384:512], pg[:, 384:512], Tanh)
            t1 = sb2.tile([128, 128], FP32, name="t1", tag="t1")
            nc.vector.tensor_tensor(t1[:], acts[:, 0:128], acts[:, 384:512], op=Alu.mult)
            nc.vector.tensor_tensor(c_t[:], acts[:, 128:256], c_t[:], op=Alu.mult)
            nc.vector.tensor_tensor(c_t[:], c_t[:], t1[:], op=Alu.add)
            tanh_c = sb2.tile([128, 128], FP32, name="tanh_c", tag="tanh_c")
            nc.scalar.activation(tanh_c[:], c_t[:], Tanh)
            h_new = sb2.tile([128, 128], FP32, name="h_new", tag="h_new")
            nc.vector.tensor_tensor(h_new[:], acts[:, 256:384], tanh_c[:], op=Alu.mult)

            # hT = transpose(h_new); hi cast on ACT in parallel with fp32 copy on DVE
            p_ht = ps_s.tile([128, 128], FP32, name="p_ht", tag="small")
            nc.tensor.transpose(p_ht[:], h_new[:], ident[:])
            nc.scalar.copy(hTh[:], p_ht[:])
            nc.vector.tensor_copy(hT[:], p_ht[:])
            nc.vector.tensor_tensor(hTl[:], hT[:], hTh[:], op=Alu.subtract)

            # ================= vocab matmul (bias folded in) =================
            # psum layout: partition 32g+b (g = vocab quarter of shard), free 1024
            pv = ps_v.tile([128, 1024], FP32, name="pv", tag="pv")
            staged = sb2.tile([128, 1024], BF16, name="staged", tag="staged")
            v8 = [None, None]
            i8 = [None, None]
            for nt in range(2):
                for g in range(4):
                    ws = slice(1024 * g + 512 * nt, 1024 * g + 512 * (nt + 1))
                    nc.tensor.matmul(
                        pv[32 * g:32 * (g + 1), 512 * nt:512 * (nt + 1)],
                        lhsT=ones2[:, :], rhs=bias_fc2[:, ws],
                        start=True, stop=False, tile_position=(0, 32 * g),
                        skip_group_check=True,
                    )
                for q in range(4):
                    cs = slice(32 * q, 32 * (q + 1))
                    for g in range(4):
                        ws = slice(1024 * g + 512 * nt, 1024 * g + 512 * (nt + 1))
                        out = pv[32 * g:32 * (g + 1), 512 * nt:512 * (nt + 1)]
                        passes = ((hTh[:, cs], wfh[q][:, ws]),
                                  (hTl[:, cs], wfh[q][:, ws]),
                                  (hTh[:, cs], wfl[q][:, ws]))
                        for pi, (lhsT, rhs) in enumerate(passes):
                            nc.tensor.matmul(
                                out, lhsT=lhsT, rhs=rhs,
                                start=False,
                                stop=(q == 3 and pi == 2),
                                tile_position=(0, 32 * g),
                                skip_group_check=True,
                            )
                # stage this half to DRAM (bf16) and find its candidate; the
                # nt=0 chain runs on ACT/DVE under the nt=1 matmul.
                half = slice(512 * nt, 512 * (nt + 1))
                nc.scalar.copy(staged[:, half], pv[:, half])
                if not last:
                    v8[nt] = sb2.tile([128, 8], FP32, name=f"v8{nt}", tag=f"v8{nt}")
                    i8[nt] = sb2.tile([128, 8], U32, name=f"i8{nt}", tag=f"i8{nt}")
                    nc.vector.max(v8[nt][:], pv[:, half])
                    nc.vector.max_index(i8[nt][:], v8[nt][:], pv[:, half])
            nc.scalar.dma_start(out_logits[t], staged[:])

            if not last:
                # pay rows (per psum partition 32g+b): [v_a, v_b, key_a, key_b]
                # where key = BIG - global_idx (so keys never collide with
                # logit values in the eq-fold, and max(key) = min global idx).
                pay = sb2.tile([128, 4], FP32, name="pay", tag="pay")
                iloc = sb2.tile([128, 2], FP32, name="iloc", tag="iloc")
                for nt in range(2):
                    nc.vector.tensor_copy(pay[:, nt:nt + 1], v8[nt][:, 0:1])
                    nc.vector.tensor_copy(iloc[:, nt:nt + 1], i8[nt][:, 0:1])
                    nc.vector.tensor_scalar(
                        pay[:, 2 + nt:3 + nt], iloc[:, nt:nt + 1],
                        -1.0, gkey[:, nt:nt + 1], op0=Alu.mult, op1=Alu.add)

                # transpose candidates -> [4, 128] and ship [16, 32] to the AG
                p_pa = ps_s.tile([4, 128], FP32, name="p_pa", tag="small")
                nc.tensor.transpose(p_pa[:], pay[:], ident[:])
                payT = sb2.tile([4, 128], FP32, name="payT", tag="payT")
                nc.scalar.copy(payT[:], p_pa[:])

                cc_in = dr.tile([16, 32], FP32, name="cc_in", tag="cc_in")
                cc_out = dr.tile([128, 32], FP32, name="cc_out", tag="cc_out",
                                 addr_space="Shared")
                # cc_in row = g*4 + f  <-  payT row f, free g*32+b
                nc.sync.dma_start(
                    cc_in[:].rearrange("(g f) b -> f g b", g=4, f=4),
                    payT[:].rearrange("f (g b) -> f g b", g=4))
                nc.gpsimd.collective_compute(
                    "AllGather", Alu.bypass,
                    replica_groups=[list(range(N_CORES))],
                    ins=[cc_in[:]], outs=[cc_out[:]],
                )

            # ================= next-step gates: bias + h rounds =================
            if not last:
                pg = ps_g.tile([128, 512], FP32, name="pg", tag="pg")
                emit_gates_bias_h(pg)

                # ================= PE fillers: keep HAM warm across the AG =====
                p_fil = ps_s.tile([128, 512], FP32, name="p_fil", tag="small")
                for f in range(N_FILL):
                    nc.tensor.matmul(
                        p_fil[:], lhsT=zeros[:, 0:128], rhs=zeros[:],
                        start=True, stop=True, skip_group_check=True,
                    )

                # ================= AG result: fold over 64 candidates ==========
                agb = sb2.tile([128, 32], FP32, name="agb", tag="agb")
                nc.sync.dma_start(agb[:], cc_out[:])
                p_ag = ps_s.tile([32, 128], FP32, name="p_ag", tag="small")
                nc.tensor.transpose(p_ag[:], agb[:], ident[:])
                t32 = sb2.tile([32, 128], FP32, name="t32", tag="t32")
                nc.vector.tensor_copy(t32[:], p_ag[:])

                # col = r*16 + g*4 + f; candidate order (r, g, f) is global-idx
                # order, and key = BIG - gidx makes max pick the first occurrence.
                tv = t32[:].rearrange("p (r g f) -> p r g f", r=8, g=4, f=4)
                vals = tv[:, :, :, 0:2]
                keys = tv[:, :, :, 2:4]
                gv32 = sb2.tile([32, 1], FP32, name="gv32", tag="gv32")
                nc.vector.tensor_reduce(gv32[:], vals, axis=mybir.AxisListType.XYZ,
                                        op=Alu.max)
                eqt = sb2.tile([32, 64], FP32, name="eqt", tag="eqt")
                eqv = eqt[:].rearrange("p (r g f) -> p r g f", r=8, g=4, f=2)
                nc.vector.tensor_scalar(eqv, vals, gv32[:, 0:1], None,
                                        op0=Alu.is_equal)
                mselt = sb2.tile([32, 64], FP32, name="mselt", tag="mselt")
                mselv = mselt[:].rearrange("p (r g f) -> p r g f", r=8, g=4, f=2)
                nc.vector.tensor_tensor(mselv, eqv, keys, op=Alu.mult)
                m2r = sb2.tile([32, 1], FP32, name="m2r", tag="m2r")
                nc.vector.tensor_reduce(m2r[:], mselv, axis=mybir.AxisListType.XYZ,
                                        op=Alu.max)
                idxf = sb2.tile([32, 1], FP32, name="idxf", tag="idxf")
                nc.vector.tensor_scalar(idxf[:], m2r[:], -1.0, BIG,
                                        op0=Alu.mult, op1=Alu.add)
                idx32 = sb2.tile([32, 1], I32, name="idx32", tag="idx32")
                nc.vector.tensor_copy(idx32[:], idxf[:])

                # ================= embedding gather + transpose =================
                x_rows = sb2.tile([32, 512], FP32, name="x_rows", tag="x_rows")
                nc.gpsimd.indirect_dma_start(
                    out=x_rows[:], out_offset=None, in_=emb[:],
                    in_offset=bass.IndirectOffsetOnAxis(ap=idx32[:, 0:1], axis=0),
                )
                p_x = ps_s.tile([128, 128], FP32, name="p_x", tag="small")
                for q in range(4):
                    nc.tensor.transpose(
                        p_x[:, 32 * q:32 * (q + 1)],
                        x_rows[:, 128 * q:128 * (q + 1)], ident[0:32, 0:32])
                nc.scalar.copy(xTh[:], p_x[:])
                nc.vector.tensor_copy(xT[:], p_x[:])
                nc.vector.tensor_tensor(xTl[:], xT[:], xTh[:], op=Alu.subtract)

        for p in reversed(pools):
            p.release()


def host_prep(inputs):
    """Build per-core in_maps from the full problem inputs."""
    z = np.asarray(inputs["z"], np.float32)
    embedding = np.ascontiguousarray(np.asarray(inputs["embedding"], np.float32))
    Wh = np.asarray(inputs["Wh"], np.float32)
    bh = np.asarray(inputs["bh"], np.float32)
    Wc = np.asarray(inputs["Wc"], np.float32)
    bc = np.asarray(inputs["bc"], np.float32)
    Wih = np.asarray(inputs["Wih"], np.float32)
    Whh = np.asarray(inputs["Whh"], np.float32)
    bih = np.asarray(inputs["bih"], np.float32)
    bhh = np.asarray(inputs["bhh"], np.float32)
    Wfc = np.asarray(inputs["Wfc"], np.float32)
    bfc = np.asarray(inputs["bfc"], np.float32)

    h0 = (z @ Wh.T + bh).astype(np.float32)   # [B, H]
    c0 = (z @ Wc.T + bc).astype(np.float32)
    b_gates = (bih + bhh).astype(np.float32)  # [4H]

    # gate column permutation: c' = q*512 + slot*128 + hw with slot order
    # [i, f, o, g] so the sigmoid gates are one contiguous 384-wide range.
    cp = np.arange(2048)
    qq, rem = cp // 512, cp % 512
    slot, hw = rem // 128, rem % 128
    gate = np.array([0, 1, 3, 2])[slot]        # slot -> original gate (i,f,o,g)
    perm = gate * 512 + qq * 128 + hw          # original col index for permuted col c'
    Wall = np.concatenate([Wih, Whh], axis=1)  # [2048, 1024] (k = [x | h])
    Wperm = Wall[perm]                         # [2048, 1024]
    wgates = np.ascontiguousarray(Wperm.T)     # [1024, 2048]

    import ml_dtypes

    def split_bf16(w):
        hi = w.astype(ml_dtypes.bfloat16)
        lo = (w - hi.astype(np.float32)).astype(ml_dtypes.bfloat16)
        return np.ascontiguousarray(hi), np.ascontiguousarray(lo)

    wgates_hi, wgates_lo = split_bf16(wgates)
    bg_hi, bg_lo = split_bf16(b_gates[perm][None, :])
    bias_g2 = np.ascontiguousarray(np.concatenate([bg_hi, bg_lo], axis=0))  # [2, 2048]

    # state layout tiles
    h0t = np.zeros((128, 128), np.float32)     # h0t[p, q*32+b] = h0[b, 128q+p]
    c0t = np.zeros((128, 128), np.float32)     # c0t[32q+b, hw] = c0[b, 128q+hw]
    for q in range(4):
        h0t[:, 32 * q:32 * (q + 1)] = h0[:, 128 * q:128 * (q + 1)].T
        c0t[32 * q:32 * (q + 1), :] = c0[:, 128 * q:128 * (q + 1)]

    ident = np.eye(128, dtype=np.float32)
    ones2 = np.ones((2, 32), ml_dtypes.bfloat16)

    in_maps = []
    for j in range(N_CORES):
        shard = Wfc[VS * j:VS * (j + 1)]                    # [4000, 512]
        shard_p = np.zeros((VSP, H), np.float32)
        shard_p[:VS] = shard
        wfc_in = np.ascontiguousarray(shard_p.T)            # [512, 4096]
        wfc_hi, wfc_lo = split_bf16(wfc_in)
        bfc_p = np.full(VSP, -1e30, np.float32)
        bfc_p[:VS] = bfc[VS * j:VS * (j + 1)]
        bf_hi, bf_lo = split_bf16(bfc_p[None, :])
        bias_fc2 = np.ascontiguousarray(np.concatenate([bf_hi, bf_lo], axis=0))
        # gkey[p, nt] = BIG - (VS*j + 1024*(p//32) + 512*nt)
        gbase = VS * j + (np.arange(128) // 32) * 1024
        gkey = np.stack([BIG - gbase, BIG - gbase - 512], axis=1).astype(np.float32)
        in_maps.append({
            "wfc_hi": wfc_hi,
            "wfc_lo": wfc_lo,
            "wgates_hi": wgates_hi,
            "wgates_lo": wgates_lo,
            "bias_g2": bias_g2,
            "bias_fc2": bias_fc2,
            "gkey": np.ascontiguousarray(gkey),
            "ident": ident,
            "ones2": ones2,
            "h0t": h0t,
            "c0": c0t,
            "emb": embedding,
        })
    return in_maps


def declare_io(nc, n_steps):
    io = {}
    io["wfc_hi"] = nc.dram_tensor("wfc_hi", [512, VSP], BF16, kind="ExternalInput").ap()
    io["wfc_lo"] = nc.dram_tensor("wfc_lo", [512, VSP], BF16, kind="ExternalInput").ap()
    io["wgates_hi"] = nc.dram_tensor("wgates_hi", [1024, 2048], BF16, kind="ExternalInput").ap()
    io["wgates_lo"] = nc.dram_tensor("wgates_lo", [1024, 2048], BF16, kind="ExternalInput").ap()
    io["bias_g2"] = nc.dram_tensor("bias_g2", [2, 2048], BF16, kind="ExternalInput").ap()
    io["bias_fc2"] = nc.dram_tensor("bias_fc2", [2, VSP], BF16, kind="ExternalInput").ap()
    io["gkey"] = nc.dram_tensor("gkey", [128, 2], FP32, kind="ExternalInput").ap()
    io["ident"] = nc.dram_tensor("ident", [128, 128], FP32, kind="ExternalInput").ap()
    io["ones2"] = nc.dram_tensor("ones2", [2, 32], BF16, kind="ExternalInput").ap()
    io["h0t"] = nc.dram_tensor("h0t", [128, 128], FP32, kind="ExternalInput").ap()
    io["c0"] = nc.dram_tensor("c0", [128, 128], FP32, kind="ExternalInput").ap()
    io["emb"] = nc.dram_tensor("emb", [V, E], FP32, kind="ExternalInput").ap()
    io["logits"] = nc.dram_tensor("logits", [n_steps, 128, 1024], BF16,
                                  kind="ExternalOutput").ap()
    return io


_BUILT = {}


def build(n_steps=T):
    if n_steps in _BUILT:
        return _BUILT[n_steps]
    nc = bacc.Bacc("TRN2", target_bir_lowering=False, debug=False,
                   num_devices=N_CORES)
    io = declare_io(nc, n_steps)
    build_decoder(nc, io, n_steps)
    nc.compile()
    _BUILT[n_steps] = nc
    return nc


def assemble(results, n_steps=T):
    """results: list of per-core out dicts -> full [B, T, V] fp32."""
    full = np.empty((B, n_steps, V), np.float32)
    for j in range(N_CORES):
        arr = results[j]["logits"].astype(np.float32)
        arr = arr.reshape(n_steps, 4, 32, 1024)
        arr = arr.transpose(2, 0, 1, 3).reshape(B, n_steps, VSP)[:, :, :VS]
        full[:, :, VS * j:VS * (j + 1)] = arr
    return full


def kernel(**inputs):
    n_steps = int(inputs.get("context_length", T))
    assert n_steps == T, f"kernel hardcodes T={T}, got {n_steps}"
    nc = build(T)
    in_maps = host_prep(inputs)
    res = run_bass_kernel_spmd(nc, in_maps, core_ids=list(range(N_CORES)))
    return assemble(res.results, T)


if __name__ == "__main__":
    import reference
    inputs = reference.setup_inputs()
    out = kernel(**{k: np.asarray(v) if hasattr(v, "shape") else v
                    for k, v in inputs.items()})
    print("output shape:", out.shape)


# revision 8
# speedup vs baseline: 1.0112x; 1.0112x over previous
"""Trainium2 Bass kernel for nn_AutoregressiveDecoder (LSTM decoder w/ greedy sampling).

Strategy (8 NeuronCores, SPMD):
  - Vocab-shard the fc projection: core j holds Wfc rows [4000j, 4000(j+1)) padded to
    4096 (pad bias = -1e30), resident in SBUF.
  - LSTM weights replicated per core, SBUF-resident, gate columns permuted so that
    PE column-group q computes [i|f|g|o] for hidden quarter q -> full-partition
    elementwise state updates.
  - All matmuls bf16x3 (hi*hi + hi*lo + lo*hi, fp32 psum accumulate) with 4-way PE
    column tiling; fc bias folded into the matmul via a 2-row ones lhsT so the
    argmax reads finished logits straight out of PSUM.
  - Greedy token: per-half (512-wide) max8/max_index pipelined under the second
    half's matmul, 4 candidates/core AllGather'd ([16,32] payload), single
    multi-axis-reduce fold after the exchange, indirect-DMA embedding gather.
  - Filler matmuls (zero operands) span the AllGather window so the PE's HAM
    clock gate stays at full rate across the per-step collective stall.
  - Logits stream to DRAM as bf16 [T, 128, 1024] per core; host reassembles
    [B, T, V] in fp32 (output tolerance is 2e-2; bf16 staging halves the
    copy+DMA cost while the on-device argmax stays fp32-exact).
"""
import sys

sys.path.insert(0, "/opt/trn_rl_repo")

import numpy as np

import concourse.bass as bass
import concourse.bacc as bacc
import concourse.tile as tile
import concourse.mybir as mybir
from concourse.bass_utils import run_bass_kernel_spmd

FP32 = mybir.dt.float32
BF16 = mybir.dt.bfloat16
I32 = mybir.dt.int32
U32 = mybir.dt.uint32

N_CORES = 8
B, L, H, E, V, T = 32, 256, 512, 512, 32000, 64
VS = V // N_CORES          # 4000 true shard
VSP = 4096                 # padded shard
BIG = 65536.0
# fp32 filler matmuls (~1us each warm) spanning the three PE-idle windows of a
# step so the HAM clock gate never sees a >3.4us idle window and re-throttles:
# A: activations/state chain, B: AllGather wait, C: fold+gather+x-prep.
FILL_A, FILL_B, FILL_C = 2, 5, 3

Sigmoid = mybir.ActivationFunctionType.Sigmoid
Tanh = mybir.ActivationFunctionType.Tanh
Alu = mybir.AluOpType


def build_decoder(nc, io, n_steps):
    """Emit the full unrolled decoder. io: dict name -> DRAM AP."""
    with tile.TileContext(nc) as tc:
        sb = tc.alloc_tile_pool(name="sb", bufs=1)
        sb2 = tc.alloc_tile_pool(name="sb2", bufs=3)
        ps_g = tc.alloc_tile_pool(name="ps_g", bufs=2, space="PSUM")
        ps_v = tc.alloc_tile_pool(name="ps_v", bufs=2, space="PSUM")
        ps_s = tc.alloc_tile_pool(name="ps_s", bufs=2, space="PSUM")
        dr = tc.alloc_tile_pool(name="dr", bufs=4, space="DRAM")
        pools = [sb, sb2, ps_g, ps_v, ps_s, dr]

        # ---- persistent SBUF state & weights ----
        wfh = [sb.tile([128, VSP], BF16, name=f"wfh{q}") for q in range(4)]
        wfl = [sb.tile([128, VSP], BF16, name=f"wfl{q}") for q in range(4)]
        wgh = [sb.tile([128, 2048], BF16, name=f"wgh{r}") for r in range(8)]
        wgl = [sb.tile([128, 2048], BF16, name=f"wgl{r}") for r in range(8)]
        bias_g2 = sb.tile([2, 2048], BF16, name="bias_g2")
        bias_fc2 = sb.tile([2, VSP], BF16, name="bias_fc2")
        gkey = sb.tile([128, 2], FP32, name="gkey")   # BIG - (VS*j + 1024*g + 512*nt)
        ident = sb.tile([128, 128], FP32, name="ident")
        ones2 = sb.tile([2, 32], BF16, name="ones2")
        zeros = sb.tile([128, 512], FP32, name="zeros")
        xT = sb.tile([128, 128], FP32, name="xT")
        hT = sb.tile([128, 128], FP32, name="hT")
        xTh = sb.tile([128, 128], BF16, name="xTh")
        xTl = sb.tile([128, 128], BF16, name="xTl")
        hTh = sb.tile([128, 128], BF16, name="hTh")
        hTl = sb.tile([128, 128], BF16, name="hTl")
        c_t = sb.tile([128, 128], FP32, name="c_t")

        for q in range(4):
            nc.sync.dma_start(wfh[q][:], io["wfc_hi"][128 * q:128 * (q + 1), :])
            nc.sync.dma_start(wfl[q][:], io["wfc_lo"][128 * q:128 * (q + 1), :])
        for r in range(8):
            nc.sync.dma_start(wgh[r][:], io["wgates_hi"][128 * r:128 * (r + 1), :])
            nc.sync.dma_start(wgl[r][:], io["wgates_lo"][128 * r:128 * (r + 1), :])
        nc.sync.dma_start(bias_g2[:], io["bias_g2"][:])
        nc.sync.dma_start(bias_fc2[:], io["bias_fc2"][:])
        nc.sync.dma_start(gkey[:], io["gkey"][:])
        nc.sync.dma_start(ident[:], io["ident"][:])
        nc.sync.dma_start(ones2[:], io["ones2"][:])
        nc.sync.dma_start(xT[:], io["h0t"][:])
        nc.sync.dma_start(hT[:], io["h0t"][:])
        nc.sync.dma_start(c_t[:], io["c0"][:])
        nc.vector.memset(zeros[:], 0.0)
        # initial hi/lo splits of the (identical) x0 = h0 state
        nc.vector.tensor_copy(hTh[:], hT[:])
        nc.vector.tensor_tensor(hTl[:], hT[:], hTh[:], op=Alu.subtract)
        nc.vector.tensor_copy(xTh[:], hTh[:])
        nc.vector.tensor_copy(xTl[:], hTl[:])

        emb = io["emb"]
        out_logits = io["logits"]  # [T, 128, 1024] bf16

        # ---- gates matmul emission helpers ----
        # psum layout: partition 32q+b, free = gate*128+hw (cols permuted on host)
        def emit_gates_bias_h(pg):
            for g in range(4):
                nc.tensor.matmul(
                    pg[32 * g:32 * (g + 1), :], lhsT=ones2[:, :],
                    rhs=bias_g2[:, 512 * g:512 * (g + 1)],
                    start=True, stop=False, tile_position=(0, 32 * g),
                    skip_group_check=True,
                )
            emit_gates_rounds(pg, [4, 5, 6, 7], stop=False)

        def emit_gates_rounds(pg, rounds, stop):
            for r in rounds:
                hi, lo = (xTh, xTl) if r < 4 else (hTh, hTl)
                q = r % 4
                cs = slice(32 * q, 32 * (q + 1))
                for g in range(4):
                    gs = slice(512 * g, 512 * (g + 1))
                    out = pg[32 * g:32 * (g + 1), :]
                    passes = ((hi[:, cs], wgh[r][:, gs]),
                              (lo[:, cs], wgh[r][:, gs]),
                              (hi[:, cs], wgl[r][:, gs]))
                    for pi, (lhsT, rhs) in enumerate(passes):
                        nc.tensor.matmul(
                            out, lhsT=lhsT, rhs=rhs,
                            start=False,
                            stop=(stop and r == rounds[-1] and pi == 2),
                            tile_position=(0, 32 * g),
                            skip_group_check=True,
                        )

        def emit_fillers(n):
            # fp32 matmuls over zeros: 2048 cycles each (~1us warm) of PE
            # activity with minimal switching power; results never read.
            p_fil = ps_s.tile([128, 512], FP32, name="p_fil", tag="small")
            for _ in range(n):
                nc.tensor.matmul(
                    p_fil[:], lhsT=zeros[:, 0:128], rhs=zeros[:],
                    start=True, stop=True, skip_group_check=True,
                )

        # step-0 gates: bias + h-rounds up front (x == h0 so all 8 rounds)
        pg = ps_g.tile([128, 512], FP32, name="pg", tag="pg")
        emit_gates_bias_h(pg)

        for t in range(n_steps):
            last = t == n_steps - 1
            # ================= gates matmul: x-rounds =================
            emit_gates_rounds(pg, [0, 1, 2, 3], stop=True)
            emit_fillers(FILL_A)

            # ================= activations / state =================
            # gate slots after host permutation: [i | f | o | g(tanh)]
            acts = sb2.tile([128, 512], FP32, name="acts", tag="acts")
            nc.scalar.activation(acts[:, 0:256], pg[:, 0:256], Sigmoid)
            nc.scalar.activation(acts[:, 384:512], pg[:, 384:512], Tanh)
            nc.scalar.activation(acts[:, 256:384], pg[:, 256:384], Sigmoid)
            nc.vector.tensor_tensor(c_t[:], acts[:, 128:256], c_t[:], op=Alu.mult)
            t1 = sb2.tile([128, 128], FP32, name="t1", tag="t1")
            nc.vector.tensor_tensor(t1[:], acts[:, 0:128], acts[:, 384:512], op=Alu.mult)
            nc.vector.tensor_tensor(c_t[:], c_t[:], t1[:], op=Alu.add)
            tanh_c = sb2.tile([128, 128], FP32, name="tanh_c", tag="tanh_c")
            nc.scalar.activation(tanh_c[:], c_t[:], Tanh)
            h_new = sb2.tile([128, 128], FP32, name="h_new", tag="h_new")
            nc.vector.tensor_tensor(h_new[:], acts[:, 256:384], tanh_c[:], op=Alu.mult)

            # hT = transpose(h_new); hi cast on ACT in parallel with fp32 copy on DVE
            p_ht = ps_s.tile([128, 128], FP32, name="p_ht", tag="small")
            nc.tensor.transpose(p_ht[:], h_new[:], ident[:])
            nc.scalar.copy(hTh[:], p_ht[:])
            nc.vector.tensor_copy(hT[:], p_ht[:])
            nc.vector.tensor_tensor(hTl[:], hT[:], hTh[:], op=Alu.subtract)

            # ================= vocab matmul (bias folded in) =================
            # psum layout: partition 32g+b (g = vocab quarter of shard), free 1024
            pv = ps_v.tile([128, 1024], FP32, name="pv", tag="pv")
            staged = sb2.tile([128, 1024], BF16, name="staged", tag="staged")
            v8 = [None, None]
            i8 = [None, None]
            for nt in range(2):
                for g in range(4):
                    ws = slice(1024 * g + 512 * nt, 1024 * g + 512 * (nt + 1))
                    nc.tensor.matmul(
                        pv[32 * g:32 * (g + 1), 512 * nt:512 * (nt + 1)],
                        lhsT=ones2[:, :], rhs=bias_fc2[:, ws],
                        start=True, stop=False, tile_position=(0, 32 * g),
                        skip_group_check=True,
                    )
                for q in range(4):
                    cs = slice(32 * q, 32 * (q + 1))
                    for g in range(4):
                        ws = slice(1024 * g + 512 * nt, 1024 * g + 512 * (nt + 1))
                        out = pv[32 * g:32 * (g + 1), 512 * nt:512 * (nt + 1)]
                        passes = ((hTh[:, cs], wfh[q][:, ws]),
                                  (hTl[:, cs], wfh[q][:, ws]),
                                  (hTh[:, cs], wfl[q][:, ws]))
                        for pi, (lhsT, rhs) in enumerate(passes):
                            nc.tensor.matmul(
                                out, lhsT=lhsT, rhs=rhs,
                                start=False,
                                stop=(q == 3 and pi == 2),
                                tile_position=(0, 32 * g),
                                skip_group_check=True,
                            )
                # stage this half to DRAM (bf16) and find its candidate; the
                # nt=0 chain runs on ACT/DVE under the nt=1 matmul.
                # pay rows (per psum partition 32g+b): [v_a, v_b, key_a, key_b]
                # where key = BIG - global_idx (so keys never collide with
                # logit values in the eq-fold, and max(key) = min global idx).
                half = slice(512 * nt, 512 * (nt + 1))
                nc.scalar.copy(staged[:, half], pv[:, half])
                if not last:
                    if nt == 0:
                        pay = sb2.tile([128, 4], FP32, name="pay", tag="pay")
                        iloc = sb2.tile([128, 2], FP32, name="iloc", tag="iloc")
                    v8[nt] = sb2.tile([128, 8], FP32, name=f"v8{nt}", tag=f"v8{nt}")
                    i8[nt] = sb2.tile([128, 8], U32, name=f"i8{nt}", tag=f"i8{nt}")
                    nc.vector.max(v8[nt][:], pv[:, half])
                    nc.vector.max_index(i8[nt][:], v8[nt][:], pv[:, half])
                    nc.vector.tensor_copy(pay[:, nt:nt + 1], v8[nt][:, 0:1])
                    nc.vector.tensor_copy(iloc[:, nt:nt + 1], i8[nt][:, 0:1])
                    nc.vector.tensor_scalar(
                        pay[:, 2 + nt:3 + nt], iloc[:, nt:nt + 1],
                        -1.0, gkey[:, nt:nt + 1], op0=Alu.mult, op1=Alu.add)
            nc.scalar.dma_start(out_logits[t], staged[:])

            if not last:
                # transpose candidates -> [4, 128] and ship [16, 32] to the AG
                p_pa = ps_s.tile([4, 128], FP32, name="p_pa", tag="small")
                nc.tensor.transpose(p_pa[:], pay[:], ident[:])
                payT = sb2.tile([4, 128], FP32, name="payT", tag="payT")
                nc.scalar.copy(payT[:], p_pa[:])

                cc_in = dr.tile([16, 32], FP32, name="cc_in", tag="cc_in")
                cc_out = dr.tile([128, 32], FP32, name="cc_out", tag="cc_out",
                                 addr_space="Shared")
                # cc_in row = g*4 + f  <-  payT row f, free g*32+b
                nc.sync.dma_start(
                    cc_in[:].rearrange("(g f) b -> f g b", g=4, f=4),
                    payT[:].rearrange("f (g b) -> f g b", g=4))
                nc.gpsimd.collective_compute(
                    "AllGather", Alu.bypass,
                    replica_groups=[list(range(N_CORES))],
                    ins=[cc_in[:]], outs=[cc_out[:]],
                )

            # ================= next-step gates: bias + h rounds =================
            if not last:
                pg = ps_g.tile([128, 512], FP32, name="pg", tag="pg")
                emit_gates_bias_h(pg)
                emit_fillers(FILL_B)

                # ================= AG result: fold over 64 candidates ==========
                agb = sb2.tile([128, 32], FP32, name="agb", tag="agb")
                nc.sync.dma_start(agb[:], cc_out[:])
                p_ag = ps_s.tile([32, 128], FP32, name="p_ag", tag="small")
                nc.tensor.transpose(p_ag[:], agb[:], ident[:])
                emit_fillers(FILL_C)
                t32 = sb2.tile([32, 128], FP32, name="t32", tag="t32")
                nc.vector.tensor_copy(t32[:], p_ag[:])

                # col = r*16 + g*4 + f; candidate order (r, g, f) is global-idx
                # order, and key = BIG - gidx makes max pick the first occurrence.
                tv = t32[:].rearrange("p (r g f) -> p r g f", r=8, g=4, f=4)
                vals = tv[:, :, :, 0:2]
                keys = tv[:, :, :, 2:4]
                gv32 = sb2.tile([32, 1], FP32, name="gv32", tag="gv32")
                nc.vector.tensor_reduce(gv32[:], vals, axis=mybir.AxisListType.XYZ,
                                        op=Alu.max)
                eqt = sb2.tile([32, 64], FP32, name="eqt", tag="eqt")
                eqv = eqt[:].rearrange("p (r g f) -> p r g f", r=8, g=4, f=2)
                nc.vector.tensor_scalar(eqv, vals, gv32[:, 0:1], None,
                                        op0=Alu.is_equal)
                mselt = sb2.tile([32, 64], FP32, name="mselt", tag="mselt")
                mselv = mselt[:].rearrange("p (r g f) -> p r g f", r=8, g=4, f=2)
                nc.vector.tensor_tensor(mselv, eqv, keys, op=Alu.mult)
                m2r = sb2.tile([32, 1], FP32, name="m2r", tag="m2r")
                nc.vector.tensor_reduce(m2r[:], mselv, axis=mybir.AxisListType.XYZ,
                                        op=Alu.max)
                idxf = sb2.tile([32, 1], FP32, name="idxf", tag="idxf")
                nc.vector.tensor_scalar(idxf[:], m2r[:], -1.0, BIG,
                                        op0=Alu.mult, op1=Alu.add)
                idx32 = sb2.tile([32, 1], I32, name="idx32", tag="idx32")
                nc.vector.tensor_copy(idx32[:], idxf[:])

                # ================= embedding gather + transpose =================
                x_rows = sb2.tile([32, 512], FP32, name="x_rows", tag="x_rows")
                nc.gpsimd.indirect_dma_start(
                    out=x_rows[:], out_offset=None, in_=emb[:],
                    in_offset=bass.IndirectOffsetOnAxis(ap=idx32[:, 0:1], axis=0),
                )
                p_x = ps_s.tile([128, 128], FP32, name="p_x", tag="small")
                for q in range(4):
                    nc.tensor.transpose(
                        p_x[:, 32 * q:32 * (q + 1)],
                        x_rows[:, 128 * q:128 * (q + 1)], ident[0:32, 0:32])
                nc.scalar.copy(xTh[:], p_x[:])
                nc.vector.tensor_copy(xT[:], p_x[:])
                nc.vector.tensor_tensor(xTl[:], xT[:], xTh[:], op=Alu.subtract)

        for p in reversed(pools):
            p.release()


def host_prep(inputs):
    """Build per-core in_maps from the full problem inputs."""
    z = np.asarray(inputs["z"], np.float32)
    embedding = np.ascontiguousarray(np.asarray(inputs["embedding"], np.float32))
    Wh = np.asarray(inputs["Wh"], np.float32)
    bh = np.asarray(inputs["bh"], np.float32)
    Wc = np.asarray(inputs["Wc"], np.float32)
    bc = np.asarray(inputs["bc"], np.float32)
    Wih = np.asarray(inputs["Wih"], np.float32)
    Whh = np.asarray(inputs["Whh"], np.float32)
    bih = np.asarray(inputs["bih"], np.float32)
    bhh = np.asarray(inputs["bhh"], np.float32)
    Wfc = np.asarray(inputs["Wfc"], np.float32)
    bfc = np.asarray(inputs["bfc"], np.float32)

    h0 = (z @ Wh.T + bh).astype(np.float32)   # [B, H]
    c0 = (z @ Wc.T + bc).astype(np.float32)
    b_gates = (bih + bhh).astype(np.float32)  # [4H]

    # gate column permutation: c' = q*512 + slot*128 + hw with slot order
    # [i, f, o, g] so the sigmoid gates are one contiguous 384-wide range.
    cp = np.arange(2048)
    qq, rem = cp // 512, cp % 512
    slot, hw = rem // 128, rem % 128
    gate = np.array([0, 1, 3, 2])[slot]        # slot -> original gate (i,f,o,g)
    perm = gate * 512 + qq * 128 + hw          # original col index for permuted col c'
    Wall = np.concatenate([Wih, Whh], axis=1)  # [2048, 1024] (k = [x | h])
    Wperm = Wall[perm]                         # [2048, 1024]
    wgates = np.ascontiguousarray(Wperm.T)     # [1024, 2048]

    import ml_dtypes

    def split_bf16(w):
        hi = w.astype(ml_dtypes.bfloat16)
        lo = (w - hi.astype(np.float32)).astype(ml_dtypes.bfloat16)
        return np.ascontiguousarray(hi), np.ascontiguousarray(lo)

    wgates_hi, wgates_lo = split_bf16(wgates)
    bg_hi, bg_lo = split_bf16(b_gates[perm][None, :])
    bias_g2 = np.ascontiguousarray(np.concatenate([bg_hi, bg_lo], axis=0))  # [2, 2048]

    # state layout tiles
    h0t = np.zeros((128, 128), np.float32)     # h0t[p, q*32+b] = h0[b, 128q+p]
    c0t = np.zeros((128, 128), np.float32)     # c0t[32q+b, hw] = c0[b, 128q+hw]
    for q in range(4):
        h0t[:, 32 * q:32 * (q + 1)] = h0[:, 128 * q:128 * (q + 1)].T
        c0t[32 * q:32 * (q + 1), :] = c0[:, 128 * q:128 * (q + 1)]

    ident = np.eye(128, dtype=np.float32)
    ones2 = np.ones((2, 32), ml_dtypes.bfloat16)

    in_maps = []
    for j in range(N_CORES):
        shard = Wfc[VS * j:VS * (j + 1)]                    # [4000, 512]
        shard_p = np.zeros((VSP, H), np.float32)
        shard_p[:VS] = shard
        wfc_in = np.ascontiguousarray(shard_p.T)            # [512, 4096]
        wfc_hi, wfc_lo = split_bf16(wfc_in)
        bfc_p = np.full(VSP, -1e30, np.float32)
        bfc_p[:VS] = bfc[VS * j:VS * (j + 1)]
        bf_hi, bf_lo = split_bf16(bfc_p[None, :])
        bias_fc2 = np.ascontiguousarray(np.concatenate([bf_hi, bf_lo], axis=0))
        # gkey[p, nt] = BIG - (VS*j + 1024*(p//32) + 512*nt)
        gbase = VS * j + (np.arange(128) // 32) * 1024
        gkey = np.stack([BIG - gbase, BIG - gbase - 512], axis=1).astype(np.float32)
        in_maps.append({
            "wfc_hi": wfc_hi,
            "wfc_lo": wfc_lo,
            "wgates_hi": wgates_hi,
            "wgates_lo": wgates_lo,
            "bias_g2": bias_g2,
            "bias_fc2": bias_fc2,
            "gkey": np.ascontiguousarray(gkey),
            "ident": ident,
            "ones2": ones2,
            "h0t": h0t,
            "c0": c0t,
            "emb": embedding,
        })
    return in_maps


def declare_io(nc, n_steps):
    io = {}
    io["wfc_hi"] = nc.dram_tensor("wfc_hi", [512, VSP], BF16, kind="ExternalInput").ap()
    io["wfc_lo"] = nc.dram_tensor("wfc_lo", [512, VSP], BF16, kind="ExternalInput").ap()
    io["wgates_hi"] = nc.dram_tensor("wgates_hi", [1024, 2048], BF16, kind="ExternalInput").ap()
    io["wgates_lo"] = nc.dram_tensor("wgates_lo", [1024, 2048], BF16, kind="ExternalInput").ap()
    io["bias_g2"] = nc.dram_tensor("bias_g2", [2, 2048], BF16, kind="ExternalInput").ap()
    io["bias_fc2"] = nc.dram_tensor("bias_fc2", [2, VSP], BF16, kind="ExternalInput").ap()
    io["gkey"] = nc.dram_tensor("gkey", [128, 2], FP32, kind="ExternalInput").ap()
    io["ident"] = nc.dram_tensor("ident", [128, 128], FP32, kind="ExternalInput").ap()
    io["ones2"] = nc.dram_tensor("ones2", [2, 32], BF16, kind="ExternalInput").ap()
    io["h0t"] = nc.dram_tensor("h0t", [128, 128], FP32, kind="ExternalInput").ap()
    io["c0"] = nc.dram_tensor("c0", [128, 128], FP32, kind="ExternalInput").ap()
    io["emb"] = nc.dram_tensor("emb", [V, E], FP32, kind="ExternalInput").ap()
    io["logits"] = nc.dram_tensor("logits", [n_steps, 128, 1024], BF16,
                                  kind="ExternalOutput").ap()
    return io


_BUILT = {}


def build(n_steps=T):
    if n_steps in _BUILT:
        return _BUILT[n_steps]
    nc = bacc.Bacc("TRN2", target_bir_lowering=False, debug=False,
                   num_devices=N_CORES)
    io = declare_io(nc, n_steps)
    build_decoder(nc, io, n_steps)
    nc.compile()
    _BUILT[n_steps] = nc
    return nc


def assemble(results, n_steps=T):
    """results: list of per-core out dicts -> full [B, T, V] fp32."""
    full = np.empty((B, n_steps, V), np.float32)
    for j in range(N_CORES):
        arr = results[j]["logits"].astype(np.float32)
        arr = arr.reshape(n_steps, 4, 32, 1024)
        arr = arr.transpose(2, 0, 1, 3).reshape(B, n_steps, VSP)[:, :, :VS]
        full[:, :, VS * j:VS * (j + 1)] = arr
    return full


def kernel(**inputs):
    n_steps = int(inputs.get("context_length", T))
    assert n_steps == T, f"kernel hardcodes T={T}, got {n_steps}"
    nc = build(T)
    in_maps = host_prep(inputs)
    res = run_bass_kernel_spmd(nc, in_maps, core_ids=list(range(N_CORES)))
    return assemble(res.results, T)


if __name__ == "__main__":
    import reference
    inputs = reference.setup_inputs()
    out = kernel(**{k: np.asarray(v) if hasattr(v, "shape") else v
                    for k, v in inputs.items()})
    print("output shape:", out.shape)


# revision 10
# speedup vs baseline: 1.0436x; 1.0321x over previous
"""Trainium2 Bass kernel for nn_AutoregressiveDecoder (LSTM decoder w/ greedy sampling).

Strategy (8 NeuronCores, SPMD):
  - Vocab-shard the fc projection: core j holds Wfc rows [4000j, 4000(j+1)) padded to
    4096 (pad bias = -1e30), resident in SBUF.
  - LSTM weights replicated per core, SBUF-resident, gate columns permuted so that
    PE column-group q computes [i|f|g|o] for hidden quarter q -> full-partition
    elementwise state updates.
  - All matmuls bf16x3 (hi*hi + hi*lo + lo*hi, fp32 psum accumulate) with 4-way PE
    column tiling; fc bias folded into the matmul via a 2-row ones lhsT so the
    argmax reads finished logits straight out of PSUM.
  - Greedy token: per-half (512-wide) max8/max_index pipelined under the second
    half's matmul, 4 candidates/core AllGather'd ([16,32] payload), single
    multi-axis-reduce fold after the exchange, indirect-DMA embedding gather.
  - Filler matmuls (zero operands) span the AllGather window so the PE's HAM
    clock gate stays at full rate across the per-step collective stall.
  - Logits stream to DRAM as bf16 [T, 128, 1024] per core; host reassembles
    [B, T, V] in fp32 (output tolerance is 2e-2; bf16 staging halves the
    copy+DMA cost while the on-device argmax stays fp32-exact).
"""
import sys

sys.path.insert(0, "/opt/trn_rl_repo")

import numpy as np

import concourse.bass as bass
import concourse.bacc as bacc
import concourse.tile as tile
import concourse.mybir as mybir
from concourse.bass_utils import run_bass_kernel_spmd

FP32 = mybir.dt.float32
BF16 = mybir.dt.bfloat16
I32 = mybir.dt.int32
U32 = mybir.dt.uint32

N_CORES = 8
B, L, H, E, V, T = 32, 256, 512, 512, 32000, 64
VS = V // N_CORES          # 4000 true shard
VSP = 4096                 # padded shard
BIG = 65536.0
# bf16 filler matmuls (~215-430ns each) spanning the PE-idle windows of a step
# so the HAM clock gate never sees a low-duty window and re-throttles:
# A: activations/state chain, P: argmax tail before the pay transpose,
# B: AllGather wait, C: fold+gather+x-prep.
FILL_A, FILL_P, FILL_B, FILL_C = 10, 4, 18, 14

Sigmoid = mybir.ActivationFunctionType.Sigmoid
Tanh = mybir.ActivationFunctionType.Tanh
Alu = mybir.AluOpType


def build_decoder(nc, io, n_steps):
    """Emit the full unrolled decoder. io: dict name -> DRAM AP."""
    with tile.TileContext(nc) as tc:
        sb = tc.alloc_tile_pool(name="sb", bufs=1)
        sb2 = tc.alloc_tile_pool(name="sb2", bufs=3)
        ps_g = tc.alloc_tile_pool(name="ps_g", bufs=2, space="PSUM")
        ps_v = tc.alloc_tile_pool(name="ps_v", bufs=2, space="PSUM")
        ps_s = tc.alloc_tile_pool(name="ps_s", bufs=2, space="PSUM")
        dr = tc.alloc_tile_pool(name="dr", bufs=4, space="DRAM")
        pools = [sb, sb2, ps_g, ps_v, ps_s, dr]

        # ---- persistent SBUF state & weights ----
        wfh = [sb.tile([128, VSP], BF16, name=f"wfh{q}") for q in range(4)]
        wfl = [sb.tile([128, VSP], BF16, name=f"wfl{q}") for q in range(4)]
        wgh = [sb.tile([128, 2048], BF16, name=f"wgh{r}") for r in range(8)]
        wgl = [sb.tile([128, 2048], BF16, name=f"wgl{r}") for r in range(8)]
        bias_g2 = sb.tile([2, 2048], BF16, name="bias_g2")
        bias_fc2 = sb.tile([2, VSP], BF16, name="bias_fc2")
        gkey = sb.tile([128, 2], FP32, name="gkey")   # BIG - (VS*j + 1024*g + 512*nt)
        ident = sb.tile([128, 128], FP32, name="ident")
        ones2 = sb.tile([2, 32], BF16, name="ones2")
        zeros = sb.tile([128, 512], BF16, name="zeros")
        xT = sb.tile([128, 128], FP32, name="xT")
        hT = sb.tile([128, 128], FP32, name="hT")
        xTh = sb.tile([128, 128], BF16, name="xTh")
        xTl = sb.tile([128, 128], BF16, name="xTl")
        hTh = sb.tile([128, 128], BF16, name="hTh")
        hTl = sb.tile([128, 128], BF16, name="hTl")
        c_t = sb.tile([128, 128], FP32, name="c_t")

        for q in range(4):
            nc.sync.dma_start(wfh[q][:], io["wfc_hi"][128 * q:128 * (q + 1), :])
            nc.sync.dma_start(wfl[q][:], io["wfc_lo"][128 * q:128 * (q + 1), :])
        for r in range(8):
            nc.sync.dma_start(wgh[r][:], io["wgates_hi"][128 * r:128 * (r + 1), :])
            nc.sync.dma_start(wgl[r][:], io["wgates_lo"][128 * r:128 * (r + 1), :])
        nc.sync.dma_start(bias_g2[:], io["bias_g2"][:])
        nc.sync.dma_start(bias_fc2[:], io["bias_fc2"][:])
        nc.sync.dma_start(gkey[:], io["gkey"][:])
        nc.sync.dma_start(ident[:], io["ident"][:])
        nc.sync.dma_start(ones2[:], io["ones2"][:])
        nc.sync.dma_start(xT[:], io["h0t"][:])
        nc.sync.dma_start(hT[:], io["h0t"][:])
        nc.sync.dma_start(c_t[:], io["c0"][:])
        nc.vector.memset(zeros[:], 0.0)
        # initial hi/lo splits of the (identical) x0 = h0 state
        nc.vector.tensor_copy(hTh[:], hT[:])
        nc.vector.tensor_tensor(hTl[:], hT[:], hTh[:], op=Alu.subtract)
        nc.vector.tensor_copy(xTh[:], hTh[:])
        nc.vector.tensor_copy(xTl[:], hTl[:])

        emb = io["emb"]
        out_logits = io["logits"]  # [T, 128, 1024] bf16

        # ---- gates matmul emission helpers ----
        # psum layout: partition 32q+b, free = gate*128+hw (cols permuted on host)
        def emit_gates_bias_h(pg):
            for g in range(4):
                nc.tensor.matmul(
                    pg[32 * g:32 * (g + 1), :], lhsT=ones2[:, :],
                    rhs=bias_g2[:, 512 * g:512 * (g + 1)],
                    start=True, stop=False, tile_position=(0, 32 * g),
                    skip_group_check=True,
                )
            emit_gates_rounds(pg, [4, 5, 6, 7], stop=False)

        def emit_gates_rounds(pg, rounds, stop):
            for r in rounds:
                hi, lo = (xTh, xTl) if r < 4 else (hTh, hTl)
                q = r % 4
                cs = slice(32 * q, 32 * (q + 1))
                for g in range(4):
                    gs = slice(512 * g, 512 * (g + 1))
                    out = pg[32 * g:32 * (g + 1), :]
                    passes = ((hi[:, cs], wgh[r][:, gs]),
                              (lo[:, cs], wgh[r][:, gs]),
                              (hi[:, cs], wgl[r][:, gs]))
                    for pi, (lhsT, rhs) in enumerate(passes):
                        nc.tensor.matmul(
                            out, lhsT=lhsT, rhs=rhs,
                            start=False,
                            stop=(stop and r == rounds[-1] and pi == 2),
                            tile_position=(0, 32 * g),
                            skip_group_check=True,
                        )

        def emit_fillers(n):
            # bf16 matmuls over zeros: 512 cycles each of PE activity with
            # minimal switching power; results never read.
            p_fil = ps_s.tile([128, 512], FP32, name="p_fil", tag="small")
            for _ in range(n):
                nc.tensor.matmul(
                    p_fil[:], lhsT=zeros[:, 0:128], rhs=zeros[:],
                    start=True, stop=True, skip_group_check=True,
                )

        # step-0 gates: bias + h-rounds up front (x == h0 so all 8 rounds)
        pg = ps_g.tile([128, 512], FP32, name="pg", tag="pg")
        emit_gates_bias_h(pg)

        for t in range(n_steps):
            last = t == n_steps - 1
            # ================= gates matmul: x-rounds =================
            emit_gates_rounds(pg, [0, 1, 2, 3], stop=True)
            emit_fillers(FILL_A)

            # ================= activations / state =================
            # gate slots after host permutation: [i | f | o | g(tanh)]
            acts = sb2.tile([128, 512], FP32, name="acts", tag="acts")
            nc.scalar.activation(acts[:, 0:256], pg[:, 0:256], Sigmoid)
            nc.scalar.activation(acts[:, 384:512], pg[:, 384:512], Tanh)
            nc.scalar.activation(acts[:, 256:384], pg[:, 256:384], Sigmoid)
            nc.vector.tensor_tensor(c_t[:], acts[:, 128:256], c_t[:], op=Alu.mult)
            t1 = sb2.tile([128, 128], FP32, name="t1", tag="t1")
            nc.vector.tensor_tensor(t1[:], acts[:, 0:128], acts[:, 384:512], op=Alu.mult)
            nc.vector.tensor_tensor(c_t[:], c_t[:], t1[:], op=Alu.add)
            tanh_c = sb2.tile([128, 128], FP32, name="tanh_c", tag="tanh_c")
            nc.scalar.activation(tanh_c[:], c_t[:], Tanh)
            h_new = sb2.tile([128, 128], FP32, name="h_new", tag="h_new")
            nc.vector.tensor_tensor(h_new[:], acts[:, 256:384], tanh_c[:], op=Alu.mult)

            # hT = transpose(h_new); hi cast on ACT in parallel with fp32 copy on DVE
            p_ht = ps_s.tile([128, 128], FP32, name="p_ht", tag="small")
            nc.tensor.transpose(p_ht[:], h_new[:], ident[:])
            nc.scalar.copy(hTh[:], p_ht[:])
            nc.vector.tensor_copy(hT[:], p_ht[:])
            nc.vector.tensor_tensor(hTl[:], hT[:], hTh[:], op=Alu.subtract)

            # ================= vocab matmul (bias folded in) =================
            # psum layout: partition 32g+b (g = vocab quarter of shard), free 1024
            pv = ps_v.tile([128, 1024], FP32, name="pv", tag="pv")
            staged = sb2.tile([128, 1024], BF16, name="staged", tag="staged")
            v8 = [None, None]
            i8 = [None, None]
            for nt in range(2):
                for g in range(4):
                    ws = slice(1024 * g + 512 * nt, 1024 * g + 512 * (nt + 1))
                    nc.tensor.matmul(
                        pv[32 * g:32 * (g + 1), 512 * nt:512 * (nt + 1)],
                        lhsT=ones2[:, :], rhs=bias_fc2[:, ws],
                        start=True, stop=False, tile_position=(0, 32 * g),
                        skip_group_check=True,
                    )
                for q in range(4):
                    cs = slice(32 * q, 32 * (q + 1))
                    for g in range(4):
                        ws = slice(1024 * g + 512 * nt, 1024 * g + 512 * (nt + 1))
                        out = pv[32 * g:32 * (g + 1), 512 * nt:512 * (nt + 1)]
                        passes = ((hTh[:, cs], wfh[q][:, ws]),
                                  (hTl[:, cs], wfh[q][:, ws]),
                                  (hTh[:, cs], wfl[q][:, ws]))
                        for pi, (lhsT, rhs) in enumerate(passes):
                            nc.tensor.matmul(
                                out, lhsT=lhsT, rhs=rhs,
                                start=False,
                                stop=(q == 3 and pi == 2),
                                tile_position=(0, 32 * g),
                                skip_group_check=True,
                            )
                # stage this half to DRAM (bf16) and find its candidate; the
                # nt=0 chain runs on ACT/DVE under the nt=1 matmul.
                # pay rows (per psum partition 32g+b): [v_a, v_b, key_a, key_b]
                # where key = BIG - global_idx (so keys never collide with
                # logit values in the eq-fold, and max(key) = min global idx).
                half = slice(512 * nt, 512 * (nt + 1))
                nc.scalar.copy(staged[:, half], pv[:, half])
                if not last:
                    if nt == 0:
                        pay = sb2.tile([128, 4], FP32, name="pay", tag="pay")
                        iloc = sb2.tile([128, 2], FP32, name="iloc", tag="iloc")
                    v8[nt] = sb2.tile([128, 8], FP32, name=f"v8{nt}", tag=f"v8{nt}")
                    i8[nt] = sb2.tile([128, 8], U32, name=f"i8{nt}", tag=f"i8{nt}")
                    nc.vector.max(v8[nt][:], pv[:, half])
                    nc.vector.max_index(i8[nt][:], v8[nt][:], pv[:, half])
                    nc.vector.tensor_copy(pay[:, nt:nt + 1], v8[nt][:, 0:1])
                    nc.vector.tensor_copy(iloc[:, nt:nt + 1], i8[nt][:, 0:1])
                    nc.vector.tensor_scalar(
                        pay[:, 2 + nt:3 + nt], iloc[:, nt:nt + 1],
                        -1.0, gkey[:, nt:nt + 1], op0=Alu.mult, op1=Alu.add)
            nc.scalar.dma_start(out_logits[t], staged[:])

            if not last:
                emit_fillers(FILL_P)
                # transpose candidates -> [4, 128] and ship [16, 32] to the AG
                p_pa = ps_s.tile([4, 128], FP32, name="p_pa", tag="small")
                nc.tensor.transpose(p_pa[:], pay[:], ident[:])
                payT = sb2.tile([4, 128], FP32, name="payT", tag="payT")
                nc.scalar.copy(payT[:], p_pa[:])

                cc_in = dr.tile([16, 32], FP32, name="cc_in", tag="cc_in")
                cc_out = dr.tile([128, 32], FP32, name="cc_out", tag="cc_out",
                                 addr_space="Shared")
                # cc_in row = g*4 + f  <-  payT row f, free g*32+b
                nc.sync.dma_start(
                    cc_in[:].rearrange("(g f) b -> f g b", g=4, f=4),
                    payT[:].rearrange("f (g b) -> f g b", g=4))
                nc.gpsimd.collective_compute(
                    "AllGather", Alu.bypass,
                    replica_groups=[list(range(N_CORES))],
                    ins=[cc_in[:]], outs=[cc_out[:]],
                )

            # ================= next-step gates: bias + h rounds =================
            if not last:
                pg = ps_g.tile([128, 512], FP32, name="pg", tag="pg")
                emit_gates_bias_h(pg)
                emit_fillers(FILL_B)

                # ================= AG result: fold over 64 candidates ==========
                agb = sb2.tile([128, 32], FP32, name="agb", tag="agb")
                nc.sync.dma_start(agb[:], cc_out[:])
                p_ag = ps_s.tile([32, 128], FP32, name="p_ag", tag="small")
                nc.tensor.transpose(p_ag[:], agb[:], ident[:])
                emit_fillers(FILL_C)
                t32 = sb2.tile([32, 128], FP32, name="t32", tag="t32")
                nc.vector.tensor_copy(t32[:], p_ag[:])

                # col = r*16 + g*4 + f; candidate order (r, g, f) is global-idx
                # order, and key = BIG - gidx makes max pick the first occurrence.
                tv = t32[:].rearrange("p (r g f) -> p r g f", r=8, g=4, f=4)
                vals = tv[:, :, :, 0:2]
                keys = tv[:, :, :, 2:4]
                gv32 = sb2.tile([32, 1], FP32, name="gv32", tag="gv32")
                nc.vector.tensor_reduce(gv32[:], vals, axis=mybir.AxisListType.XYZ,
                                        op=Alu.max)
                eqt = sb2.tile([32, 64], FP32, name="eqt", tag="eqt")
                eqv = eqt[:].rearrange("p (r g f) -> p r g f", r=8, g=4, f=2)
                nc.vector.tensor_scalar(eqv, vals, gv32[:, 0:1], None,
                                        op0=Alu.is_equal)
                mselt = sb2.tile([32, 64], FP32, name="mselt", tag="mselt")
                mselv = mselt[:].rearrange("p (r g f) -> p r g f", r=8, g=4, f=2)
                nc.vector.tensor_tensor(mselv, eqv, keys, op=Alu.mult)
                m2r = sb2.tile([32, 1], FP32, name="m2r", tag="m2r")
                nc.vector.tensor_reduce(m2r[:], mselv, axis=mybir.AxisListType.XYZ,
                                        op=Alu.max)
                idxf = sb2.tile([32, 1], FP32, name="idxf", tag="idxf")
                nc.vector.tensor_scalar(idxf[:], m2r[:], -1.0, BIG,
                                        op0=Alu.mult, op1=Alu.add)
                idx32 = sb2.tile([32, 1], I32, name="idx32", tag="idx32")
                nc.vector.tensor_copy(idx32[:], idxf[:])

                # ================= embedding gather + transpose =================
                x_rows = sb2.tile([32, 512], FP32, name="x_rows", tag="x_rows")
                nc.gpsimd.indirect_dma_start(
                    out=x_rows[:], out_offset=None, in_=emb[:],
                    in_offset=bass.IndirectOffsetOnAxis(ap=idx32[:, 0:1], axis=0),
                )
                p_x = ps_s.tile([128, 128], FP32, name="p_x", tag="small")
                for q in range(4):
                    nc.tensor.transpose(
                        p_x[:, 32 * q:32 * (q + 1)],
                        x_rows[:, 128 * q:128 * (q + 1)], ident[0:32, 0:32])
                nc.scalar.copy(xTh[:], p_x[:])
                nc.vector.tensor_copy(xT[:], p_x[:])
                nc.vector.tensor_tensor(xTl[:], xT[:], xTh[:], op=Alu.subtract)

        for p in reversed(pools):
            p.release()


def host_prep(inputs):
    """Build per-core in_maps from the full problem inputs."""
    z = np.asarray(inputs["z"], np.float32)
    embedding = np.ascontiguousarray(np.asarray(inputs["embedding"], np.float32))
    Wh = np.asarray(inputs["Wh"], np.float32)
    bh = np.asarray(inputs["bh"], np.float32)
    Wc = np.asarray(inputs["Wc"], np.float32)
    bc = np.asarray(inputs["bc"], np.float32)
    Wih = np.asarray(inputs["Wih"], np.float32)
    Whh = np.asarray(inputs["Whh"], np.float32)
    bih = np.asarray(inputs["bih"], np.float32)
    bhh = np.asarray(inputs["bhh"], np.float32)
    Wfc = np.asarray(inputs["Wfc"], np.float32)
    bfc = np.asarray(inputs["bfc"], np.float32)

    h0 = (z @ Wh.T + bh).astype(np.float32)   # [B, H]
    c0 = (z @ Wc.T + bc).astype(np.float32)
    b_gates = (bih + bhh).astype(np.float32)  # [4H]

    # gate column permutation: c' = q*512 + slot*128 + hw with slot order
    # [i, f, o, g] so the sigmoid gates are one contiguous 384-wide range.
    cp = np.arange(2048)
    qq, rem = cp // 512, cp % 512
    slot, hw = rem // 128, rem % 128
    gate = np.array([0, 1, 3, 2])[slot]        # slot -> original gate (i,f,o,g)
    perm = gate * 512 + qq * 128 + hw          # original col index for permuted col c'
    Wall = np.concatenate([Wih, Whh], axis=1)  # [2048, 1024] (k = [x | h])
    Wperm = Wall[perm]                         # [2048, 1024]
    wgates = np.ascontiguousarray(Wperm.T)     # [1024, 2048]

    import ml_dtypes

    def split_bf16(w):
        hi = w.astype(ml_dtypes.bfloat16)
        lo = (w - hi.astype(np.float32)).astype(ml_dtypes.bfloat16)
        return np.ascontiguousarray(hi), np.ascontiguousarray(lo)

    wgates_hi, wgates_lo = split_bf16(wgates)
    bg_hi, bg_lo = split_bf16(b_gates[perm][None, :])
    bias_g2 = np.ascontiguousarray(np.concatenate([bg_hi, bg_lo], axis=0))  # [2, 2048]

    # state layout tiles
    h0t = np.zeros((128, 128), np.float32)     # h0t[p, q*32+b] = h0[b, 128q+p]
    c0t = np.zeros((128, 128), np.float32)     # c0t[32q+b, hw] = c0[b, 128q+hw]
    for q in range(4):
        h0t[:, 32 * q:32 * (q + 1)] = h0[:, 128 * q:128 * (q + 1)].T
        c0t[32 * q:32 * (q + 1), :] = c0[:, 128 * q:128 * (q + 1)]

    ident = np.eye(128, dtype=np.float32)
    ones2 = np.ones((2, 32), ml_dtypes.bfloat16)

    in_maps = []
    for j in range(N_CORES):
        shard = Wfc[VS * j:VS * (j + 1)]                    # [4000, 512]
        shard_p = np.zeros((VSP, H), np.float32)
        shard_p[:VS] = shard
        wfc_in = np.ascontiguousarray(shard_p.T)            # [512, 4096]
        wfc_hi, wfc_lo = split_bf16(wfc_in)
        bfc_p = np.full(VSP, -1e30, np.float32)
        bfc_p[:VS] = bfc[VS * j:VS * (j + 1)]
        bf_hi, bf_lo = split_bf16(bfc_p[None, :])
        bias_fc2 = np.ascontiguousarray(np.concatenate([bf_hi, bf_lo], axis=0))
        # gkey[p, nt] = BIG - (VS*j + 1024*(p//32) + 512*nt)
        gbase = VS * j + (np.arange(128) // 32) * 1024
        gkey = np.stack([BIG - gbase, BIG - gbase - 512], axis=1).astype(np.float32)
        in_maps.append({
            "wfc_hi": wfc_hi,
            "wfc_lo": wfc_lo,
            "wgates_hi": wgates_hi,
            "wgates_lo": wgates_lo,
            "bias_g2": bias_g2,
            "bias_fc2": bias_fc2,
            "gkey": np.ascontiguousarray(gkey),
            "ident": ident,
            "ones2": ones2,
            "h0t": h0t,
            "c0": c0t,
            "emb": embedding,
        })
    return in_maps


def declare_io(nc, n_steps):
    io = {}
    io["wfc_hi"] = nc.dram_tensor("wfc_hi", [512, VSP], BF16, kind="ExternalInput").ap()
    io["wfc_lo"] = nc.dram_tensor("wfc_lo", [512, VSP], BF16, kind="ExternalInput").ap()
    io["wgates_hi"] = nc.dram_tensor("wgates_hi", [1024, 2048], BF16, kind="ExternalInput").ap()
    io["wgates_lo"] = nc.dram_tensor("wgates_lo", [1024, 2048], BF16, kind="ExternalInput").ap()
    io["bias_g2"] = nc.dram_tensor("bias_g2", [2, 2048], BF16, kind="ExternalInput").ap()
    io["bias_fc2"] = nc.dram_tensor("bias_fc2", [2, VSP], BF16, kind="ExternalInput").ap()
    io["gkey"] = nc.dram_tensor("gkey", [128, 2], FP32, kind="ExternalInput").ap()
    io["ident"] = nc.dram_tensor("ident", [128, 128], FP32, kind="ExternalInput").ap()
    io["ones2"] = nc.dram_tensor("ones2", [2, 32], BF16, kind="ExternalInput").ap()
    io["h0t"] = nc.dram_tensor("h0t", [128, 128], FP32, kind="ExternalInput").ap()
    io["c0"] = nc.dram_tensor("c0", [128, 128], FP32, kind="ExternalInput").ap()
    io["emb"] = nc.dram_tensor("emb", [V, E], FP32, kind="ExternalInput").ap()
    io["logits"] = nc.dram_tensor("logits", [n_steps, 128, 1024], BF16,
                                  kind="ExternalOutput").ap()
    return io


_BUILT = {}


def build(n_steps=T):
    if n_steps in _BUILT:
        return _BUILT[n_steps]
    nc = bacc.Bacc("TRN2", target_bir_lowering=False, debug=False,
                   num_devices=N_CORES)
    io = declare_io(nc, n_steps)
    build_decoder(nc, io, n_steps)
    nc.compile()
    _BUILT[n_steps] = nc
    return nc


def assemble(results, n_steps=T):
    """results: list of per-core out dicts -> full [B, T, V] fp32."""
    full = np.empty((B, n_steps, V), np.float32)
    for j in range(N_CORES):
        arr = results[j]["logits"].astype(np.float32)
        arr = arr.reshape(n_steps, 4, 32, 1024)
        arr = arr.transpose(2, 0, 1, 3).reshape(B, n_steps, VSP)[:, :, :VS]
        full[:, :, VS * j:VS * (j + 1)] = arr
    return full


def kernel(**inputs):
    n_steps = int(inputs.get("context_length", T))
    assert n_steps == T, f"kernel hardcodes T={T}, got {n_steps}"
    nc = build(T)
    in_maps = host_prep(inputs)
    res = run_bass_kernel_spmd(nc, in_maps, core_ids=list(range(N_CORES)))
    return assemble(res.results, T)


if __name__ == "__main__":
    import reference
    inputs = reference.setup_inputs()
    out = kernel(**{k: np.asarray(v) if hasattr(v, "shape") else v
                    for k, v in inputs.items()})
    print("output shape:", out.shape)


# revision 12
# speedup vs baseline: 1.0723x; 1.0275x over previous
"""Trainium2 Bass kernel for nn_AutoregressiveDecoder (LSTM decoder w/ greedy sampling).

Strategy (8 NeuronCores, SPMD):
  - Vocab-shard the fc projection: core j holds Wfc rows [4000j, 4000(j+1)) padded to
    4096 (pad bias = -1e30), resident in SBUF.
  - LSTM weights replicated per core, SBUF-resident, gate columns permuted so that
    PE column-group q computes [i|f|g|o] for hidden quarter q -> full-partition
    elementwise state updates.
  - All matmuls bf16x3 (hi*hi + hi*lo + lo*hi, fp32 psum accumulate) with 4-way PE
    column tiling; fc bias folded into the matmul via a 2-row ones lhsT so the
    argmax reads finished logits straight out of PSUM.
  - Greedy token: per-half (512-wide) max8/max_index pipelined under the second
    half's matmul, 4 candidates/core AllGather'd ([16,32] payload), single
    multi-axis-reduce fold after the exchange, indirect-DMA embedding gather.
  - Filler matmuls (zero operands) span the AllGather window so the PE's HAM
    clock gate stays at full rate across the per-step collective stall.
  - Logits stream to DRAM as bf16 [T, 128, 1024] per core; host reassembles
    [B, T, V] in fp32 (output tolerance is 2e-2; bf16 staging halves the
    copy+DMA cost while the on-device argmax stays fp32-exact).
"""
import sys

sys.path.insert(0, "/opt/trn_rl_repo")

import numpy as np

import concourse.bass as bass
import concourse.bacc as bacc
import concourse.tile as tile
import concourse.mybir as mybir
from concourse.bass_utils import run_bass_kernel_spmd

FP32 = mybir.dt.float32
BF16 = mybir.dt.bfloat16
I32 = mybir.dt.int32
U32 = mybir.dt.uint32

N_CORES = 8
B, L, H, E, V, T = 32, 256, 512, 512, 32000, 64
VS = V // N_CORES          # 4000 true shard
VSP = 4096                 # padded shard
BIG = 65536.0
# bf16 filler matmuls (~215-430ns each) spanning the PE-idle windows of a step
# so the HAM clock gate never sees a low-duty window and re-throttles:
# A: activations/state chain, P: argmax tail before the pay transpose,
# B: AllGather wait, C: fold+gather+x-prep.
FILL_A, FILL_P, FILL_B, FILL_C = 10, 8, 22, 14

Sigmoid = mybir.ActivationFunctionType.Sigmoid
Tanh = mybir.ActivationFunctionType.Tanh
Alu = mybir.AluOpType


def build_decoder(nc, io, n_steps):
    """Emit the full unrolled decoder. io: dict name -> DRAM AP."""
    with tile.TileContext(nc) as tc:
        sb = tc.alloc_tile_pool(name="sb", bufs=1)
        sb2 = tc.alloc_tile_pool(name="sb2", bufs=3)
        ps_g = tc.alloc_tile_pool(name="ps_g", bufs=2, space="PSUM")
        ps_v = tc.alloc_tile_pool(name="ps_v", bufs=2, space="PSUM")
        ps_s = tc.alloc_tile_pool(name="ps_s", bufs=2, space="PSUM")
        dr = tc.alloc_tile_pool(name="dr", bufs=4, space="DRAM")
        pools = [sb, sb2, ps_g, ps_v, ps_s, dr]

        # ---- persistent SBUF state & weights ----
        wfh = [sb.tile([128, VSP], BF16, name=f"wfh{q}") for q in range(4)]
        wfl = [sb.tile([128, VSP], BF16, name=f"wfl{q}") for q in range(4)]
        wgh = [sb.tile([128, 2048], BF16, name=f"wgh{r}") for r in range(8)]
        wgl = [sb.tile([128, 2048], BF16, name=f"wgl{r}") for r in range(8)]
        bias_g2 = sb.tile([2, 2048], BF16, name="bias_g2")
        bias_fc2 = sb.tile([2, VSP], BF16, name="bias_fc2")
        gkey = sb.tile([128, 2], FP32, name="gkey")   # BIG - (VS*j + 1024*g + 512*nt)
        ident = sb.tile([128, 128], FP32, name="ident")
        ones2 = sb.tile([2, 32], BF16, name="ones2")
        zeros = sb.tile([128, 512], BF16, name="zeros")
        xT = sb.tile([128, 128], FP32, name="xT")
        hT = sb.tile([128, 128], FP32, name="hT")
        xTh = sb.tile([128, 128], BF16, name="xTh")
        xTl = sb.tile([128, 128], BF16, name="xTl")
        hTh = sb.tile([128, 128], BF16, name="hTh")
        hTl = sb.tile([128, 128], BF16, name="hTl")
        c_t = sb.tile([128, 128], FP32, name="c_t")

        for q in range(4):
            nc.sync.dma_start(wfh[q][:], io["wfc_hi"][128 * q:128 * (q + 1), :])
            nc.sync.dma_start(wfl[q][:], io["wfc_lo"][128 * q:128 * (q + 1), :])
        for r in range(8):
            nc.sync.dma_start(wgh[r][:], io["wgates_hi"][128 * r:128 * (r + 1), :])
            nc.sync.dma_start(wgl[r][:], io["wgates_lo"][128 * r:128 * (r + 1), :])
        nc.sync.dma_start(bias_g2[:], io["bias_g2"][:])
        nc.sync.dma_start(bias_fc2[:], io["bias_fc2"][:])
        nc.sync.dma_start(gkey[:], io["gkey"][:])
        nc.sync.dma_start(ident[:], io["ident"][:])
        nc.sync.dma_start(ones2[:], io["ones2"][:])
        nc.sync.dma_start(xT[:], io["h0t"][:])
        nc.sync.dma_start(hT[:], io["h0t"][:])
        nc.sync.dma_start(c_t[:], io["c0"][:])
        nc.vector.memset(zeros[:], 0.0)
        # initial hi/lo splits of the (identical) x0 = h0 state
        nc.vector.tensor_copy(hTh[:], hT[:])
        nc.vector.tensor_tensor(hTl[:], hT[:], hTh[:], op=Alu.subtract)
        nc.vector.tensor_copy(xTh[:], hTh[:])
        nc.vector.tensor_copy(xTl[:], hTl[:])

        emb = io["emb"]
        out_logits = io["logits"]  # [T, 128, 1024] bf16

        # ---- gates matmul emission helpers ----
        # psum layout: partition 32q+b, free = gate*128+hw (cols permuted on host)
        def emit_gates_bias_h(pg):
            for g in range(4):
                nc.tensor.matmul(
                    pg[32 * g:32 * (g + 1), :], lhsT=ones2[:, :],
                    rhs=bias_g2[:, 512 * g:512 * (g + 1)],
                    start=True, stop=False, tile_position=(0, 32 * g),
                    skip_group_check=True,
                )
            emit_gates_rounds(pg, [4, 5, 6, 7], stop=False)

        def emit_gates_rounds(pg, rounds, stop):
            for r in rounds:
                hi, lo = (xTh, xTl) if r < 4 else (hTh, hTl)
                q = r % 4
                cs = slice(32 * q, 32 * (q + 1))
                for g in range(4):
                    gs = slice(512 * g, 512 * (g + 1))
                    out = pg[32 * g:32 * (g + 1), :]
                    passes = ((hi[:, cs], wgh[r][:, gs]),
                              (lo[:, cs], wgh[r][:, gs]),
                              (hi[:, cs], wgl[r][:, gs]))
                    for pi, (lhsT, rhs) in enumerate(passes):
                        nc.tensor.matmul(
                            out, lhsT=lhsT, rhs=rhs,
                            start=False,
                            stop=(stop and r == rounds[-1] and pi == 2),
                            tile_position=(0, 32 * g),
                            skip_group_check=True,
                        )

        def emit_fillers(n):
            # bf16 matmuls over zeros: 512 cycles each of PE activity with
            # minimal switching power; results never read.
            p_fil = ps_s.tile([128, 512], FP32, name="p_fil", tag="small")
            for _ in range(n):
                nc.tensor.matmul(
                    p_fil[:], lhsT=zeros[:, 0:128], rhs=zeros[:],
                    start=True, stop=True, skip_group_check=True,
                )

        # step-0 gates: bias + h-rounds up front (x == h0 so all 8 rounds)
        pg = ps_g.tile([128, 512], FP32, name="pg", tag="pg")
        emit_gates_bias_h(pg)

        for t in range(n_steps):
            last = t == n_steps - 1
            # ================= gates matmul: x-rounds =================
            emit_gates_rounds(pg, [0, 1, 2, 3], stop=True)
            emit_fillers(FILL_A)

            # ================= activations / state =================
            # gate slots after host permutation: [i | f | o | g(tanh)]
            acts = sb2.tile([128, 512], FP32, name="acts", tag="acts")
            nc.scalar.activation(acts[:, 0:256], pg[:, 0:256], Sigmoid)
            nc.scalar.activation(acts[:, 384:512], pg[:, 384:512], Tanh)
            nc.scalar.activation(acts[:, 256:384], pg[:, 256:384], Sigmoid)
            nc.vector.tensor_tensor(c_t[:], acts[:, 128:256], c_t[:], op=Alu.mult)
            t1 = sb2.tile([128, 128], FP32, name="t1", tag="t1")
            nc.vector.tensor_tensor(t1[:], acts[:, 0:128], acts[:, 384:512], op=Alu.mult)
            nc.vector.tensor_tensor(c_t[:], c_t[:], t1[:], op=Alu.add)
            tanh_c = sb2.tile([128, 128], FP32, name="tanh_c", tag="tanh_c")
            nc.scalar.activation(tanh_c[:], c_t[:], Tanh)
            h_new = sb2.tile([128, 128], FP32, name="h_new", tag="h_new")
            nc.vector.tensor_tensor(h_new[:], acts[:, 256:384], tanh_c[:], op=Alu.mult)

            # hT = transpose(h_new); hi cast on ACT in parallel with fp32 copy on DVE
            p_ht = ps_s.tile([128, 128], FP32, name="p_ht", tag="small")
            nc.tensor.transpose(p_ht[:], h_new[:], ident[:])
            nc.scalar.copy(hTh[:], p_ht[:])
            nc.vector.tensor_copy(hT[:], p_ht[:])
            nc.vector.tensor_tensor(hTl[:], hT[:], hTh[:], op=Alu.subtract)

            # ================= vocab matmul (bias folded in) =================
            # psum layout: partition 32g+b (g = vocab quarter of shard); two
            # separate psum tiles per half so half-0's argmax reads don't WAR-
            # block half-1's matmuls under tile-granular dep tracking.
            pvs = [ps_v.tile([128, 512], FP32, name=f"pv{nt}", tag=f"pv{nt}")
                   for nt in range(2)]
            staged = sb2.tile([128, 1024], BF16, name="staged", tag="staged")
            v8 = [None, None]
            i8 = [None, None]
            for nt in range(2):
                pv = pvs[nt]
                for g in range(4):
                    ws = slice(1024 * g + 512 * nt, 1024 * g + 512 * (nt + 1))
                    nc.tensor.matmul(
                        pv[32 * g:32 * (g + 1), :],
                        lhsT=ones2[:, :], rhs=bias_fc2[:, ws],
                        start=True, stop=False, tile_position=(0, 32 * g),
                        skip_group_check=True,
                    )
                for q in range(4):
                    cs = slice(32 * q, 32 * (q + 1))
                    for g in range(4):
                        ws = slice(1024 * g + 512 * nt, 1024 * g + 512 * (nt + 1))
                        out = pv[32 * g:32 * (g + 1), :]
                        passes = ((hTh[:, cs], wfh[q][:, ws]),
                                  (hTl[:, cs], wfh[q][:, ws]),
                                  (hTh[:, cs], wfl[q][:, ws]))
                        for pi, (lhsT, rhs) in enumerate(passes):
                            nc.tensor.matmul(
                                out, lhsT=lhsT, rhs=rhs,
                                start=False,
                                stop=(q == 3 and pi == 2),
                                tile_position=(0, 32 * g),
                                skip_group_check=True,
                            )
                # candidate first (critical path), then stage to DRAM (bf16);
                # the nt=0 chain runs on ACT/DVE under the nt=1 matmul.
                # pay rows (per psum partition 32g+b): [v_a, v_b, key_a, key_b]
                # where key = BIG - global_idx (so keys never collide with
                # logit values in the eq-fold, and max(key) = min global idx).
                half = slice(512 * nt, 512 * (nt + 1))
                if not last:
                    if nt == 0:
                        pay = sb2.tile([128, 4], FP32, name="pay", tag="pay")
                        iloc = sb2.tile([128, 2], FP32, name="iloc", tag="iloc")
                    v8[nt] = sb2.tile([128, 8], FP32, name=f"v8{nt}", tag=f"v8{nt}")
                    i8[nt] = sb2.tile([128, 8], U32, name=f"i8{nt}", tag=f"i8{nt}")
                    nc.vector.max(v8[nt][:], pv[:, :])
                    nc.vector.max_index(i8[nt][:], v8[nt][:], pv[:, :])
                    nc.vector.tensor_copy(pay[:, nt:nt + 1], v8[nt][:, 0:1])
                    nc.vector.tensor_copy(iloc[:, nt:nt + 1], i8[nt][:, 0:1])
                    nc.vector.tensor_scalar(
                        pay[:, 2 + nt:3 + nt], iloc[:, nt:nt + 1],
                        -1.0, gkey[:, nt:nt + 1], op0=Alu.mult, op1=Alu.add)
                nc.scalar.copy(staged[:, half], pv[:, :])
            nc.scalar.dma_start(out_logits[t], staged[:])

            if not last:
                emit_fillers(FILL_P)
                # transpose candidates -> [4, 128] and ship [16, 32] to the AG
                p_pa = ps_s.tile([4, 128], FP32, name="p_pa", tag="small")
                nc.tensor.transpose(p_pa[:], pay[:], ident[:])
                payT = sb2.tile([4, 128], FP32, name="payT", tag="payT")
                nc.scalar.copy(payT[:], p_pa[:])

                cc_in = dr.tile([16, 32], FP32, name="cc_in", tag="cc_in")
                cc_out = dr.tile([128, 32], FP32, name="cc_out", tag="cc_out",
                                 addr_space="Shared")
                # cc_in row = g*4 + f  <-  payT row f, free g*32+b
                nc.gpsimd.dma_start(
                    cc_in[:].rearrange("(g f) b -> f g b", g=4, f=4),
                    payT[:].rearrange("f (g b) -> f g b", g=4))
                nc.gpsimd.collective_compute(
                    "AllGather", Alu.bypass,
                    replica_groups=[list(range(N_CORES))],
                    ins=[cc_in[:]], outs=[cc_out[:]],
                )

            # ================= next-step gates: bias + h rounds =================
            if not last:
                pg = ps_g.tile([128, 512], FP32, name="pg", tag="pg")
                emit_gates_bias_h(pg)
                emit_fillers(FILL_B)

                # ================= AG result: fold over 64 candidates ==========
                agb = sb2.tile([128, 32], FP32, name="agb", tag="agb")
                nc.sync.dma_start(agb[:], cc_out[:])
                p_ag = ps_s.tile([32, 128], FP32, name="p_ag", tag="small")
                nc.tensor.transpose(p_ag[:], agb[:], ident[:])
                emit_fillers(FILL_C)
                t32 = sb2.tile([32, 128], FP32, name="t32", tag="t32")
                nc.vector.tensor_copy(t32[:], p_ag[:])

                # col = r*16 + g*4 + f; candidate order (r, g, f) is global-idx
                # order, and key = BIG - gidx makes max pick the first occurrence.
                tv = t32[:].rearrange("p (r g f) -> p r g f", r=8, g=4, f=4)
                vals = tv[:, :, :, 0:2]
                keys = tv[:, :, :, 2:4]
                gv32 = sb2.tile([32, 1], FP32, name="gv32", tag="gv32")
                nc.vector.tensor_reduce(gv32[:], vals, axis=mybir.AxisListType.XYZ,
                                        op=Alu.max)
                eqt = sb2.tile([32, 64], FP32, name="eqt", tag="eqt")
                eqv = eqt[:].rearrange("p (r g f) -> p r g f", r=8, g=4, f=2)
                nc.vector.tensor_scalar(eqv, vals, gv32[:, 0:1], None,
                                        op0=Alu.is_equal)
                mselt = sb2.tile([32, 64], FP32, name="mselt", tag="mselt")
                mselv = mselt[:].rearrange("p (r g f) -> p r g f", r=8, g=4, f=2)
                nc.vector.tensor_tensor(mselv, eqv, keys, op=Alu.mult)
                m2r = sb2.tile([32, 1], FP32, name="m2r", tag="m2r")
                nc.vector.tensor_reduce(m2r[:], mselv, axis=mybir.AxisListType.XYZ,
                                        op=Alu.max)
                idxf = sb2.tile([32, 1], FP32, name="idxf", tag="idxf")
                nc.vector.tensor_scalar(idxf[:], m2r[:], -1.0, BIG,
                                        op0=Alu.mult, op1=Alu.add)
                idx32 = sb2.tile([32, 1], I32, name="idx32", tag="idx32")
                nc.vector.tensor_copy(idx32[:], idxf[:])

                # ================= embedding gather + transpose =================
                x_rows = sb2.tile([32, 512], FP32, name="x_rows", tag="x_rows")
                nc.gpsimd.indirect_dma_start(
                    out=x_rows[:], out_offset=None, in_=emb[:],
                    in_offset=bass.IndirectOffsetOnAxis(ap=idx32[:, 0:1], axis=0),
                )
                p_x = ps_s.tile([128, 128], FP32, name="p_x", tag="small")
                for q in range(4):
                    nc.tensor.transpose(
                        p_x[:, 32 * q:32 * (q + 1)],
                        x_rows[:, 128 * q:128 * (q + 1)], ident[0:32, 0:32])
                nc.scalar.copy(xTh[:], p_x[:])
                nc.vector.tensor_copy(xT[:], p_x[:])
                nc.vector.tensor_tensor(xTl[:], xT[:], xTh[:], op=Alu.subtract)

        for p in reversed(pools):
            p.release()


def host_prep(inputs):
    """Build per-core in_maps from the full problem inputs."""
    z = np.asarray(inputs["z"], np.float32)
    embedding = np.ascontiguousarray(np.asarray(inputs["embedding"], np.float32))
    Wh = np.asarray(inputs["Wh"], np.float32)
    bh = np.asarray(inputs["bh"], np.float32)
    Wc = np.asarray(inputs["Wc"], np.float32)
    bc = np.asarray(inputs["bc"], np.float32)
    Wih = np.asarray(inputs["Wih"], np.float32)
    Whh = np.asarray(inputs["Whh"], np.float32)
    bih = np.asarray(inputs["bih"], np.float32)
    bhh = np.asarray(inputs["bhh"], np.float32)
    Wfc = np.asarray(inputs["Wfc"], np.float32)
    bfc = np.asarray(inputs["bfc"], np.float32)

    h0 = (z @ Wh.T + bh).astype(np.float32)   # [B, H]
    c0 = (z @ Wc.T + bc).astype(np.float32)
    b_gates = (bih + bhh).astype(np.float32)  # [4H]

    # gate column permutation: c' = q*512 + slot*128 + hw with slot order
    # [i, f, o, g] so the sigmoid gates are one contiguous 384-wide range.
    cp = np.arange(2048)
    qq, rem = cp // 512, cp % 512
    slot, hw = rem // 128, rem % 128
    gate = np.array([0, 1, 3, 2])[slot]        # slot -> original gate (i,f,o,g)
    perm = gate * 512 + qq * 128 + hw          # original col index for permuted col c'
    Wall = np.concatenate([Wih, Whh], axis=1)  # [2048, 1024] (k = [x | h])
    Wperm = Wall[perm]                         # [2048, 1024]
    wgates = np.ascontiguousarray(Wperm.T)     # [1024, 2048]

    import ml_dtypes

    def split_bf16(w):
        hi = w.astype(ml_dtypes.bfloat16)
        lo = (w - hi.astype(np.float32)).astype(ml_dtypes.bfloat16)
        return np.ascontiguousarray(hi), np.ascontiguousarray(lo)

    wgates_hi, wgates_lo = split_bf16(wgates)
    bg_hi, bg_lo = split_bf16(b_gates[perm][None, :])
    bias_g2 = np.ascontiguousarray(np.concatenate([bg_hi, bg_lo], axis=0))  # [2, 2048]

    # state layout tiles
    h0t = np.zeros((128, 128), np.float32)     # h0t[p, q*32+b] = h0[b, 128q+p]
    c0t = np.zeros((128, 128), np.float32)     # c0t[32q+b, hw] = c0[b, 128q+hw]
    for q in range(4):
        h0t[:, 32 * q:32 * (q + 1)] = h0[:, 128 * q:128 * (q + 1)].T
        c0t[32 * q:32 * (q + 1), :] = c0[:, 128 * q:128 * (q + 1)]

    ident = np.eye(128, dtype=np.float32)
    ones2 = np.ones((2, 32), ml_dtypes.bfloat16)

    in_maps = []
    for j in range(N_CORES):
        shard = Wfc[VS * j:VS * (j + 1)]                    # [4000, 512]
        shard_p = np.zeros((VSP, H), np.float32)
        shard_p[:VS] = shard
        wfc_in = np.ascontiguousarray(shard_p.T)            # [512, 4096]
        wfc_hi, wfc_lo = split_bf16(wfc_in)
        bfc_p = np.full(VSP, -1e30, np.float32)
        bfc_p[:VS] = bfc[VS * j:VS * (j + 1)]
        bf_hi, bf_lo = split_bf16(bfc_p[None, :])
        bias_fc2 = np.ascontiguousarray(np.concatenate([bf_hi, bf_lo], axis=0))
        # gkey[p, nt] = BIG - (VS*j + 1024*(p//32) + 512*nt)
        gbase = VS * j + (np.arange(128) // 32) * 1024
        gkey = np.stack([BIG - gbase, BIG - gbase - 512], axis=1).astype(np.float32)
        in_maps.append({
            "wfc_hi": wfc_hi,
            "wfc_lo": wfc_lo,
            "wgates_hi": wgates_hi,
            "wgates_lo": wgates_lo,
            "bias_g2": bias_g2,
            "bias_fc2": bias_fc2,
            "gkey": np.ascontiguousarray(gkey),
            "ident": ident,
            "ones2": ones2,
            "h0t": h0t,
            "c0": c0t,
            "emb": embedding,
        })
    return in_maps


def declare_io(nc, n_steps):
    io = {}
    io["wfc_hi"] = nc.dram_tensor("wfc_hi", [512, VSP], BF16, kind="ExternalInput").ap()
    io["wfc_lo"] = nc.dram_tensor("wfc_lo", [512, VSP], BF16, kind="ExternalInput").ap()
    io["wgates_hi"] = nc.dram_tensor("wgates_hi", [1024, 2048], BF16, kind="ExternalInput").ap()
    io["wgates_lo"] = nc.dram_tensor("wgates_lo", [1024, 2048], BF16, kind="ExternalInput").ap()
    io["bias_g2"] = nc.dram_tensor("bias_g2", [2, 2048], BF16, kind="ExternalInput").ap()
    io["bias_fc2"] = nc.dram_tensor("bias_fc2", [2, VSP], BF16, kind="ExternalInput").ap()
    io["gkey"] = nc.dram_tensor("gkey", [128, 2], FP32, kind="ExternalInput").ap()
    io["ident"] = nc.dram_tensor("ident", [128, 128], FP32, kind="ExternalInput").ap()
    io["ones2"] = nc.dram_tensor("ones2", [2, 32], BF16, kind="ExternalInput").ap()
    io["h0t"] = nc.dram_tensor("h0t", [128, 128], FP32, kind="ExternalInput").ap()
    io["c0"] = nc.dram_tensor("c0", [128, 128], FP32, kind="ExternalInput").ap()
    io["emb"] = nc.dram_tensor("emb", [V, E], FP32, kind="ExternalInput").ap()
    io["logits"] = nc.dram_tensor("logits", [n_steps, 128, 1024], BF16,
                                  kind="ExternalOutput").ap()
    return io


_BUILT = {}


def build(n_steps=T):
    if n_steps in _BUILT:
        return _BUILT[n_steps]
    nc = bacc.Bacc("TRN2", target_bir_lowering=False, debug=False,
                   num_devices=N_CORES)
    io = declare_io(nc, n_steps)
    build_decoder(nc, io, n_steps)
    nc.compile()
    _BUILT[n_steps] = nc
    return nc


def assemble(results, n_steps=T):
    """results: list of per-core out dicts -> full [B, T, V] fp32."""
    full = np.empty((B, n_steps, V), np.float32)
    for j in range(N_CORES):
        arr = results[j]["logits"].astype(np.float32)
        arr = arr.reshape(n_steps, 4, 32, 1024)
        arr = arr.transpose(2, 0, 1, 3).reshape(B, n_steps, VSP)[:, :, :VS]
        full[:, :, VS * j:VS * (j + 1)] = arr
    return full


def kernel(**inputs):
    n_steps = int(inputs.get("context_length", T))
    assert n_steps == T, f"kernel hardcodes T={T}, got {n_steps}"
    nc = build(T)
    in_maps = host_prep(inputs)
    res = run_bass_kernel_spmd(nc, in_maps, core_ids=list(range(N_CORES)))
    return assemble(res.results, T)


if __name__ == "__main__":
    import reference
    inputs = reference.setup_inputs()
    out = kernel(**{k: np.asarray(v) if hasattr(v, "shape") else v
                    for k, v in inputs.items()})
    print("output shape:", out.shape)


# revision 13
# speedup vs baseline: 1.1402x; 1.0634x over previous
"""Trainium2 Bass kernel for nn_AutoregressiveDecoder (LSTM decoder w/ greedy sampling).

Strategy (8 NeuronCores, SPMD):
  - Vocab-shard the fc projection: core j holds Wfc rows [4000j, 4000(j+1)) padded to
    4096 (pad bias = -1e30), resident in SBUF.
  - LSTM weights replicated per core, SBUF-resident, gate columns permuted so that
    PE column-group q computes [i|f|g|o] for hidden quarter q -> full-partition
    elementwise state updates.
  - All matmuls bf16x3 (hi*hi + hi*lo + lo*hi, fp32 psum accumulate) with 4-way PE
    column tiling; fc bias folded into the matmul via a 2-row ones lhsT so the
    argmax reads finished logits straight out of PSUM.
  - Greedy token: per-half (512-wide) max8/max_index pipelined under the second
    half's matmul, 4 candidates/core AllGather'd ([16,32] payload), single
    multi-axis-reduce fold after the exchange, indirect-DMA embedding gather.
  - Filler matmuls (zero operands) span the AllGather window so the PE's HAM
    clock gate stays at full rate across the per-step collective stall.
  - Logits stream to DRAM as bf16 [T, 128, 1024] per core; host reassembles
    [B, T, V] in fp32 (output tolerance is 2e-2; bf16 staging halves the
    copy+DMA cost while the on-device argmax stays fp32-exact).
"""
import sys

sys.path.insert(0, "/opt/trn_rl_repo")

import numpy as np

import concourse.bass as bass
import concourse.bacc as bacc
import concourse.tile as tile
import concourse.mybir as mybir
from concourse.bass_utils import run_bass_kernel_spmd

FP32 = mybir.dt.float32
BF16 = mybir.dt.bfloat16
I32 = mybir.dt.int32
U32 = mybir.dt.uint32

N_CORES = 8
B, L, H, E, V, T = 32, 256, 512, 512, 32000, 64
VS = V // N_CORES          # 4000 true shard
VSP = 4096                 # padded shard
BIG = 65536.0
# bf16 filler matmuls (~215-430ns each) spanning the PE-idle windows of a step
# so the HAM clock gate never sees a low-duty window and re-throttles:
# A: activations/state chain, P: argmax tail before the pay transpose,
# B: AllGather wait, C: fold+gather+x-prep.
FILL_A, FILL_P, FILL_B, FILL_C = 10, 8, 22, 18

Sigmoid = mybir.ActivationFunctionType.Sigmoid
Tanh = mybir.ActivationFunctionType.Tanh
Alu = mybir.AluOpType


def build_decoder(nc, io, n_steps):
    """Emit the full unrolled decoder. io: dict name -> DRAM AP."""
    with tile.TileContext(nc) as tc:
        sb = tc.alloc_tile_pool(name="sb", bufs=1)
        sb2 = tc.alloc_tile_pool(name="sb2", bufs=3)
        ps_g = tc.alloc_tile_pool(name="ps_g", bufs=2, space="PSUM")
        ps_v = tc.alloc_tile_pool(name="ps_v", bufs=2, space="PSUM")
        ps_s = tc.alloc_tile_pool(name="ps_s", bufs=2, space="PSUM")
        dr = tc.alloc_tile_pool(name="dr", bufs=4, space="DRAM")
        pools = [sb, sb2, ps_g, ps_v, ps_s, dr]

        # ---- persistent SBUF state & weights ----
        wfh = [sb.tile([128, VSP], BF16, name=f"wfh{q}") for q in range(4)]
        wfl = [sb.tile([128, VSP], BF16, name=f"wfl{q}") for q in range(4)]
        wgh = [sb.tile([128, 2048], BF16, name=f"wgh{r}") for r in range(8)]
        wgl = [sb.tile([128, 2048], BF16, name=f"wgl{r}") for r in range(8)]
        bias_g2 = sb.tile([2, 2048], BF16, name="bias_g2")
        bias_fc2 = sb.tile([2, VSP], BF16, name="bias_fc2")
        gkey = sb.tile([128, 2], FP32, name="gkey")   # BIG - (VS*j + 1024*g + 512*nt)
        ident = sb.tile([128, 128], FP32, name="ident")
        ones2 = sb.tile([2, 32], BF16, name="ones2")
        zeros = sb.tile([128, 512], BF16, name="zeros")
        xT = sb.tile([128, 128], FP32, name="xT")
        hT = sb.tile([128, 128], FP32, name="hT")
        xTh = sb.tile([128, 128], BF16, name="xTh")
        xTl = sb.tile([128, 128], BF16, name="xTl")
        hTh = sb.tile([128, 128], BF16, name="hTh")
        hTl = sb.tile([128, 128], BF16, name="hTl")
        c_t = sb.tile([128, 128], FP32, name="c_t")

        for q in range(4):
            nc.sync.dma_start(wfh[q][:], io["wfc_hi"][128 * q:128 * (q + 1), :])
            nc.sync.dma_start(wfl[q][:], io["wfc_lo"][128 * q:128 * (q + 1), :])
        for r in range(8):
            nc.sync.dma_start(wgh[r][:], io["wgates_hi"][128 * r:128 * (r + 1), :])
            nc.sync.dma_start(wgl[r][:], io["wgates_lo"][128 * r:128 * (r + 1), :])
        nc.sync.dma_start(bias_g2[:], io["bias_g2"][:])
        nc.sync.dma_start(bias_fc2[:], io["bias_fc2"][:])
        nc.sync.dma_start(gkey[:], io["gkey"][:])
        nc.sync.dma_start(ident[:], io["ident"][:])
        nc.sync.dma_start(ones2[:], io["ones2"][:])
        nc.sync.dma_start(xT[:], io["h0t"][:])
        nc.sync.dma_start(hT[:], io["h0t"][:])
        nc.sync.dma_start(c_t[:], io["c0"][:])
        nc.vector.memset(zeros[:], 0.0)
        # initial hi/lo splits of the (identical) x0 = h0 state
        nc.vector.tensor_copy(hTh[:], hT[:])
        nc.vector.tensor_tensor(hTl[:], hT[:], hTh[:], op=Alu.subtract)
        nc.vector.tensor_copy(xTh[:], hTh[:])
        nc.vector.tensor_copy(xTl[:], hTl[:])

        emb = io["emb"]
        out_logits = io["logits"]  # [T, 128, 1024] bf16

        # ---- gates matmul emission helpers ----
        # psum layout: partition 32q+b, free = gate*128+hw (cols permuted on host)
        def emit_gates_bias_h(pg):
            for g in range(4):
                nc.tensor.matmul(
                    pg[32 * g:32 * (g + 1), :], lhsT=ones2[:, :],
                    rhs=bias_g2[:, 512 * g:512 * (g + 1)],
                    start=True, stop=False, tile_position=(0, 32 * g),
                    skip_group_check=True,
                )
            emit_gates_rounds(pg, [4, 5, 6, 7], stop=False)

        def emit_gates_rounds(pg, rounds, stop):
            for r in rounds:
                hi, lo = (xTh, xTl) if r < 4 else (hTh, hTl)
                q = r % 4
                cs = slice(32 * q, 32 * (q + 1))
                for g in range(4):
                    gs = slice(512 * g, 512 * (g + 1))
                    out = pg[32 * g:32 * (g + 1), :]
                    passes = ((hi[:, cs], wgh[r][:, gs]),
                              (lo[:, cs], wgh[r][:, gs]),
                              (hi[:, cs], wgl[r][:, gs]))
                    for pi, (lhsT, rhs) in enumerate(passes):
                        nc.tensor.matmul(
                            out, lhsT=lhsT, rhs=rhs,
                            start=False,
                            stop=(stop and r == rounds[-1] and pi == 2),
                            tile_position=(0, 32 * g),
                            skip_group_check=True,
                        )

        def emit_fillers(n):
            # bf16 matmuls over zeros: 512 cycles each of PE activity with
            # minimal switching power; results never read.
            p_fil = ps_s.tile([128, 512], FP32, name="p_fil", tag="small")
            for _ in range(n):
                nc.tensor.matmul(
                    p_fil[:], lhsT=zeros[:, 0:128], rhs=zeros[:],
                    start=True, stop=True, skip_group_check=True,
                )

        # step-0 gates: bias + h-rounds up front (x == h0 so all 8 rounds)
        pg = ps_g.tile([128, 512], FP32, name="pg", tag="pg")
        emit_gates_bias_h(pg)

        for t in range(n_steps):
            last = t == n_steps - 1
            # ================= gates matmul: x-rounds =================
            emit_gates_rounds(pg, [0, 1, 2, 3], stop=True)
            emit_fillers(FILL_A)

            # ================= activations / state =================
            # gate slots after host permutation: [i | f | o | g(tanh)]
            acts = sb2.tile([128, 512], FP32, name="acts", tag="acts")
            nc.scalar.activation(acts[:, 0:256], pg[:, 0:256], Sigmoid)
            nc.scalar.activation(acts[:, 384:512], pg[:, 384:512], Tanh)
            nc.scalar.activation(acts[:, 256:384], pg[:, 256:384], Sigmoid)
            nc.vector.tensor_tensor(c_t[:], acts[:, 128:256], c_t[:], op=Alu.mult)
            t1 = sb2.tile([128, 128], FP32, name="t1", tag="t1")
            nc.vector.tensor_tensor(t1[:], acts[:, 0:128], acts[:, 384:512], op=Alu.mult)
            nc.vector.tensor_tensor(c_t[:], c_t[:], t1[:], op=Alu.add)
            tanh_c = sb2.tile([128, 128], FP32, name="tanh_c", tag="tanh_c")
            nc.scalar.activation(tanh_c[:], c_t[:], Tanh)
            h_new = sb2.tile([128, 128], FP32, name="h_new", tag="h_new")
            nc.vector.tensor_tensor(h_new[:], acts[:, 256:384], tanh_c[:], op=Alu.mult)

            # hT = transpose(h_new); hi cast on ACT in parallel with fp32 copy on DVE
            p_ht = ps_s.tile([128, 128], FP32, name="p_ht", tag="small")
            nc.tensor.transpose(p_ht[:], h_new[:], ident[:])
            nc.scalar.copy(hTh[:], p_ht[:])
            nc.vector.tensor_copy(hT[:], p_ht[:])
            nc.vector.tensor_tensor(hTl[:], hT[:], hTh[:], op=Alu.subtract)

            # ================= vocab matmul (bias folded in) =================
            # psum layout: partition 32g+b (g = vocab quarter of shard); two
            # separate psum tiles per half so half-0's argmax reads don't WAR-
            # block half-1's matmuls under tile-granular dep tracking.
            pvs = [ps_v.tile([128, 512], FP32, name=f"pv{nt}", tag=f"pv{nt}")
                   for nt in range(2)]
            staged = sb2.tile([128, 1024], BF16, name="staged", tag="staged")
            v8 = [None, None]
            i8 = [None, None]
            for nt in range(2):
                pv = pvs[nt]
                for g in range(4):
                    ws = slice(1024 * g + 512 * nt, 1024 * g + 512 * (nt + 1))
                    nc.tensor.matmul(
                        pv[32 * g:32 * (g + 1), :],
                        lhsT=ones2[:, :], rhs=bias_fc2[:, ws],
                        start=True, stop=False, tile_position=(0, 32 * g),
                        skip_group_check=True,
                    )
                for q in range(4):
                    cs = slice(32 * q, 32 * (q + 1))
                    for g in range(4):
                        ws = slice(1024 * g + 512 * nt, 1024 * g + 512 * (nt + 1))
                        out = pv[32 * g:32 * (g + 1), :]
                        passes = ((hTh[:, cs], wfh[q][:, ws]),
                                  (hTl[:, cs], wfh[q][:, ws]),
                                  (hTh[:, cs], wfl[q][:, ws]))
                        for pi, (lhsT, rhs) in enumerate(passes):
                            nc.tensor.matmul(
                                out, lhsT=lhsT, rhs=rhs,
                                start=False,
                                stop=(q == 3 and pi == 2),
                                tile_position=(0, 32 * g),
                                skip_group_check=True,
                            )
                # candidate first (critical path), then stage to DRAM (bf16);
                # the nt=0 chain runs on ACT/DVE under the nt=1 matmul.
                # pay rows (per psum partition 32g+b): [v_a, v_b, key_a, key_b]
                # where key = BIG - global_idx (so keys never collide with
                # logit values in the eq-fold, and max(key) = min global idx).
                half = slice(512 * nt, 512 * (nt + 1))
                if not last:
                    if nt == 0:
                        pay = sb2.tile([128, 4], FP32, name="pay", tag="pay")
                        iloc = sb2.tile([128, 2], FP32, name="iloc", tag="iloc")
                    v8[nt] = sb2.tile([128, 8], FP32, name=f"v8{nt}", tag=f"v8{nt}")
                    i8[nt] = sb2.tile([128, 8], U32, name=f"i8{nt}", tag=f"i8{nt}")
                    nc.vector.max(v8[nt][:], pv[:, :])
                    nc.vector.max_index(i8[nt][:], v8[nt][:], pv[:, :])
                    nc.vector.tensor_copy(pay[:, nt:nt + 1], v8[nt][:, 0:1])
                    nc.vector.tensor_copy(iloc[:, nt:nt + 1], i8[nt][:, 0:1])
                    nc.vector.tensor_scalar(
                        pay[:, 2 + nt:3 + nt], iloc[:, nt:nt + 1],
                        -1.0, gkey[:, nt:nt + 1], op0=Alu.mult, op1=Alu.add)
                if last:
                    nc.scalar.copy(staged[:, half], pv[:, :])
            if last:
                nc.scalar.dma_start(out_logits[t], staged[:])

            if not last:
                emit_fillers(FILL_P)
                # transpose candidates -> [4, 128] and ship [16, 32] to the AG
                p_pa = ps_s.tile([4, 128], FP32, name="p_pa", tag="small")
                nc.tensor.transpose(p_pa[:], pay[:], ident[:])
                payT = sb2.tile([4, 128], FP32, name="payT", tag="payT")
                nc.scalar.copy(payT[:], p_pa[:])
                # staged copies after payT on the ACT queue: the AG trigger is
                # critical, the logits write has a full step of slack.
                for nt in range(2):
                    nc.scalar.copy(staged[:, 512 * nt:512 * (nt + 1)], pvs[nt][:, :])
                nc.scalar.dma_start(out_logits[t], staged[:])

                cc_in = dr.tile([16, 32], FP32, name="cc_in", tag="cc_in")
                cc_out = dr.tile([128, 32], FP32, name="cc_out", tag="cc_out",
                                 addr_space="Shared")
                # cc_in row = g*4 + f  <-  payT row f, free g*32+b
                nc.sync.dma_start(
                    cc_in[:].rearrange("(g f) b -> f g b", g=4, f=4),
                    payT[:].rearrange("f (g b) -> f g b", g=4))
                nc.gpsimd.collective_compute(
                    "AllGather", Alu.bypass,
                    replica_groups=[list(range(N_CORES))],
                    ins=[cc_in[:]], outs=[cc_out[:]],
                )

            # ================= next-step gates: bias + h rounds =================
            if not last:
                pg = ps_g.tile([128, 512], FP32, name="pg", tag="pg")
                emit_gates_bias_h(pg)
                emit_fillers(FILL_B)

                # ================= AG result: fold over 64 candidates ==========
                agb = sb2.tile([128, 32], FP32, name="agb", tag="agb")
                nc.sync.dma_start(agb[:], cc_out[:])
                p_ag = ps_s.tile([32, 128], FP32, name="p_ag", tag="small")
                nc.tensor.transpose(p_ag[:], agb[:], ident[:])
                emit_fillers(FILL_C)
                t32 = sb2.tile([32, 128], FP32, name="t32", tag="t32")
                nc.vector.tensor_copy(t32[:], p_ag[:])

                # col = r*16 + g*4 + f; candidate order (r, g, f) is global-idx
                # order, and key = BIG - gidx makes max pick the first occurrence.
                tv = t32[:].rearrange("p (r g f) -> p r g f", r=8, g=4, f=4)
                vals = tv[:, :, :, 0:2]
                keys = tv[:, :, :, 2:4]
                gv32 = sb2.tile([32, 1], FP32, name="gv32", tag="gv32")
                nc.vector.tensor_reduce(gv32[:], vals, axis=mybir.AxisListType.XYZ,
                                        op=Alu.max)
                eqt = sb2.tile([32, 64], FP32, name="eqt", tag="eqt")
                eqv = eqt[:].rearrange("p (r g f) -> p r g f", r=8, g=4, f=2)
                nc.vector.tensor_scalar(eqv, vals, gv32[:, 0:1], None,
                                        op0=Alu.is_equal)
                mselt = sb2.tile([32, 64], FP32, name="mselt", tag="mselt")
                mselv = mselt[:].rearrange("p (r g f) -> p r g f", r=8, g=4, f=2)
                nc.vector.tensor_tensor(mselv, eqv, keys, op=Alu.mult)
                m2r = sb2.tile([32, 1], FP32, name="m2r", tag="m2r")
                nc.vector.tensor_reduce(m2r[:], mselv, axis=mybir.AxisListType.XYZ,
                                        op=Alu.max)
                idxf = sb2.tile([32, 1], FP32, name="idxf", tag="idxf")
                nc.vector.tensor_scalar(idxf[:], m2r[:], -1.0, BIG,
                                        op0=Alu.mult, op1=Alu.add)
                idx32 = sb2.tile([32, 1], I32, name="idx32", tag="idx32")
                nc.vector.tensor_copy(idx32[:], idxf[:])

                # ================= embedding gather + transpose =================
                x_rows = sb2.tile([32, 512], FP32, name="x_rows", tag="x_rows")
                nc.gpsimd.indirect_dma_start(
                    out=x_rows[:], out_offset=None, in_=emb[:],
                    in_offset=bass.IndirectOffsetOnAxis(ap=idx32[:, 0:1], axis=0),
                )
                p_x = ps_s.tile([128, 128], FP32, name="p_x", tag="small")
                for q in range(4):
                    nc.tensor.transpose(
                        p_x[:, 32 * q:32 * (q + 1)],
                        x_rows[:, 128 * q:128 * (q + 1)], ident[0:32, 0:32])
                nc.scalar.copy(xTh[:], p_x[:])
                nc.vector.tensor_copy(xT[:], p_x[:])
                nc.vector.tensor_tensor(xTl[:], xT[:], xTh[:], op=Alu.subtract)

        for p in reversed(pools):
            p.release()


def host_prep(inputs):
    """Build per-core in_maps from the full problem inputs."""
    z = np.asarray(inputs["z"], np.float32)
    embedding = np.ascontiguousarray(np.asarray(inputs["embedding"], np.float32))
    Wh = np.asarray(inputs["Wh"], np.float32)
    bh = np.asarray(inputs["bh"], np.float32)
    Wc = np.asarray(inputs["Wc"], np.float32)
    bc = np.asarray(inputs["bc"], np.float32)
    Wih = np.asarray(inputs["Wih"], np.float32)
    Whh = np.asarray(inputs["Whh"], np.float32)
    bih = np.asarray(inputs["bih"], np.float32)
    bhh = np.asarray(inputs["bhh"], np.float32)
    Wfc = np.asarray(inputs["Wfc"], np.float32)
    bfc = np.asarray(inputs["bfc"], np.float32)

    h0 = (z @ Wh.T + bh).astype(np.float32)   # [B, H]
    c0 = (z @ Wc.T + bc).astype(np.float32)
    b_gates = (bih + bhh).astype(np.float32)  # [4H]

    # gate column permutation: c' = q*512 + slot*128 + hw with slot order
    # [i, f, o, g] so the sigmoid gates are one contiguous 384-wide range.
    cp = np.arange(2048)
    qq, rem = cp // 512, cp % 512
    slot, hw = rem // 128, rem % 128
    gate = np.array([0, 1, 3, 2])[slot]        # slot -> original gate (i,f,o,g)
    perm = gate * 512 + qq * 128 + hw          # original col index for permuted col c'
    Wall = np.concatenate([Wih, Whh], axis=1)  # [2048, 1024] (k = [x | h])
    Wperm = Wall[perm]                         # [2048, 1024]
    wgates = np.ascontiguousarray(Wperm.T)     # [1024, 2048]

    import ml_dtypes

    def split_bf16(w):
        hi = w.astype(ml_dtypes.bfloat16)
        lo = (w - hi.astype(np.float32)).astype(ml_dtypes.bfloat16)
        return np.ascontiguousarray(hi), np.ascontiguousarray(lo)

    wgates_hi, wgates_lo = split_bf16(wgates)
    bg_hi, bg_lo = split_bf16(b_gates[perm][None, :])
    bias_g2 = np.ascontiguousarray(np.concatenate([bg_hi, bg_lo], axis=0))  # [2, 2048]

    # state layout tiles
    h0t = np.zeros((128, 128), np.float32)     # h0t[p, q*32+b] = h0[b, 128q+p]
    c0t = np.zeros((128, 128), np.float32)     # c0t[32q+b, hw] = c0[b, 128q+hw]
    for q in range(4):
        h0t[:, 32 * q:32 * (q + 1)] = h0[:, 128 * q:128 * (q + 1)].T
        c0t[32 * q:32 * (q + 1), :] = c0[:, 128 * q:128 * (q + 1)]

    ident = np.eye(128, dtype=np.float32)
    ones2 = np.ones((2, 32), ml_dtypes.bfloat16)

    in_maps = []
    for j in range(N_CORES):
        shard = Wfc[VS * j:VS * (j + 1)]                    # [4000, 512]
        shard_p = np.zeros((VSP, H), np.float32)
        shard_p[:VS] = shard
        wfc_in = np.ascontiguousarray(shard_p.T)            # [512, 4096]
        wfc_hi, wfc_lo = split_bf16(wfc_in)
        bfc_p = np.full(VSP, -1e30, np.float32)
        bfc_p[:VS] = bfc[VS * j:VS * (j + 1)]
        bf_hi, bf_lo = split_bf16(bfc_p[None, :])
        bias_fc2 = np.ascontiguousarray(np.concatenate([bf_hi, bf_lo], axis=0))
        # gkey[p, nt] = BIG - (VS*j + 1024*(p//32) + 512*nt)
        gbase = VS * j + (np.arange(128) // 32) * 1024
        gkey = np.stack([BIG - gbase, BIG - gbase - 512], axis=1).astype(np.float32)
        in_maps.append({
            "wfc_hi": wfc_hi,
            "wfc_lo": wfc_lo,
            "wgates_hi": wgates_hi,
            "wgates_lo": wgates_lo,
            "bias_g2": bias_g2,
            "bias_fc2": bias_fc2,
            "gkey": np.ascontiguousarray(gkey),
            "ident": ident,
            "ones2": ones2,
            "h0t": h0t,
            "c0": c0t,
            "emb": embedding,
        })
    return in_maps


def declare_io(nc, n_steps):
    io = {}
    io["wfc_hi"] = nc.dram_tensor("wfc_hi", [512, VSP], BF16, kind="ExternalInput").ap()
    io["wfc_lo"] = nc.dram_tensor("wfc_lo", [512, VSP], BF16, kind="ExternalInput").ap()
    io["wgates_hi"] = nc.dram_tensor("wgates_hi", [1024, 2048], BF16, kind="ExternalInput").ap()
    io["wgates_lo"] = nc.dram_tensor("wgates_lo", [1024, 2048], BF16, kind="ExternalInput").ap()
    io["bias_g2"] = nc.dram_tensor("bias_g2", [2, 2048], BF16, kind="ExternalInput").ap()
    io["bias_fc2"] = nc.dram_tensor("bias_fc2", [2, VSP], BF16, kind="ExternalInput").ap()
    io["gkey"] = nc.dram_tensor("gkey", [128, 2], FP32, kind="ExternalInput").ap()
    io["ident"] = nc.dram_tensor("ident", [128, 128], FP32, kind="ExternalInput").ap()
    io["ones2"] = nc.dram_tensor("ones2", [2, 32], BF16, kind="ExternalInput").ap()
    io["h0t"] = nc.dram_tensor("h0t", [128, 128], FP32, kind="ExternalInput").ap()
    io["c0"] = nc.dram_tensor("c0", [128, 128], FP32, kind="ExternalInput").ap()
    io["emb"] = nc.dram_tensor("emb", [V, E], FP32, kind="ExternalInput").ap()
    io["logits"] = nc.dram_tensor("logits", [n_steps, 128, 1024], BF16,
                                  kind="ExternalOutput").ap()
    return io


_BUILT = {}


def build(n_steps=T):
    if n_steps in _BUILT:
        return _BUILT[n_steps]
    nc = bacc.Bacc("TRN2", target_bir_lowering=False, debug=False,
                   num_devices=N_CORES)
    io = declare_io(nc, n_steps)
    build_decoder(nc, io, n_steps)
    nc.compile()
    _BUILT[n_steps] = nc
    return nc


def assemble(results, n_steps=T):
    """results: list of per-core out dicts -> full [B, T, V] fp32."""
    full = np.empty((B, n_steps, V), np.float32)
    for j in range(N_CORES):
        arr = results[j]["logits"].astype(np.float32)
        arr = arr.reshape(n_steps, 4, 32, 1024)
        arr = arr.transpose(2, 0, 1, 3).reshape(B, n_steps, VSP)[:, :, :VS]
        full[:, :, VS * j:VS * (j + 1)] = arr
    return full


def kernel(**inputs):
    n_steps = int(inputs.get("context_length", T))
    assert n_steps == T, f"kernel hardcodes T={T}, got {n_steps}"
    nc = build(T)
    in_maps = host_prep(inputs)
    res = run_bass_kernel_spmd(nc, in_maps, core_ids=list(range(N_CORES)))
    return assemble(res.results, T)


if __name__ == "__main__":
    import reference
    inputs = reference.setup_inputs()
    out = kernel(**{k: np.asarray(v) if hasattr(v, "shape") else v
                    for k, v in inputs.items()})
    print("output shape:", out.shape)


# revision 14
# speedup vs baseline: 1.1490x; 1.0077x over previous
"""Trainium2 Bass kernel for nn_AutoregressiveDecoder (LSTM decoder w/ greedy sampling).

Strategy (8 NeuronCores, SPMD):
  - Vocab-shard the fc projection: core j holds Wfc rows [4000j, 4000(j+1)) padded to
    4096 (pad bias = -1e30), resident in SBUF.
  - LSTM weights replicated per core, SBUF-resident, gate columns permuted so that
    PE column-group q computes [i|f|g|o] for hidden quarter q -> full-partition
    elementwise state updates.
  - All matmuls bf16x3 (hi*hi + hi*lo + lo*hi, fp32 psum accumulate) with 4-way PE
    column tiling; fc bias folded into the matmul via a 2-row ones lhsT so the
    argmax reads finished logits straight out of PSUM.
  - Greedy token: per-half (512-wide) max8/max_index pipelined under the second
    half's matmul, 4 candidates/core AllGather'd ([16,32] payload), single
    multi-axis-reduce fold after the exchange, indirect-DMA embedding gather.
  - Filler matmuls (zero operands) span the AllGather window so the PE's HAM
    clock gate stays at full rate across the per-step collective stall.
  - Logits stream to DRAM as bf16 [T, 128, 1024] per core; host reassembles
    [B, T, V] in fp32 (output tolerance is 2e-2; bf16 staging halves the
    copy+DMA cost while the on-device argmax stays fp32-exact).
"""
import sys

sys.path.insert(0, "/opt/trn_rl_repo")

import numpy as np

import concourse.bass as bass
import concourse.bacc as bacc
import concourse.tile as tile
import concourse.mybir as mybir
from concourse.bass_utils import run_bass_kernel_spmd

FP32 = mybir.dt.float32
BF16 = mybir.dt.bfloat16
I32 = mybir.dt.int32
U32 = mybir.dt.uint32

N_CORES = 8
B, L, H, E, V, T = 32, 256, 512, 512, 32000, 64
VS = V // N_CORES          # 4000 true shard
VSP = 4096                 # padded shard
BIG = 65536.0
# bf16 filler matmuls (~215-430ns each) spanning the PE-idle windows of a step
# so the HAM clock gate never sees a low-duty window and re-throttles:
# A: activations/state chain, P: argmax tail before the pay transpose,
# B: AllGather wait, C: fold+gather+x-prep.
FILL_A, FILL_P, FILL_B, FILL_C = 14, 8, 22, 20

Sigmoid = mybir.ActivationFunctionType.Sigmoid
Tanh = mybir.ActivationFunctionType.Tanh
Alu = mybir.AluOpType


def build_decoder(nc, io, n_steps):
    """Emit the full unrolled decoder. io: dict name -> DRAM AP."""
    with tile.TileContext(nc) as tc:
        sb = tc.alloc_tile_pool(name="sb", bufs=1)
        sb2 = tc.alloc_tile_pool(name="sb2", bufs=3)
        ps_g = tc.alloc_tile_pool(name="ps_g", bufs=2, space="PSUM")
        ps_v = tc.alloc_tile_pool(name="ps_v", bufs=2, space="PSUM")
        ps_s = tc.alloc_tile_pool(name="ps_s", bufs=2, space="PSUM")
        dr = tc.alloc_tile_pool(name="dr", bufs=4, space="DRAM")
        pools = [sb, sb2, ps_g, ps_v, ps_s, dr]

        # ---- persistent SBUF state & weights ----
        wfh = [sb.tile([128, VSP], BF16, name=f"wfh{q}") for q in range(4)]
        wfl = [sb.tile([128, VSP], BF16, name=f"wfl{q}") for q in range(4)]
        wgh = [sb.tile([128, 2048], BF16, name=f"wgh{r}") for r in range(8)]
        wgl = [sb.tile([128, 2048], BF16, name=f"wgl{r}") for r in range(8)]
        bias_g2 = sb.tile([2, 2048], BF16, name="bias_g2")
        bias_fc2 = sb.tile([2, VSP], BF16, name="bias_fc2")
        gkey = sb.tile([128, 2], FP32, name="gkey")   # BIG - (VS*j + 1024*g + 512*nt)
        ident = sb.tile([128, 128], FP32, name="ident")
        ones2 = sb.tile([2, 32], BF16, name="ones2")
        zeros = sb.tile([128, 512], BF16, name="zeros")
        identb = sb.tile([32, 32], BF16, name="identb")
        hT = sb.tile([128, 128], FP32, name="hT")
        xTh = sb.tile([128, 128], BF16, name="xTh")
        xTl = sb.tile([128, 128], BF16, name="xTl")
        hTh = sb.tile([128, 128], BF16, name="hTh")
        hTl = sb.tile([128, 128], BF16, name="hTl")
        c_t = sb.tile([128, 128], FP32, name="c_t")

        for q in range(4):
            nc.sync.dma_start(wfh[q][:], io["wfc_hi"][128 * q:128 * (q + 1), :])
            nc.sync.dma_start(wfl[q][:], io["wfc_lo"][128 * q:128 * (q + 1), :])
        for r in range(8):
            nc.sync.dma_start(wgh[r][:], io["wgates_hi"][128 * r:128 * (r + 1), :])
            nc.sync.dma_start(wgl[r][:], io["wgates_lo"][128 * r:128 * (r + 1), :])
        nc.sync.dma_start(bias_g2[:], io["bias_g2"][:])
        nc.sync.dma_start(bias_fc2[:], io["bias_fc2"][:])
        nc.sync.dma_start(gkey[:], io["gkey"][:])
        nc.sync.dma_start(ident[:], io["ident"][:])
        nc.sync.dma_start(ones2[:], io["ones2"][:])
        nc.sync.dma_start(identb[:], io["identb"][:])
        nc.sync.dma_start(hT[:], io["h0t"][:])
        nc.sync.dma_start(c_t[:], io["c0"][:])
        nc.vector.memset(zeros[:], 0.0)
        # initial hi/lo splits of the (identical) x0 = h0 state
        nc.vector.tensor_copy(hTh[:], hT[:])
        nc.vector.tensor_tensor(hTl[:], hT[:], hTh[:], op=Alu.subtract)
        nc.vector.tensor_copy(xTh[:], hTh[:])
        nc.vector.tensor_copy(xTl[:], hTl[:])

        emb = io["emb"]
        out_logits = io["logits"]  # [T, 128, 1024] bf16

        # ---- gates matmul emission helpers ----
        # psum layout: partition 32q+b, free = gate*128+hw (cols permuted on host)
        def emit_gates_bias_h(pg):
            for g in range(4):
                nc.tensor.matmul(
                    pg[32 * g:32 * (g + 1), :], lhsT=ones2[:, :],
                    rhs=bias_g2[:, 512 * g:512 * (g + 1)],
                    start=True, stop=False, tile_position=(0, 32 * g),
                    skip_group_check=True,
                )
            emit_gates_rounds(pg, [4, 5, 6, 7], stop=False)

        def emit_gates_rounds(pg, rounds, stop):
            for r in rounds:
                hi, lo = (xTh, xTl) if r < 4 else (hTh, hTl)
                q = r % 4
                cs = slice(32 * q, 32 * (q + 1))
                for g in range(4):
                    gs = slice(512 * g, 512 * (g + 1))
                    out = pg[32 * g:32 * (g + 1), :]
                    passes = ((hi[:, cs], wgh[r][:, gs]),
                              (lo[:, cs], wgh[r][:, gs]),
                              (hi[:, cs], wgl[r][:, gs]))
                    for pi, (lhsT, rhs) in enumerate(passes):
                        nc.tensor.matmul(
                            out, lhsT=lhsT, rhs=rhs,
                            start=False,
                            stop=(stop and r == rounds[-1] and pi == 2),
                            tile_position=(0, 32 * g),
                            skip_group_check=True,
                        )

        def emit_fillers(n):
            # bf16 matmuls over zeros: 512 cycles each of PE activity with
            # minimal switching power; results never read.
            p_fil = ps_s.tile([128, 512], FP32, name="p_fil", tag="small")
            for _ in range(n):
                nc.tensor.matmul(
                    p_fil[:], lhsT=zeros[:, 0:128], rhs=zeros[:],
                    start=True, stop=True, skip_group_check=True,
                )

        # step-0 gates: bias + h-rounds up front (x == h0 so all 8 rounds)
        pg = ps_g.tile([128, 512], FP32, name="pg", tag="pg")
        emit_gates_bias_h(pg)

        for t in range(n_steps):
            last = t == n_steps - 1
            # ================= gates matmul: x-rounds =================
            emit_gates_rounds(pg, [0, 1, 2, 3], stop=True)
            emit_fillers(FILL_A)

            # ================= activations / state =================
            # gate slots after host permutation: [i | f | o | g(tanh)]
            acts = sb2.tile([128, 512], FP32, name="acts", tag="acts")
            nc.scalar.activation(acts[:, 0:256], pg[:, 0:256], Sigmoid)
            nc.scalar.activation(acts[:, 384:512], pg[:, 384:512], Tanh)
            nc.scalar.activation(acts[:, 256:384], pg[:, 256:384], Sigmoid)
            nc.vector.tensor_tensor(c_t[:], acts[:, 128:256], c_t[:], op=Alu.mult)
            t1 = sb2.tile([128, 128], FP32, name="t1", tag="t1")
            nc.vector.tensor_tensor(t1[:], acts[:, 0:128], acts[:, 384:512], op=Alu.mult)
            nc.vector.tensor_tensor(c_t[:], c_t[:], t1[:], op=Alu.add)
            tanh_c = sb2.tile([128, 128], FP32, name="tanh_c", tag="tanh_c")
            nc.scalar.activation(tanh_c[:], c_t[:], Tanh)
            h_new = sb2.tile([128, 128], FP32, name="h_new", tag="h_new")
            nc.vector.tensor_tensor(h_new[:], acts[:, 256:384], tanh_c[:], op=Alu.mult)

            # hT = transpose(h_new); hi cast on ACT in parallel with fp32 copy on DVE
            p_ht = ps_s.tile([128, 128], FP32, name="p_ht", tag="small")
            nc.tensor.transpose(p_ht[:], h_new[:], ident[:])
            nc.scalar.copy(hTh[:], p_ht[:])
            nc.vector.tensor_copy(hT[:], p_ht[:])
            nc.vector.tensor_tensor(hTl[:], hT[:], hTh[:], op=Alu.subtract)

            # ================= vocab matmul (bias folded in) =================
            # psum layout: partition 32g+b (g = vocab quarter of shard); two
            # separate psum tiles per half so half-0's argmax reads don't WAR-
            # block half-1's matmuls under tile-granular dep tracking.
            pvs = [ps_v.tile([128, 512], FP32, name=f"pv{nt}", tag=f"pv{nt}")
                   for nt in range(2)]
            staged = sb2.tile([128, 1024], BF16, name="staged", tag="staged")
            v8 = [None, None]
            i8 = [None, None]
            for nt in range(2):
                pv = pvs[nt]
                for g in range(4):
                    ws = slice(1024 * g + 512 * nt, 1024 * g + 512 * (nt + 1))
                    nc.tensor.matmul(
                        pv[32 * g:32 * (g + 1), :],
                        lhsT=ones2[:, :], rhs=bias_fc2[:, ws],
                        start=True, stop=False, tile_position=(0, 32 * g),
                        skip_group_check=True,
                    )
                for q in range(4):
                    cs = slice(32 * q, 32 * (q + 1))
                    for g in range(4):
                        ws = slice(1024 * g + 512 * nt, 1024 * g + 512 * (nt + 1))
                        out = pv[32 * g:32 * (g + 1), :]
                        passes = ((hTh[:, cs], wfh[q][:, ws]),
                                  (hTl[:, cs], wfh[q][:, ws]),
                                  (hTh[:, cs], wfl[q][:, ws]))
                        for pi, (lhsT, rhs) in enumerate(passes):
                            nc.tensor.matmul(
                                out, lhsT=lhsT, rhs=rhs,
                                start=False,
                                stop=(q == 3 and pi == 2),
                                tile_position=(0, 32 * g),
                                skip_group_check=True,
                            )
                # candidate first (critical path), then stage to DRAM (bf16);
                # the nt=0 chain runs on ACT/DVE under the nt=1 matmul.
                # pay rows (per psum partition 32g+b): [v_a, v_b, key_a, key_b]
                # where key = BIG - global_idx (so keys never collide with
                # logit values in the eq-fold, and max(key) = min global idx).
                half = slice(512 * nt, 512 * (nt + 1))
                if not last:
                    if nt == 0:
                        pay = sb2.tile([128, 4], FP32, name="pay", tag="pay")
                        iloc = sb2.tile([128, 2], FP32, name="iloc", tag="iloc")
                    v8[nt] = sb2.tile([128, 8], FP32, name=f"v8{nt}", tag=f"v8{nt}")
                    i8[nt] = sb2.tile([128, 8], U32, name=f"i8{nt}", tag=f"i8{nt}")
                    nc.vector.max(v8[nt][:], pv[:, :])
                    nc.vector.max_index(i8[nt][:], v8[nt][:], pv[:, :])
                    nc.vector.tensor_copy(pay[:, nt:nt + 1], v8[nt][:, 0:1])
                    nc.vector.tensor_copy(iloc[:, nt:nt + 1], i8[nt][:, 0:1])
                    nc.vector.tensor_scalar(
                        pay[:, 2 + nt:3 + nt], iloc[:, nt:nt + 1],
                        -1.0, gkey[:, nt:nt + 1], op0=Alu.mult, op1=Alu.add)
                if last:
                    nc.scalar.copy(staged[:, half], pv[:, :])
            if last:
                nc.scalar.dma_start(out_logits[t], staged[:])

            if not last:
                emit_fillers(FILL_P)
                # transpose candidates -> [4, 128] and ship [16, 32] to the AG
                p_pa = ps_s.tile([4, 128], FP32, name="p_pa", tag="small")
                nc.tensor.transpose(p_pa[:], pay[:], ident[:])
                payT = sb2.tile([4, 128], FP32, name="payT", tag="payT")
                nc.scalar.copy(payT[:], p_pa[:])
                # staged copies after payT on the ACT queue: the AG trigger is
                # critical, the logits write has a full step of slack.
                for nt in range(2):
                    nc.scalar.copy(staged[:, 512 * nt:512 * (nt + 1)], pvs[nt][:, :])
                nc.scalar.dma_start(out_logits[t], staged[:])

                cc_in = dr.tile([16, 32], FP32, name="cc_in", tag="cc_in")
                cc_out = dr.tile([128, 32], FP32, name="cc_out", tag="cc_out",
                                 addr_space="Shared")
                # cc_in row = g*4 + f  <-  payT row f, free g*32+b
                nc.scalar.dma_start(
                    cc_in[:].rearrange("(g f) b -> f g b", g=4, f=4),
                    payT[:].rearrange("f (g b) -> f g b", g=4))
                nc.gpsimd.collective_compute(
                    "AllGather", Alu.bypass,
                    replica_groups=[list(range(N_CORES))],
                    ins=[cc_in[:]], outs=[cc_out[:]],
                )

            # ================= next-step gates: bias + h rounds =================
            if not last:
                pg = ps_g.tile([128, 512], FP32, name="pg", tag="pg")
                emit_gates_bias_h(pg)
                emit_fillers(FILL_B)

                # ================= AG result: fold over 64 candidates ==========
                agb = sb2.tile([128, 32], FP32, name="agb", tag="agb")
                nc.sync.dma_start(agb[:], cc_out[:])
                p_ag = ps_s.tile([32, 128], FP32, name="p_ag", tag="small")
                nc.tensor.transpose(p_ag[:], agb[:], ident[:])
                emit_fillers(FILL_C)
                t32 = sb2.tile([32, 128], FP32, name="t32", tag="t32")
                nc.vector.tensor_copy(t32[:], p_ag[:])

                # col = r*16 + g*4 + f; candidate order (r, g, f) is global-idx
                # order, and key = BIG - gidx makes max pick the first occurrence.
                tv = t32[:].rearrange("p (r g f) -> p r g f", r=8, g=4, f=4)
                vals = tv[:, :, :, 0:2]
                keys = tv[:, :, :, 2:4]
                gv32 = sb2.tile([32, 1], FP32, name="gv32", tag="gv32")
                nc.vector.tensor_reduce(gv32[:], vals, axis=mybir.AxisListType.XYZ,
                                        op=Alu.max)
                eqt = sb2.tile([32, 64], FP32, name="eqt", tag="eqt")
                eqv = eqt[:].rearrange("p (r g f) -> p r g f", r=8, g=4, f=2)
                nc.vector.tensor_scalar(eqv, vals, gv32[:, 0:1], None,
                                        op0=Alu.is_equal)
                mselt = sb2.tile([32, 64], FP32, name="mselt", tag="mselt")
                mselv = mselt[:].rearrange("p (r g f) -> p r g f", r=8, g=4, f=2)
                nc.vector.tensor_tensor(mselv, eqv, keys, op=Alu.mult)
                m2r = sb2.tile([32, 1], FP32, name="m2r", tag="m2r")
                nc.vector.tensor_reduce(m2r[:], mselv, axis=mybir.AxisListType.XYZ,
                                        op=Alu.max)
                idxf = sb2.tile([32, 1], FP32, name="idxf", tag="idxf")
                nc.vector.tensor_scalar(idxf[:], m2r[:], -1.0, BIG,
                                        op0=Alu.mult, op1=Alu.add)
                idx32 = sb2.tile([32, 1], I32, name="idx32", tag="idx32")
                nc.vector.tensor_copy(idx32[:], idxf[:])

                # ================= embedding gather + transpose =================
                # emb rows ship pre-split as [hi | lo] bf16, so the transposed
                # chunks land directly as xTh/xTl with no fp32 split chain.
                x_rows = sb2.tile([32, 1024], BF16, name="x_rows", tag="x_rows")
                nc.gpsimd.indirect_dma_start(
                    out=x_rows[:], out_offset=None, in_=emb[:],
                    in_offset=bass.IndirectOffsetOnAxis(ap=idx32[:, 0:1], axis=0),
                )
                p_x = ps_s.tile([128, 256], BF16, name="p_x", tag="small")
                for q in range(4):
                    nc.tensor.transpose(
                        p_x[:, 32 * q:32 * (q + 1)],
                        x_rows[:, 128 * q:128 * (q + 1)], identb[:])
                    nc.tensor.transpose(
                        p_x[:, 128 + 32 * q:128 + 32 * (q + 1)],
                        x_rows[:, 512 + 128 * q:512 + 128 * (q + 1)], identb[:])
                nc.scalar.copy(xTh[:], p_x[:, 0:128])
                nc.vector.tensor_copy(xTl[:], p_x[:, 128:256])

        for p in reversed(pools):
            p.release()


def host_prep(inputs):
    """Build per-core in_maps from the full problem inputs."""
    z = np.asarray(inputs["z"], np.float32)
    embedding = np.ascontiguousarray(np.asarray(inputs["embedding"], np.float32))
    Wh = np.asarray(inputs["Wh"], np.float32)
    bh = np.asarray(inputs["bh"], np.float32)
    Wc = np.asarray(inputs["Wc"], np.float32)
    bc = np.asarray(inputs["bc"], np.float32)
    Wih = np.asarray(inputs["Wih"], np.float32)
    Whh = np.asarray(inputs["Whh"], np.float32)
    bih = np.asarray(inputs["bih"], np.float32)
    bhh = np.asarray(inputs["bhh"], np.float32)
    Wfc = np.asarray(inputs["Wfc"], np.float32)
    bfc = np.asarray(inputs["bfc"], np.float32)

    h0 = (z @ Wh.T + bh).astype(np.float32)   # [B, H]
    c0 = (z @ Wc.T + bc).astype(np.float32)
    b_gates = (bih + bhh).astype(np.float32)  # [4H]

    # gate column permutation: c' = q*512 + slot*128 + hw with slot order
    # [i, f, o, g] so the sigmoid gates are one contiguous 384-wide range.
    cp = np.arange(2048)
    qq, rem = cp // 512, cp % 512
    slot, hw = rem // 128, rem % 128
    gate = np.array([0, 1, 3, 2])[slot]        # slot -> original gate (i,f,o,g)
    perm = gate * 512 + qq * 128 + hw          # original col index for permuted col c'
    Wall = np.concatenate([Wih, Whh], axis=1)  # [2048, 1024] (k = [x | h])
    Wperm = Wall[perm]                         # [2048, 1024]
    wgates = np.ascontiguousarray(Wperm.T)     # [1024, 2048]

    import ml_dtypes

    def split_bf16(w):
        hi = w.astype(ml_dtypes.bfloat16)
        lo = (w - hi.astype(np.float32)).astype(ml_dtypes.bfloat16)
        return np.ascontiguousarray(hi), np.ascontiguousarray(lo)

    wgates_hi, wgates_lo = split_bf16(wgates)
    bg_hi, bg_lo = split_bf16(b_gates[perm][None, :])
    bias_g2 = np.ascontiguousarray(np.concatenate([bg_hi, bg_lo], axis=0))  # [2, 2048]

    # state layout tiles
    h0t = np.zeros((128, 128), np.float32)     # h0t[p, q*32+b] = h0[b, 128q+p]
    c0t = np.zeros((128, 128), np.float32)     # c0t[32q+b, hw] = c0[b, 128q+hw]
    for q in range(4):
        h0t[:, 32 * q:32 * (q + 1)] = h0[:, 128 * q:128 * (q + 1)].T
        c0t[32 * q:32 * (q + 1), :] = c0[:, 128 * q:128 * (q + 1)]

    ident = np.eye(128, dtype=np.float32)
    identb = np.eye(32, dtype=ml_dtypes.bfloat16)
    ones2 = np.ones((2, 32), ml_dtypes.bfloat16)
    emb_hi, emb_lo = split_bf16(embedding)
    emb2 = np.ascontiguousarray(np.concatenate([emb_hi, emb_lo], axis=1))

    in_maps = []
    for j in range(N_CORES):
        shard = Wfc[VS * j:VS * (j + 1)]                    # [4000, 512]
        shard_p = np.zeros((VSP, H), np.float32)
        shard_p[:VS] = shard
        wfc_in = np.ascontiguousarray(shard_p.T)            # [512, 4096]
        wfc_hi, wfc_lo = split_bf16(wfc_in)
        bfc_p = np.full(VSP, -1e30, np.float32)
        bfc_p[:VS] = bfc[VS * j:VS * (j + 1)]
        bf_hi, bf_lo = split_bf16(bfc_p[None, :])
        bias_fc2 = np.ascontiguousarray(np.concatenate([bf_hi, bf_lo], axis=0))
        # gkey[p, nt] = BIG - (VS*j + 1024*(p//32) + 512*nt)
        gbase = VS * j + (np.arange(128) // 32) * 1024
        gkey = np.stack([BIG - gbase, BIG - gbase - 512], axis=1).astype(np.float32)
        in_maps.append({
            "wfc_hi": wfc_hi,
            "wfc_lo": wfc_lo,
            "wgates_hi": wgates_hi,
            "wgates_lo": wgates_lo,
            "bias_g2": bias_g2,
            "bias_fc2": bias_fc2,
            "gkey": np.ascontiguousarray(gkey),
            "ident": ident,
            "ones2": ones2,
            "identb": identb,
            "h0t": h0t,
            "c0": c0t,
            "emb": emb2,
        })
    return in_maps


def declare_io(nc, n_steps):
    io = {}
    io["wfc_hi"] = nc.dram_tensor("wfc_hi", [512, VSP], BF16, kind="ExternalInput").ap()
    io["wfc_lo"] = nc.dram_tensor("wfc_lo", [512, VSP], BF16, kind="ExternalInput").ap()
    io["wgates_hi"] = nc.dram_tensor("wgates_hi", [1024, 2048], BF16, kind="ExternalInput").ap()
    io["wgates_lo"] = nc.dram_tensor("wgates_lo", [1024, 2048], BF16, kind="ExternalInput").ap()
    io["bias_g2"] = nc.dram_tensor("bias_g2", [2, 2048], BF16, kind="ExternalInput").ap()
    io["bias_fc2"] = nc.dram_tensor("bias_fc2", [2, VSP], BF16, kind="ExternalInput").ap()
    io["gkey"] = nc.dram_tensor("gkey", [128, 2], FP32, kind="ExternalInput").ap()
    io["ident"] = nc.dram_tensor("ident", [128, 128], FP32, kind="ExternalInput").ap()
    io["ones2"] = nc.dram_tensor("ones2", [2, 32], BF16, kind="ExternalInput").ap()
    io["identb"] = nc.dram_tensor("identb", [32, 32], BF16, kind="ExternalInput").ap()
    io["h0t"] = nc.dram_tensor("h0t", [128, 128], FP32, kind="ExternalInput").ap()
    io["c0"] = nc.dram_tensor("c0", [128, 128], FP32, kind="ExternalInput").ap()
    io["emb"] = nc.dram_tensor("emb", [V, 2 * E], BF16, kind="ExternalInput").ap()
    io["logits"] = nc.dram_tensor("logits", [n_steps, 128, 1024], BF16,
                                  kind="ExternalOutput").ap()
    return io


_BUILT = {}


def build(n_steps=T):
    if n_steps in _BUILT:
        return _BUILT[n_steps]
    nc = bacc.Bacc("TRN2", target_bir_lowering=False, debug=False,
                   num_devices=N_CORES)
    io = declare_io(nc, n_steps)
    build_decoder(nc, io, n_steps)
    nc.compile()
    _BUILT[n_steps] = nc
    return nc


def assemble(results, n_steps=T):
    """results: list of per-core out dicts -> full [B, T, V] fp32."""
    full = np.empty((B, n_steps, V), np.float32)
    for j in range(N_CORES):
        arr = results[j]["logits"].astype(np.float32)
        arr = arr.reshape(n_steps, 4, 32, 1024)
        arr = arr.transpose(2, 0, 1, 3).reshape(B, n_steps, VSP)[:, :, :VS]
        full[:, :, VS * j:VS * (j + 1)] = arr
    return full


def kernel(**inputs):
    n_steps = int(inputs.get("context_length", T))
    assert n_steps == T, f"kernel hardcodes T={T}, got {n_steps}"
    nc = build(T)
    in_maps = host_prep(inputs)
    res = run_bass_kernel_spmd(nc, in_maps, core_ids=list(range(N_CORES)))
    return assemble(res.results, T)


if __name__ == "__main__":
    import reference
    inputs = reference.setup_inputs()
    out = kernel(**{k: np.asarray(v) if hasattr(v, "shape") else v
                    for k, v in inputs.items()})
    print("output shape:", out.shape)


# revision 15
# speedup vs baseline: 1.2479x; 1.0861x over previous
"""Trainium2 Bass kernel for nn_AutoregressiveDecoder (LSTM decoder w/ greedy sampling).

Strategy (8 NeuronCores, SPMD):
  - Vocab-shard the fc projection: core j holds Wfc rows [4000j, 4000(j+1)) padded to
    4096 (pad bias = -1e30), resident in SBUF.
  - LSTM weights replicated per core, SBUF-resident, gate columns permuted so that
    PE column-group q computes [i|f|g|o] for hidden quarter q -> full-partition
    elementwise state updates.
  - All matmuls bf16x3 (hi*hi + hi*lo + lo*hi, fp32 psum accumulate) with 4-way PE
    column tiling; fc bias folded into the matmul via a 2-row ones lhsT so the
    argmax reads finished logits straight out of PSUM.
  - Greedy token: per-half (512-wide) max8/max_index pipelined under the second
    half's matmul, 4 candidates/core AllGather'd ([16,32] payload), single
    multi-axis-reduce fold after the exchange, indirect-DMA embedding gather.
  - Filler matmuls (zero operands) span the AllGather window so the PE's HAM
    clock gate stays at full rate across the per-step collective stall.
  - Logits stream to DRAM as bf16 [T, 128, 1024] per core; host reassembles
    [B, T, V] in fp32 (output tolerance is 2e-2; bf16 staging halves the
    copy+DMA cost while the on-device argmax stays fp32-exact).
"""
import sys

sys.path.insert(0, "/opt/trn_rl_repo")

import numpy as np

import concourse.bass as bass
import concourse.bacc as bacc
import concourse.tile as tile
import concourse.mybir as mybir
from concourse.bass_utils import run_bass_kernel_spmd

FP32 = mybir.dt.float32
BF16 = mybir.dt.bfloat16
I32 = mybir.dt.int32
U32 = mybir.dt.uint32

N_CORES = 8
B, L, H, E, V, T = 32, 256, 512, 512, 32000, 64
VS = V // N_CORES          # 4000 true shard
VSP = 4096                 # padded shard
BIG = 65536.0
# bf16 filler matmuls (~215-430ns each) spanning the PE-idle windows of a step
# so the HAM clock gate never sees a low-duty window and re-throttles:
# A: activations/state chain, P: argmax tail before the pay transpose,
# B: AllGather wait, C: fold+gather+x-prep.
FILL_A, FILL_P, FILL_B, FILL_C = 14, 8, 22, 20

Sigmoid = mybir.ActivationFunctionType.Sigmoid
Tanh = mybir.ActivationFunctionType.Tanh
Alu = mybir.AluOpType


def build_decoder(nc, io, n_steps):
    """Emit the full unrolled decoder. io: dict name -> DRAM AP."""
    with tile.TileContext(nc) as tc:
        sb = tc.alloc_tile_pool(name="sb", bufs=1)
        sb2 = tc.alloc_tile_pool(name="sb2", bufs=3)
        ps_g = tc.alloc_tile_pool(name="ps_g", bufs=2, space="PSUM")
        ps_v = tc.alloc_tile_pool(name="ps_v", bufs=2, space="PSUM")
        ps_s = tc.alloc_tile_pool(name="ps_s", bufs=2, space="PSUM")
        dr = tc.alloc_tile_pool(name="dr", bufs=4, space="DRAM")
        pools = [sb, sb2, ps_g, ps_v, ps_s, dr]

        # ---- persistent SBUF state & weights ----
        wfh = [sb.tile([128, VSP], BF16, name=f"wfh{q}") for q in range(4)]
        wfl = [sb.tile([128, VSP], BF16, name=f"wfl{q}") for q in range(4)]
        wgh = [sb.tile([128, 2048], BF16, name=f"wgh{r}") for r in range(8)]
        wgl = [sb.tile([128, 2048], BF16, name=f"wgl{r}") for r in range(8)]
        bias_g2 = sb.tile([2, 2048], BF16, name="bias_g2")
        bias_fc2 = sb.tile([2, VSP], BF16, name="bias_fc2")
        gkey = sb.tile([128, 2], FP32, name="gkey")   # BIG - (VS*j + 1024*g + 512*nt)
        ident = sb.tile([128, 128], FP32, name="ident")
        ones2 = sb.tile([2, 32], BF16, name="ones2")
        zeros = sb.tile([128, 512], BF16, name="zeros")
        identb = sb.tile([32, 32], BF16, name="identb")
        hT = sb.tile([128, 128], FP32, name="hT")
        xTh = sb.tile([128, 128], BF16, name="xTh")
        xTl = sb.tile([128, 128], BF16, name="xTl")
        hTh = sb.tile([128, 128], BF16, name="hTh")
        hTl = sb.tile([128, 128], BF16, name="hTl")
        c_t = sb.tile([128, 128], FP32, name="c_t")

        for q in range(4):
            nc.sync.dma_start(wfh[q][:], io["wfc_hi"][128 * q:128 * (q + 1), :])
            nc.sync.dma_start(wfl[q][:], io["wfc_lo"][128 * q:128 * (q + 1), :])
        for r in range(8):
            nc.sync.dma_start(wgh[r][:], io["wgates_hi"][128 * r:128 * (r + 1), :])
            nc.sync.dma_start(wgl[r][:], io["wgates_lo"][128 * r:128 * (r + 1), :])
        nc.sync.dma_start(bias_g2[:], io["bias_g2"][:])
        nc.sync.dma_start(bias_fc2[:], io["bias_fc2"][:])
        nc.sync.dma_start(gkey[:], io["gkey"][:])
        nc.sync.dma_start(ident[:], io["ident"][:])
        nc.sync.dma_start(ones2[:], io["ones2"][:])
        nc.sync.dma_start(identb[:], io["identb"][:])
        nc.sync.dma_start(hT[:], io["h0t"][:])
        nc.sync.dma_start(c_t[:], io["c0"][:])
        nc.vector.memset(zeros[:], 0.0)
        # initial hi/lo splits of the (identical) x0 = h0 state
        nc.vector.tensor_copy(hTh[:], hT[:])
        nc.vector.tensor_tensor(hTl[:], hT[:], hTh[:], op=Alu.subtract)
        nc.vector.tensor_copy(xTh[:], hTh[:])
        nc.vector.tensor_copy(xTl[:], hTl[:])

        emb = io["emb"]
        out_logits = io["logits"]  # [T, 128, 1024] bf16

        # ---- gates matmul emission helpers ----
        # psum layout: partition 32q+b, free = gate*128+hw (cols permuted on host)
        def emit_gates_bias_h(pg, with_bias):
            # with_bias: step-0 form (bias wave carries start=True). Steps >=1
            # get the bias from the precomputed G table, so start moves to the
            # first h-round matmul of each column group.
            if with_bias:
                for g in range(4):
                    nc.tensor.matmul(
                        pg[32 * g:32 * (g + 1), :], lhsT=ones2[:, :],
                        rhs=bias_g2[:, 512 * g:512 * (g + 1)],
                        start=True, stop=False, tile_position=(0, 32 * g),
                        skip_group_check=True,
                    )
            emit_gates_rounds(pg, [4, 5, 6, 7], stop=False, start=not with_bias)

        def emit_gates_rounds(pg, rounds, stop, start=False):
            for r in rounds:
                hi, lo = (xTh, xTl) if r < 4 else (hTh, hTl)
                q = r % 4
                cs = slice(32 * q, 32 * (q + 1))
                for g in range(4):
                    gs = slice(512 * g, 512 * (g + 1))
                    out = pg[32 * g:32 * (g + 1), :]
                    passes = ((hi[:, cs], wgh[r][:, gs]),
                              (lo[:, cs], wgh[r][:, gs]),
                              (hi[:, cs], wgl[r][:, gs]))
                    for pi, (lhsT, rhs) in enumerate(passes):
                        nc.tensor.matmul(
                            out, lhsT=lhsT, rhs=rhs,
                            start=(start and r == rounds[0] and pi == 0),
                            stop=(stop and r == rounds[-1] and pi == 2),
                            tile_position=(0, 32 * g),
                            skip_group_check=True,
                        )

        def emit_gates_x(pg, gx):
            # inject the gathered per-token gates rows (hi|lo bf16) into the
            # psum accumulation via identity matmuls: 2 waves total.
            for w in range(2):
                for g in range(4):
                    nc.tensor.matmul(
                        pg[32 * g:32 * (g + 1), :], lhsT=identb[:],
                        rhs=gx[:, 2048 * w + 512 * g:2048 * w + 512 * (g + 1)],
                        start=False, stop=(w == 1),
                        tile_position=(0, 32 * g), skip_group_check=True,
                    )

        def emit_fillers(n):
            # bf16 matmuls over zeros: 512 cycles each of PE activity with
            # minimal switching power; results never read.
            p_fil = ps_s.tile([128, 512], FP32, name="p_fil", tag="small")
            for _ in range(n):
                nc.tensor.matmul(
                    p_fil[:], lhsT=zeros[:, 0:128], rhs=zeros[:],
                    start=True, stop=True, skip_group_check=True,
                )

        # step-0 gates: bias + h-rounds up front (x == h0 so all 8 rounds)
        pg = ps_g.tile([128, 512], FP32, name="pg", tag="pg")
        emit_gates_bias_h(pg, with_bias=True)

        for t in range(n_steps):
            last = t == n_steps - 1
            # ================= gates matmul: x contribution =================
            if t == 0:
                emit_gates_rounds(pg, [0, 1, 2, 3], stop=True)
            else:
                emit_gates_x(pg, gx)
            emit_fillers(FILL_A)

            # ================= activations / state =================
            # gate slots after host permutation: [i | f | o | g(tanh)]
            acts = sb2.tile([128, 512], FP32, name="acts", tag="acts")
            nc.scalar.activation(acts[:, 0:256], pg[:, 0:256], Sigmoid)
            nc.scalar.activation(acts[:, 384:512], pg[:, 384:512], Tanh)
            nc.scalar.activation(acts[:, 256:384], pg[:, 256:384], Sigmoid)
            nc.vector.tensor_tensor(c_t[:], acts[:, 128:256], c_t[:], op=Alu.mult)
            t1 = sb2.tile([128, 128], FP32, name="t1", tag="t1")
            nc.vector.tensor_tensor(t1[:], acts[:, 0:128], acts[:, 384:512], op=Alu.mult)
            nc.vector.tensor_tensor(c_t[:], c_t[:], t1[:], op=Alu.add)
            tanh_c = sb2.tile([128, 128], FP32, name="tanh_c", tag="tanh_c")
            nc.scalar.activation(tanh_c[:], c_t[:], Tanh)
            h_new = sb2.tile([128, 128], FP32, name="h_new", tag="h_new")
            nc.vector.tensor_tensor(h_new[:], acts[:, 256:384], tanh_c[:], op=Alu.mult)

            # hT = transpose(h_new); hi cast on ACT in parallel with fp32 copy on DVE
            p_ht = ps_s.tile([128, 128], FP32, name="p_ht", tag="small")
            nc.tensor.transpose(p_ht[:], h_new[:], ident[:])
            nc.scalar.copy(hTh[:], p_ht[:])
            nc.vector.tensor_copy(hT[:], p_ht[:])
            nc.vector.tensor_tensor(hTl[:], hT[:], hTh[:], op=Alu.subtract)

            # ================= vocab matmul (bias folded in) =================
            # psum layout: partition 32g+b (g = vocab quarter of shard); two
            # separate psum tiles per half so half-0's argmax reads don't WAR-
            # block half-1's matmuls under tile-granular dep tracking.
            pvs = [ps_v.tile([128, 512], FP32, name=f"pv{nt}", tag=f"pv{nt}")
                   for nt in range(2)]
            staged = sb2.tile([128, 1024], BF16, name="staged", tag="staged")
            v8 = [None, None]
            i8 = [None, None]
            for nt in range(2):
                pv = pvs[nt]
                for g in range(4):
                    ws = slice(1024 * g + 512 * nt, 1024 * g + 512 * (nt + 1))
                    nc.tensor.matmul(
                        pv[32 * g:32 * (g + 1), :],
                        lhsT=ones2[:, :], rhs=bias_fc2[:, ws],
                        start=True, stop=False, tile_position=(0, 32 * g),
                        skip_group_check=True,
                    )
                for q in range(4):
                    cs = slice(32 * q, 32 * (q + 1))
                    for g in range(4):
                        ws = slice(1024 * g + 512 * nt, 1024 * g + 512 * (nt + 1))
                        out = pv[32 * g:32 * (g + 1), :]
                        passes = ((hTh[:, cs], wfh[q][:, ws]),
                                  (hTl[:, cs], wfh[q][:, ws]),
                                  (hTh[:, cs], wfl[q][:, ws]))
                        for pi, (lhsT, rhs) in enumerate(passes):
                            nc.tensor.matmul(
                                out, lhsT=lhsT, rhs=rhs,
                                start=False,
                                stop=(q == 3 and pi == 2),
                                tile_position=(0, 32 * g),
                                skip_group_check=True,
                            )
                # candidate first (critical path), then stage to DRAM (bf16);
                # the nt=0 chain runs on ACT/DVE under the nt=1 matmul.
                # pay rows (per psum partition 32g+b): [v_a, v_b, key_a, key_b]
                # where key = BIG - global_idx (so keys never collide with
                # logit values in the eq-fold, and max(key) = min global idx).
                half = slice(512 * nt, 512 * (nt + 1))
                if not last:
                    if nt == 0:
                        pay = sb2.tile([128, 4], FP32, name="pay", tag="pay")
                        iloc = sb2.tile([128, 2], FP32, name="iloc", tag="iloc")
                    v8[nt] = sb2.tile([128, 8], FP32, name=f"v8{nt}", tag=f"v8{nt}")
                    i8[nt] = sb2.tile([128, 8], U32, name=f"i8{nt}", tag=f"i8{nt}")
                    nc.vector.max(v8[nt][:], pv[:, :])
                    nc.vector.max_index(i8[nt][:], v8[nt][:], pv[:, :])
                    nc.vector.tensor_copy(pay[:, nt:nt + 1], v8[nt][:, 0:1])
                    nc.vector.tensor_copy(iloc[:, nt:nt + 1], i8[nt][:, 0:1])
                    nc.vector.tensor_scalar(
                        pay[:, 2 + nt:3 + nt], iloc[:, nt:nt + 1],
                        -1.0, gkey[:, nt:nt + 1], op0=Alu.mult, op1=Alu.add)
                if last:
                    nc.scalar.copy(staged[:, half], pv[:, :])
            if last:
                nc.scalar.dma_start(out_logits[t], staged[:])

            if not last:
                emit_fillers(FILL_P)
                # transpose candidates -> [4, 128] and ship [16, 32] to the AG
                p_pa = ps_s.tile([4, 128], FP32, name="p_pa", tag="small")
                nc.tensor.transpose(p_pa[:], pay[:], ident[:])
                payT = sb2.tile([4, 128], FP32, name="payT", tag="payT")
                nc.scalar.copy(payT[:], p_pa[:])
                # staged copies after payT on the ACT queue: the AG trigger is
                # critical, the logits write has a full step of slack.
                for nt in range(2):
                    nc.scalar.copy(staged[:, 512 * nt:512 * (nt + 1)], pvs[nt][:, :])
                nc.scalar.dma_start(out_logits[t], staged[:])

                cc_in = dr.tile([16, 32], FP32, name="cc_in", tag="cc_in")
                cc_out = dr.tile([128, 32], FP32, name="cc_out", tag="cc_out",
                                 addr_space="Shared")
                # cc_in row = g*4 + f  <-  payT row f, free g*32+b
                nc.scalar.dma_start(
                    cc_in[:].rearrange("(g f) b -> f g b", g=4, f=4),
                    payT[:].rearrange("f (g b) -> f g b", g=4))
                nc.gpsimd.collective_compute(
                    "AllGather", Alu.bypass,
                    replica_groups=[list(range(N_CORES))],
                    ins=[cc_in[:]], outs=[cc_out[:]],
                )

            # ================= next-step gates: bias + h rounds =================
            if not last:
                pg = ps_g.tile([128, 512], FP32, name="pg", tag="pg")
                emit_gates_bias_h(pg, with_bias=False)
                emit_fillers(FILL_B)

                # ================= AG result: fold over 64 candidates ==========
                agb = sb2.tile([128, 32], FP32, name="agb", tag="agb")
                nc.sync.dma_start(agb[:], cc_out[:])
                p_ag = ps_s.tile([32, 128], FP32, name="p_ag", tag="small")
                nc.tensor.transpose(p_ag[:], agb[:], ident[:])
                emit_fillers(FILL_C)
                t32 = sb2.tile([32, 128], FP32, name="t32", tag="t32")
                nc.vector.tensor_copy(t32[:], p_ag[:])

                # col = r*16 + g*4 + f; candidate order (r, g, f) is global-idx
                # order, and key = BIG - gidx makes max pick the first occurrence.
                tv = t32[:].rearrange("p (r g f) -> p r g f", r=8, g=4, f=4)
                vals = tv[:, :, :, 0:2]
                keys = tv[:, :, :, 2:4]
                gv32 = sb2.tile([32, 1], FP32, name="gv32", tag="gv32")
                nc.vector.tensor_reduce(gv32[:], vals, axis=mybir.AxisListType.XYZ,
                                        op=Alu.max)
                eqt = sb2.tile([32, 64], FP32, name="eqt", tag="eqt")
                eqv = eqt[:].rearrange("p (r g f) -> p r g f", r=8, g=4, f=2)
                nc.vector.tensor_scalar(eqv, vals, gv32[:, 0:1], None,
                                        op0=Alu.is_equal)
                mselt = sb2.tile([32, 64], FP32, name="mselt", tag="mselt")
                mselv = mselt[:].rearrange("p (r g f) -> p r g f", r=8, g=4, f=2)
                nc.vector.tensor_tensor(mselv, eqv, keys, op=Alu.mult)
                m2r = sb2.tile([32, 1], FP32, name="m2r", tag="m2r")
                nc.vector.tensor_reduce(m2r[:], mselv, axis=mybir.AxisListType.XYZ,
                                        op=Alu.max)
                idxf = sb2.tile([32, 1], FP32, name="idxf", tag="idxf")
                nc.vector.tensor_scalar(idxf[:], m2r[:], -1.0, BIG,
                                        op0=Alu.mult, op1=Alu.add)
                idx32 = sb2.tile([32, 1], I32, name="idx32", tag="idx32")
                nc.vector.tensor_copy(idx32[:], idxf[:])

                # ================= G-table gather =================
                # G[v] = emb[v] @ Wih.T + b_gates, permuted to the psum gate
                # layout and split [hi | lo] bf16 on the host: the whole x-side
                # of the next step's gates matmul is one row gather.
                gx = sb2.tile([32, 4096], BF16, name="gx", tag="gx")
                nc.gpsimd.indirect_dma_start(
                    out=gx[:], out_offset=None, in_=emb[:],
                    in_offset=bass.IndirectOffsetOnAxis(ap=idx32[:, 0:1], axis=0),
                )

        for p in reversed(pools):
            p.release()


def host_prep(inputs):
    """Build per-core in_maps from the full problem inputs."""
    z = np.asarray(inputs["z"], np.float32)
    embedding = np.ascontiguousarray(np.asarray(inputs["embedding"], np.float32))
    Wh = np.asarray(inputs["Wh"], np.float32)
    bh = np.asarray(inputs["bh"], np.float32)
    Wc = np.asarray(inputs["Wc"], np.float32)
    bc = np.asarray(inputs["bc"], np.float32)
    Wih = np.asarray(inputs["Wih"], np.float32)
    Whh = np.asarray(inputs["Whh"], np.float32)
    bih = np.asarray(inputs["bih"], np.float32)
    bhh = np.asarray(inputs["bhh"], np.float32)
    Wfc = np.asarray(inputs["Wfc"], np.float32)
    bfc = np.asarray(inputs["bfc"], np.float32)

    h0 = (z @ Wh.T + bh).astype(np.float32)   # [B, H]
    c0 = (z @ Wc.T + bc).astype(np.float32)
    b_gates = (bih + bhh).astype(np.float32)  # [4H]

    # gate column permutation: c' = q*512 + slot*128 + hw with slot order
    # [i, f, o, g] so the sigmoid gates are one contiguous 384-wide range.
    cp = np.arange(2048)
    qq, rem = cp // 512, cp % 512
    slot, hw = rem // 128, rem % 128
    gate = np.array([0, 1, 3, 2])[slot]        # slot -> original gate (i,f,o,g)
    perm = gate * 512 + qq * 128 + hw          # original col index for permuted col c'
    Wall = np.concatenate([Wih, Whh], axis=1)  # [2048, 1024] (k = [x | h])
    Wperm = Wall[perm]                         # [2048, 1024]
    wgates = np.ascontiguousarray(Wperm.T)     # [1024, 2048]

    import ml_dtypes

    def split_bf16(w):
        hi = w.astype(ml_dtypes.bfloat16)
        lo = (w - hi.astype(np.float32)).astype(ml_dtypes.bfloat16)
        return np.ascontiguousarray(hi), np.ascontiguousarray(lo)

    wgates_hi, wgates_lo = split_bf16(wgates)
    bg_hi, bg_lo = split_bf16(b_gates[perm][None, :])
    bias_g2 = np.ascontiguousarray(np.concatenate([bg_hi, bg_lo], axis=0))  # [2, 2048]

    # state layout tiles
    h0t = np.zeros((128, 128), np.float32)     # h0t[p, q*32+b] = h0[b, 128q+p]
    c0t = np.zeros((128, 128), np.float32)     # c0t[32q+b, hw] = c0[b, 128q+hw]
    for q in range(4):
        h0t[:, 32 * q:32 * (q + 1)] = h0[:, 128 * q:128 * (q + 1)].T
        c0t[32 * q:32 * (q + 1), :] = c0[:, 128 * q:128 * (q + 1)]

    ident = np.eye(128, dtype=np.float32)
    identb = np.eye(32, dtype=ml_dtypes.bfloat16)
    ones2 = np.ones((2, 32), ml_dtypes.bfloat16)
    # G[v] = emb[v] @ Wih.T + b_gates in the permuted psum gate layout
    G = embedding @ wgates[0:512, :] + b_gates[perm][None, :]
    G_hi, G_lo = split_bf16(G.astype(np.float32))
    emb2 = np.ascontiguousarray(np.concatenate([G_hi, G_lo], axis=1))

    in_maps = []
    for j in range(N_CORES):
        shard = Wfc[VS * j:VS * (j + 1)]                    # [4000, 512]
        shard_p = np.zeros((VSP, H), np.float32)
        shard_p[:VS] = shard
        wfc_in = np.ascontiguousarray(shard_p.T)            # [512, 4096]
        wfc_hi, wfc_lo = split_bf16(wfc_in)
        bfc_p = np.full(VSP, -1e30, np.float32)
        bfc_p[:VS] = bfc[VS * j:VS * (j + 1)]
        bf_hi, bf_lo = split_bf16(bfc_p[None, :])
        bias_fc2 = np.ascontiguousarray(np.concatenate([bf_hi, bf_lo], axis=0))
        # gkey[p, nt] = BIG - (VS*j + 1024*(p//32) + 512*nt)
        gbase = VS * j + (np.arange(128) // 32) * 1024
        gkey = np.stack([BIG - gbase, BIG - gbase - 512], axis=1).astype(np.float32)
        in_maps.append({
            "wfc_hi": wfc_hi,
            "wfc_lo": wfc_lo,
            "wgates_hi": wgates_hi,
            "wgates_lo": wgates_lo,
            "bias_g2": bias_g2,
            "bias_fc2": bias_fc2,
            "gkey": np.ascontiguousarray(gkey),
            "ident": ident,
            "ones2": ones2,
            "identb": identb,
            "h0t": h0t,
            "c0": c0t,
            "emb": emb2,
        })
    return in_maps


def declare_io(nc, n_steps):
    io = {}
    io["wfc_hi"] = nc.dram_tensor("wfc_hi", [512, VSP], BF16, kind="ExternalInput").ap()
    io["wfc_lo"] = nc.dram_tensor("wfc_lo", [512, VSP], BF16, kind="ExternalInput").ap()
    io["wgates_hi"] = nc.dram_tensor("wgates_hi", [1024, 2048], BF16, kind="ExternalInput").ap()
    io["wgates_lo"] = nc.dram_tensor("wgates_lo", [1024, 2048], BF16, kind="ExternalInput").ap()
    io["bias_g2"] = nc.dram_tensor("bias_g2", [2, 2048], BF16, kind="ExternalInput").ap()
    io["bias_fc2"] = nc.dram_tensor("bias_fc2", [2, VSP], BF16, kind="ExternalInput").ap()
    io["gkey"] = nc.dram_tensor("gkey", [128, 2], FP32, kind="ExternalInput").ap()
    io["ident"] = nc.dram_tensor("ident", [128, 128], FP32, kind="ExternalInput").ap()
    io["ones2"] = nc.dram_tensor("ones2", [2, 32], BF16, kind="ExternalInput").ap()
    io["identb"] = nc.dram_tensor("identb", [32, 32], BF16, kind="ExternalInput").ap()
    io["h0t"] = nc.dram_tensor("h0t", [128, 128], FP32, kind="ExternalInput").ap()
    io["c0"] = nc.dram_tensor("c0", [128, 128], FP32, kind="ExternalInput").ap()
    io["emb"] = nc.dram_tensor("emb", [V, 4096], BF16, kind="ExternalInput").ap()
    io["logits"] = nc.dram_tensor("logits", [n_steps, 128, 1024], BF16,
                                  kind="ExternalOutput").ap()
    return io


_BUILT = {}


def build(n_steps=T):
    if n_steps in _BUILT:
        return _BUILT[n_steps]
    nc = bacc.Bacc("TRN2", target_bir_lowering=False, debug=False,
                   num_devices=N_CORES)
    io = declare_io(nc, n_steps)
    build_decoder(nc, io, n_steps)
    nc.compile()
    _BUILT[n_steps] = nc
    return nc


def assemble(results, n_steps=T):
    """results: list of per-core out dicts -> full [B, T, V] fp32."""
    full = np.empty((B, n_steps, V), np.float32)
    for j in range(N_CORES):
        arr = results[j]["logits"].astype(np.float32)
        arr = arr.reshape(n_steps, 4, 32, 1024)
        arr = arr.transpose(2, 0, 1, 3).reshape(B, n_steps, VSP)[:, :, :VS]
        full[:, :, VS * j:VS * (j + 1)] = arr
    return full


def kernel(**inputs):
    n_steps = int(inputs.get("context_length", T))
    assert n_steps == T, f"kernel hardcodes T={T}, got {n_steps}"
    nc = build(T)
    in_maps = host_prep(inputs)
    res = run_bass_kernel_spmd(nc, in_maps, core_ids=list(range(N_CORES)))
    return assemble(res.results, T)


if __name__ == "__main__":
    import reference
    inputs = reference.setup_inputs()
    out = kernel(**{k: np.asarray(v) if hasattr(v, "shape") else v
                    for k, v in inputs.items()})
    print("output shape:", out.shape)


# revision 16
# speedup vs baseline: 1.2549x; 1.0056x over previous
"""Trainium2 Bass kernel for nn_AutoregressiveDecoder (LSTM decoder w/ greedy sampling).

Strategy (8 NeuronCores, SPMD):
  - Vocab-shard the fc projection: core j holds Wfc rows [4000j, 4000(j+1)) padded to
    4096 (pad bias = -1e30), resident in SBUF.
  - LSTM weights replicated per core, SBUF-resident, gate columns permuted so that
    PE column-group q computes [i|f|g|o] for hidden quarter q -> full-partition
    elementwise state updates.
  - All matmuls bf16x3 (hi*hi + hi*lo + lo*hi, fp32 psum accumulate) with 4-way PE
    column tiling; fc bias folded into the matmul via a 2-row ones lhsT so the
    argmax reads finished logits straight out of PSUM.
  - Greedy token: per-half (512-wide) max8/max_index pipelined under the second
    half's matmul, 4 candidates/core AllGather'd ([16,32] payload), single
    multi-axis-reduce fold after the exchange, indirect-DMA embedding gather.
  - Filler matmuls (zero operands) span the AllGather window so the PE's HAM
    clock gate stays at full rate across the per-step collective stall.
  - Logits stream to DRAM as bf16 [T, 128, 1024] per core; host reassembles
    [B, T, V] in fp32 (output tolerance is 2e-2; bf16 staging halves the
    copy+DMA cost while the on-device argmax stays fp32-exact).
"""
import sys

sys.path.insert(0, "/opt/trn_rl_repo")

import numpy as np

import concourse.bass as bass
import concourse.bacc as bacc
import concourse.tile as tile
import concourse.mybir as mybir
from concourse.bass_utils import run_bass_kernel_spmd

FP32 = mybir.dt.float32
BF16 = mybir.dt.bfloat16
I32 = mybir.dt.int32
U32 = mybir.dt.uint32

N_CORES = 8
B, L, H, E, V, T = 32, 256, 512, 512, 32000, 64
VS = V // N_CORES          # 4000 true shard
VSP = 4096                 # padded shard
BIG = 65536.0
# bf16 filler matmuls (~215-430ns each) spanning the PE-idle windows of a step
# so the HAM clock gate never sees a low-duty window and re-throttles:
# A: activations/state chain, P: argmax tail before the pay transpose,
# B: AllGather wait, C: fold+gather+x-prep.
FILL_A, FILL_P, FILL_B, FILL_C = 14, 8, 26, 20

Sigmoid = mybir.ActivationFunctionType.Sigmoid
Tanh = mybir.ActivationFunctionType.Tanh
Alu = mybir.AluOpType


def build_decoder(nc, io, n_steps):
    """Emit the full unrolled decoder. io: dict name -> DRAM AP."""
    with tile.TileContext(nc) as tc:
        sb = tc.alloc_tile_pool(name="sb", bufs=1)
        sb2 = tc.alloc_tile_pool(name="sb2", bufs=3)
        ps_g = tc.alloc_tile_pool(name="ps_g", bufs=2, space="PSUM")
        ps_v = tc.alloc_tile_pool(name="ps_v", bufs=2, space="PSUM")
        ps_s = tc.alloc_tile_pool(name="ps_s", bufs=2, space="PSUM")
        dr = tc.alloc_tile_pool(name="dr", bufs=4, space="DRAM")
        pools = [sb, sb2, ps_g, ps_v, ps_s, dr]

        # ---- persistent SBUF state & weights ----
        wfh = [sb.tile([128, VSP], BF16, name=f"wfh{q}") for q in range(4)]
        wfl = [sb.tile([128, VSP], BF16, name=f"wfl{q}") for q in range(4)]
        wgh = [sb.tile([128, 2048], BF16, name=f"wgh{r}") for r in range(8)]
        wgl = [sb.tile([128, 2048], BF16, name=f"wgl{r}") for r in range(8)]
        bias_g2 = sb.tile([2, 2048], BF16, name="bias_g2")
        bias_fc2 = sb.tile([2, VSP], BF16, name="bias_fc2")
        gkey = sb.tile([128, 2], FP32, name="gkey")   # BIG - (VS*j + 1024*g + 512*nt)
        ident = sb.tile([128, 128], FP32, name="ident")
        ones2 = sb.tile([2, 32], BF16, name="ones2")
        zeros = sb.tile([128, 512], BF16, name="zeros")
        identb = sb.tile([32, 32], BF16, name="identb")
        hT = sb.tile([128, 128], FP32, name="hT")
        xTh = sb.tile([128, 128], BF16, name="xTh")
        xTl = sb.tile([128, 128], BF16, name="xTl")
        hTh = sb.tile([128, 128], BF16, name="hTh")
        hTl = sb.tile([128, 128], BF16, name="hTl")
        c_t = sb.tile([128, 128], FP32, name="c_t")

        for q in range(4):
            nc.sync.dma_start(wfh[q][:], io["wfc_hi"][128 * q:128 * (q + 1), :])
            nc.sync.dma_start(wfl[q][:], io["wfc_lo"][128 * q:128 * (q + 1), :])
        for r in range(8):
            nc.sync.dma_start(wgh[r][:], io["wgates_hi"][128 * r:128 * (r + 1), :])
            nc.sync.dma_start(wgl[r][:], io["wgates_lo"][128 * r:128 * (r + 1), :])
        nc.sync.dma_start(bias_g2[:], io["bias_g2"][:])
        nc.sync.dma_start(bias_fc2[:], io["bias_fc2"][:])
        nc.sync.dma_start(gkey[:], io["gkey"][:])
        nc.sync.dma_start(ident[:], io["ident"][:])
        nc.sync.dma_start(ones2[:], io["ones2"][:])
        nc.sync.dma_start(identb[:], io["identb"][:])
        nc.sync.dma_start(hT[:], io["h0t"][:])
        nc.sync.dma_start(c_t[:], io["c0"][:])
        nc.vector.memset(zeros[:], 0.0)
        # initial hi/lo splits of the (identical) x0 = h0 state
        nc.vector.tensor_copy(hTh[:], hT[:])
        nc.vector.tensor_tensor(hTl[:], hT[:], hTh[:], op=Alu.subtract)
        nc.vector.tensor_copy(xTh[:], hTh[:])
        nc.vector.tensor_copy(xTl[:], hTl[:])

        emb = io["emb"]
        out_logits = io["logits"]  # [T, 128, 1024] bf16

        # ---- gates matmul emission helpers ----
        # psum layout: partition 32q+b, free = gate*128+hw (cols permuted on host)
        def emit_gates_bias_h(pg, with_bias):
            # with_bias: step-0 form (bias wave carries start=True). Steps >=1
            # get the bias from the precomputed G table, so start moves to the
            # first h-round matmul of each column group.
            if with_bias:
                for g in range(4):
                    nc.tensor.matmul(
                        pg[32 * g:32 * (g + 1), :], lhsT=ones2[:, :],
                        rhs=bias_g2[:, 512 * g:512 * (g + 1)],
                        start=True, stop=False, tile_position=(0, 32 * g),
                        skip_group_check=True,
                    )
            emit_gates_rounds(pg, [4, 5, 6, 7], stop=False, start=not with_bias)

        def emit_gates_rounds(pg, rounds, stop, start=False):
            for r in rounds:
                hi, lo = (xTh, xTl) if r < 4 else (hTh, hTl)
                q = r % 4
                cs = slice(32 * q, 32 * (q + 1))
                for g in range(4):
                    gs = slice(512 * g, 512 * (g + 1))
                    out = pg[32 * g:32 * (g + 1), :]
                    passes = ((hi[:, cs], wgh[r][:, gs]),
                              (lo[:, cs], wgh[r][:, gs]),
                              (hi[:, cs], wgl[r][:, gs]))
                    for pi, (lhsT, rhs) in enumerate(passes):
                        nc.tensor.matmul(
                            out, lhsT=lhsT, rhs=rhs,
                            start=(start and r == rounds[0] and pi == 0),
                            stop=(stop and r == rounds[-1] and pi == 2),
                            tile_position=(0, 32 * g),
                            skip_group_check=True,
                        )

        def emit_gates_x(pg, gx):
            # inject the gathered per-token gates rows (hi|lo bf16) into the
            # psum accumulation via identity matmuls: 2 waves total.
            for w in range(2):
                for g in range(4):
                    nc.tensor.matmul(
                        pg[32 * g:32 * (g + 1), :], lhsT=identb[:],
                        rhs=gx[:, 2048 * w + 512 * g:2048 * w + 512 * (g + 1)],
                        start=False, stop=(w == 1),
                        tile_position=(0, 32 * g), skip_group_check=True,
                    )

        def emit_fillers(n):
            # bf16 matmuls over zeros: 512 cycles each of PE activity with
            # minimal switching power; results never read.
            p_fil = ps_s.tile([128, 512], FP32, name="p_fil", tag="small")
            for _ in range(n):
                nc.tensor.matmul(
                    p_fil[:], lhsT=zeros[:, 0:128], rhs=zeros[:],
                    start=True, stop=True, skip_group_check=True,
                )

        # step-0 gates: bias + h-rounds up front (x == h0 so all 8 rounds)
        pg = ps_g.tile([128, 512], FP32, name="pg", tag="pg")
        emit_gates_bias_h(pg, with_bias=True)

        for t in range(n_steps):
            last = t == n_steps - 1
            # ================= gates matmul: x contribution =================
            if t == 0:
                emit_gates_rounds(pg, [0, 1, 2, 3], stop=True)
            else:
                emit_gates_x(pg, gx)
            emit_fillers(FILL_A)

            # ================= activations / state =================
            # gate slots after host permutation: [i | f | o | g(tanh)]
            acts = sb2.tile([128, 512], FP32, name="acts", tag="acts")
            nc.scalar.activation(acts[:, 0:256], pg[:, 0:256], Sigmoid)
            nc.scalar.activation(acts[:, 384:512], pg[:, 384:512], Tanh)
            nc.scalar.activation(acts[:, 256:384], pg[:, 256:384], Sigmoid)
            nc.vector.tensor_tensor(c_t[:], acts[:, 128:256], c_t[:], op=Alu.mult)
            t1 = sb2.tile([128, 128], FP32, name="t1", tag="t1")
            nc.vector.tensor_tensor(t1[:], acts[:, 0:128], acts[:, 384:512], op=Alu.mult)
            nc.vector.tensor_tensor(c_t[:], c_t[:], t1[:], op=Alu.add)
            tanh_c = sb2.tile([128, 128], FP32, name="tanh_c", tag="tanh_c")
            nc.scalar.activation(tanh_c[:], c_t[:], Tanh)
            h_new = sb2.tile([128, 128], FP32, name="h_new", tag="h_new")
            nc.vector.tensor_tensor(h_new[:], acts[:, 256:384], tanh_c[:], op=Alu.mult)

            # hT = transpose(h_new); hi cast on ACT in parallel with fp32 copy on DVE
            p_ht = ps_s.tile([128, 128], FP32, name="p_ht", tag="small")
            nc.tensor.transpose(p_ht[:], h_new[:], ident[:])
            nc.scalar.copy(hTh[:], p_ht[:])
            nc.vector.tensor_copy(hT[:], p_ht[:])
            nc.vector.tensor_tensor(hTl[:], hT[:], hTh[:], op=Alu.subtract)

            # ================= vocab matmul (bias folded in) =================
            # psum layout: partition 32g+b (g = vocab quarter of shard); two
            # separate psum tiles per half so half-0's argmax reads don't WAR-
            # block half-1's matmuls under tile-granular dep tracking.
            pvs = [ps_v.tile([128, 512], FP32, name=f"pv{nt}", tag=f"pv{nt}")
                   for nt in range(2)]
            staged = sb2.tile([128, 1024], BF16, name="staged", tag="staged")
            v8 = [None, None]
            i8 = [None, None]
            for nt in range(2):
                pv = pvs[nt]
                for g in range(4):
                    ws = slice(1024 * g + 512 * nt, 1024 * g + 512 * (nt + 1))
                    nc.tensor.matmul(
                        pv[32 * g:32 * (g + 1), :],
                        lhsT=ones2[:, :], rhs=bias_fc2[:, ws],
                        start=True, stop=False, tile_position=(0, 32 * g),
                        skip_group_check=True,
                    )
                for q in range(4):
                    cs = slice(32 * q, 32 * (q + 1))
                    for g in range(4):
                        ws = slice(1024 * g + 512 * nt, 1024 * g + 512 * (nt + 1))
                        out = pv[32 * g:32 * (g + 1), :]
                        passes = ((hTh[:, cs], wfh[q][:, ws]),
                                  (hTl[:, cs], wfh[q][:, ws]),
                                  (hTh[:, cs], wfl[q][:, ws]))
                        for pi, (lhsT, rhs) in enumerate(passes):
                            nc.tensor.matmul(
                                out, lhsT=lhsT, rhs=rhs,
                                start=False,
                                stop=(q == 3 and pi == 2),
                                tile_position=(0, 32 * g),
                                skip_group_check=True,
                            )
                # candidate first (critical path), then stage to DRAM (bf16);
                # the nt=0 chain runs on ACT/DVE under the nt=1 matmul.
                # pay rows (per psum partition 32g+b): [v_a, v_b, key_a, key_b]
                # where key = BIG - global_idx (so keys never collide with
                # logit values in the eq-fold, and max(key) = min global idx).
                half = slice(512 * nt, 512 * (nt + 1))
                if not last:
                    if nt == 0:
                        pay = sb2.tile([128, 4], FP32, name="pay", tag="pay")
                        iloc = sb2.tile([128, 2], FP32, name="iloc", tag="iloc")
                    v8[nt] = sb2.tile([128, 8], FP32, name=f"v8{nt}", tag=f"v8{nt}")
                    i8[nt] = sb2.tile([128, 8], U32, name=f"i8{nt}", tag=f"i8{nt}")
                    nc.vector.max(v8[nt][:], pv[:, :])
                    nc.vector.max_index(i8[nt][:], v8[nt][:], pv[:, :])
                    nc.vector.tensor_copy(pay[:, nt:nt + 1], v8[nt][:, 0:1])
                    nc.vector.tensor_copy(iloc[:, nt:nt + 1], i8[nt][:, 0:1])
                    nc.vector.tensor_scalar(
                        pay[:, 2 + nt:3 + nt], iloc[:, nt:nt + 1],
                        -1.0, gkey[:, nt:nt + 1], op0=Alu.mult, op1=Alu.add)
                if last:
                    nc.scalar.copy(staged[:, half], pv[:, :])
            if last:
                nc.scalar.dma_start(out_logits[t], staged[:])

            if not last:
                emit_fillers(FILL_P)
                # transpose candidates -> [4, 128] and ship [16, 32] to the AG
                p_pa = ps_s.tile([4, 128], FP32, name="p_pa", tag="small")
                nc.tensor.transpose(p_pa[:], pay[:], ident[:])
                payT = sb2.tile([4, 128], FP32, name="payT", tag="payT")
                nc.scalar.copy(payT[:], p_pa[:])
                cc_in = dr.tile([16, 32], FP32, name="cc_in", tag="cc_in")
                cc_out = dr.tile([128, 32], FP32, name="cc_out", tag="cc_out",
                                 addr_space="Shared")
                # cc_in row = g*4 + f  <-  payT row f, free g*32+b; issued on
                # the ACT ring right behind the payT copy (no cross-engine hop,
                # and ahead of the slack-rich staged/logits traffic).
                nc.scalar.dma_start(
                    cc_in[:].rearrange("(g f) b -> f g b", g=4, f=4),
                    payT[:].rearrange("f (g b) -> f g b", g=4))
                for nt in range(2):
                    nc.scalar.copy(staged[:, 512 * nt:512 * (nt + 1)], pvs[nt][:, :])
                nc.scalar.dma_start(out_logits[t], staged[:])
                nc.gpsimd.collective_compute(
                    "AllGather", Alu.bypass,
                    replica_groups=[list(range(N_CORES))],
                    ins=[cc_in[:]], outs=[cc_out[:]],
                )

            # ================= next-step gates: bias + h rounds =================
            if not last:
                pg = ps_g.tile([128, 512], FP32, name="pg", tag="pg")
                emit_gates_bias_h(pg, with_bias=False)
                emit_fillers(FILL_B)

                # ================= AG result: fold over 64 candidates ==========
                agb = sb2.tile([128, 32], FP32, name="agb", tag="agb")
                nc.sync.dma_start(agb[:], cc_out[:])
                p_ag = ps_s.tile([32, 128], FP32, name="p_ag", tag="small")
                nc.tensor.transpose(p_ag[:], agb[:], ident[:])
                emit_fillers(FILL_C)
                t32 = sb2.tile([32, 128], FP32, name="t32", tag="t32")
                nc.vector.tensor_copy(t32[:], p_ag[:])

                # col = r*16 + g*4 + f; candidate order (r, g, f) is global-idx
                # order, and key = BIG - gidx makes max pick the first occurrence.
                tv = t32[:].rearrange("p (r g f) -> p r g f", r=8, g=4, f=4)
                vals = tv[:, :, :, 0:2]
                keys = tv[:, :, :, 2:4]
                gv32 = sb2.tile([32, 1], FP32, name="gv32", tag="gv32")
                nc.vector.tensor_reduce(gv32[:], vals, axis=mybir.AxisListType.XYZ,
                                        op=Alu.max)
                eqt = sb2.tile([32, 64], FP32, name="eqt", tag="eqt")
                eqv = eqt[:].rearrange("p (r g f) -> p r g f", r=8, g=4, f=2)
                nc.vector.tensor_scalar(eqv, vals, gv32[:, 0:1], None,
                                        op0=Alu.is_equal)
                mselt = sb2.tile([32, 64], FP32, name="mselt", tag="mselt")
                mselv = mselt[:].rearrange("p (r g f) -> p r g f", r=8, g=4, f=2)
                nc.vector.tensor_tensor(mselv, eqv, keys, op=Alu.mult)
                m2r = sb2.tile([32, 1], FP32, name="m2r", tag="m2r")
                nc.vector.tensor_reduce(m2r[:], mselv, axis=mybir.AxisListType.XYZ,
                                        op=Alu.max)
                idxf = sb2.tile([32, 1], FP32, name="idxf", tag="idxf")
                nc.vector.tensor_scalar(idxf[:], m2r[:], -1.0, BIG,
                                        op0=Alu.mult, op1=Alu.add)
                idx32 = sb2.tile([32, 1], I32, name="idx32", tag="idx32")
                nc.vector.tensor_copy(idx32[:], idxf[:])

                # ================= G-table gather =================
                # G[v] = emb[v] @ Wih.T + b_gates, permuted to the psum gate
                # layout and split [hi | lo] bf16 on the host: the whole x-side
                # of the next step's gates matmul is one row gather.
                gx = sb2.tile([32, 4096], BF16, name="gx", tag="gx")
                nc.gpsimd.indirect_dma_start(
                    out=gx[:], out_offset=None, in_=emb[:],
                    in_offset=bass.IndirectOffsetOnAxis(ap=idx32[:, 0:1], axis=0),
                )

        for p in reversed(pools):
            p.release()


def host_prep(inputs):
    """Build per-core in_maps from the full problem inputs."""
    z = np.asarray(inputs["z"], np.float32)
    embedding = np.ascontiguousarray(np.asarray(inputs["embedding"], np.float32))
    Wh = np.asarray(inputs["Wh"], np.float32)
    bh = np.asarray(inputs["bh"], np.float32)
    Wc = np.asarray(inputs["Wc"], np.float32)
    bc = np.asarray(inputs["bc"], np.float32)
    Wih = np.asarray(inputs["Wih"], np.float32)
    Whh = np.asarray(inputs["Whh"], np.float32)
    bih = np.asarray(inputs["bih"], np.float32)
    bhh = np.asarray(inputs["bhh"], np.float32)
    Wfc = np.asarray(inputs["Wfc"], np.float32)
    bfc = np.asarray(inputs["bfc"], np.float32)

    h0 = (z @ Wh.T + bh).astype(np.float32)   # [B, H]
    c0 = (z @ Wc.T + bc).astype(np.float32)
    b_gates = (bih + bhh).astype(np.float32)  # [4H]

    # gate column permutation: c' = q*512 + slot*128 + hw with slot order
    # [i, f, o, g] so the sigmoid gates are one contiguous 384-wide range.
    cp = np.arange(2048)
    qq, rem = cp // 512, cp % 512
    slot, hw = rem // 128, rem % 128
    gate = np.array([0, 1, 3, 2])[slot]        # slot -> original gate (i,f,o,g)
    perm = gate * 512 + qq * 128 + hw          # original col index for permuted col c'
    Wall = np.concatenate([Wih, Whh], axis=1)  # [2048, 1024] (k = [x | h])
    Wperm = Wall[perm]                         # [2048, 1024]
    wgates = np.ascontiguousarray(Wperm.T)     # [1024, 2048]

    import ml_dtypes

    def split_bf16(w):
        hi = w.astype(ml_dtypes.bfloat16)
        lo = (w - hi.astype(np.float32)).astype(ml_dtypes.bfloat16)
        return np.ascontiguousarray(hi), np.ascontiguousarray(lo)

    wgates_hi, wgates_lo = split_bf16(wgates)
    bg_hi, bg_lo = split_bf16(b_gates[perm][None, :])
    bias_g2 = np.ascontiguousarray(np.concatenate([bg_hi, bg_lo], axis=0))  # [2, 2048]

    # state layout tiles
    h0t = np.zeros((128, 128), np.float32)     # h0t[p, q*32+b] = h0[b, 128q+p]
    c0t = np.zeros((128, 128), np.float32)     # c0t[32q+b, hw] = c0[b, 128q+hw]
    for q in range(4):
        h0t[:, 32 * q:32 * (q + 1)] = h0[:, 128 * q:128 * (q + 1)].T
        c0t[32 * q:32 * (q + 1), :] = c0[:, 128 * q:128 * (q + 1)]

    ident = np.eye(128, dtype=np.float32)
    identb = np.eye(32, dtype=ml_dtypes.bfloat16)
    ones2 = np.ones((2, 32), ml_dtypes.bfloat16)
    # G[v] = emb[v] @ Wih.T + b_gates in the permuted psum gate layout
    G = embedding @ wgates[0:512, :] + b_gates[perm][None, :]
    G_hi, G_lo = split_bf16(G.astype(np.float32))
    emb2 = np.ascontiguousarray(np.concatenate([G_hi, G_lo], axis=1))

    in_maps = []
    for j in range(N_CORES):
        shard = Wfc[VS * j:VS * (j + 1)]                    # [4000, 512]
        shard_p = np.zeros((VSP, H), np.float32)
        shard_p[:VS] = shard
        wfc_in = np.ascontiguousarray(shard_p.T)            # [512, 4096]
        wfc_hi, wfc_lo = split_bf16(wfc_in)
        bfc_p = np.full(VSP, -1e30, np.float32)
        bfc_p[:VS] = bfc[VS * j:VS * (j + 1)]
        bf_hi, bf_lo = split_bf16(bfc_p[None, :])
        bias_fc2 = np.ascontiguousarray(np.concatenate([bf_hi, bf_lo], axis=0))
        # gkey[p, nt] = BIG - (VS*j + 1024*(p//32) + 512*nt)
        gbase = VS * j + (np.arange(128) // 32) * 1024
        gkey = np.stack([BIG - gbase, BIG - gbase - 512], axis=1).astype(np.float32)
        in_maps.append({
            "wfc_hi": wfc_hi,
            "wfc_lo": wfc_lo,
            "wgates_hi": wgates_hi,
            "wgates_lo": wgates_lo,
            "bias_g2": bias_g2,
            "bias_fc2": bias_fc2,
            "gkey": np.ascontiguousarray(gkey),
            "ident": ident,
            "ones2": ones2,
            "identb": identb,
            "h0t": h0t,
            "c0": c0t,
            "emb": emb2,
        })
    return in_maps


def declare_io(nc, n_steps):
    io = {}
    io["wfc_hi"] = nc.dram_tensor("wfc_hi", [512, VSP], BF16, kind="ExternalInput").ap()
    io["wfc_lo"] = nc.dram_tensor("wfc_lo", [512, VSP], BF16, kind="ExternalInput").ap()
    io["wgates_hi"] = nc.dram_tensor("wgates_hi", [1024, 2048], BF16, kind="ExternalInput").ap()
    io["wgates_lo"] = nc.dram_tensor("wgates_lo", [1024, 2048], BF16, kind="ExternalInput").ap()
    io["bias_g2"] = nc.dram_tensor("bias_g2", [2, 2048], BF16, kind="ExternalInput").ap()
    io["bias_fc2"] = nc.dram_tensor("bias_fc2", [2, VSP], BF16, kind="ExternalInput").ap()
    io["gkey"] = nc.dram_tensor("gkey", [128, 2], FP32, kind="ExternalInput").ap()
    io["ident"] = nc.dram_tensor("ident", [128, 128], FP32, kind="ExternalInput").ap()
    io["ones2"] = nc.dram_tensor("ones2", [2, 32], BF16, kind="ExternalInput").ap()
    io["identb"] = nc.dram_tensor("identb", [32, 32], BF16, kind="ExternalInput").ap()
    io["h0t"] = nc.dram_tensor("h0t", [128, 128], FP32, kind="ExternalInput").ap()
    io["c0"] = nc.dram_tensor("c0", [128, 128], FP32, kind="ExternalInput").ap()
    io["emb"] = nc.dram_tensor("emb", [V, 4096], BF16, kind="ExternalInput").ap()
    io["logits"] = nc.dram_tensor("logits", [n_steps, 128, 1024], BF16,
                                  kind="ExternalOutput").ap()
    return io


_BUILT = {}


def build(n_steps=T):
    if n_steps in _BUILT:
        return _BUILT[n_steps]
    nc = bacc.Bacc("TRN2", target_bir_lowering=False, debug=False,
                   num_devices=N_CORES)
    io = declare_io(nc, n_steps)
    build_decoder(nc, io, n_steps)
    nc.compile()
    _BUILT[n_steps] = nc
    return nc


def assemble(results, n_steps=T):
    """results: list of per-core out dicts -> full [B, T, V] fp32."""
    full = np.empty((B, n_steps, V), np.float32)
    for j in range(N_CORES):
        arr = results[j]["logits"].astype(np.float32)
        arr = arr.reshape(n_steps, 4, 32, 1024)
        arr = arr.transpose(2, 0, 1, 3).reshape(B, n_steps, VSP)[:, :, :VS]
        full[:, :, VS * j:VS * (j + 1)] = arr
    return full


def kernel(**inputs):
    n_steps = int(inputs.get("context_length", T))
    assert n_steps == T, f"kernel hardcodes T={T}, got {n_steps}"
    nc = build(T)
    in_maps = host_prep(inputs)
    res = run_bass_kernel_spmd(nc, in_maps, core_ids=list(range(N_CORES)))
    return assemble(res.results, T)


if __name__ == "__main__":
    import reference
    inputs = reference.setup_inputs()
    out = kernel(**{k: np.asarray(v) if hasattr(v, "shape") else v
                    for k, v in inputs.items()})
    print("output shape:", out.shape)


# revision 20
# speedup vs baseline: 1.2702x; 1.0122x over previous
"""Trainium2 Bass kernel for nn_AutoregressiveDecoder (LSTM decoder w/ greedy sampling).

Strategy (8 NeuronCores, SPMD):
  - Vocab-shard the fc projection: core j holds Wfc rows [4000j, 4000(j+1)) padded to
    4096 (pad bias = -1e30), resident in SBUF.
  - LSTM weights replicated per core, SBUF-resident, gate columns permuted so that
    PE column-group q computes [i|f|g|o] for hidden quarter q -> full-partition
    elementwise state updates.
  - All matmuls bf16x3 (hi*hi + hi*lo + lo*hi, fp32 psum accumulate) with 4-way PE
    column tiling; fc bias folded into the matmul via a 2-row ones lhsT so the
    argmax reads finished logits straight out of PSUM.
  - Greedy token: per-half (512-wide) max8/max_index pipelined under the second
    half's matmul, 4 candidates/core AllGather'd ([16,32] payload), single
    multi-axis-reduce fold after the exchange, indirect-DMA embedding gather.
  - Filler matmuls (zero operands) span the AllGather window so the PE's HAM
    clock gate stays at full rate across the per-step collective stall.
  - Logits stream to DRAM as bf16 [T, 128, 1024] per core; host reassembles
    [B, T, V] in fp32 (output tolerance is 2e-2; bf16 staging halves the
    copy+DMA cost while the on-device argmax stays fp32-exact).
"""
import sys

sys.path.insert(0, "/opt/trn_rl_repo")

import numpy as np

import concourse.bass as bass
import concourse.bacc as bacc
import concourse.tile as tile
import concourse.mybir as mybir
from concourse.bass_utils import run_bass_kernel_spmd

FP32 = mybir.dt.float32
BF16 = mybir.dt.bfloat16
I32 = mybir.dt.int32
U32 = mybir.dt.uint32

N_CORES = 8
B, L, H, E, V, T = 32, 256, 512, 512, 32000, 64
VS = V // N_CORES          # 4000 true shard
VSP = 4096                 # padded shard
BIG = 65536.0
# bf16 filler matmuls (~215-430ns each) spanning the PE-idle windows of a step
# so the HAM clock gate never sees a low-duty window and re-throttles:
# A: activations/state chain, P: argmax tail before the pay transpose,
# B: AllGather wait, C: fold+gather+x-prep.
FILL_A, FILL_P, FILL_B, FILL_C = 10, 8, 30, 20

Sigmoid = mybir.ActivationFunctionType.Sigmoid
Tanh = mybir.ActivationFunctionType.Tanh
Alu = mybir.AluOpType


def build_decoder(nc, io, n_steps):
    """Emit the full unrolled decoder. io: dict name -> DRAM AP."""
    with tile.TileContext(nc) as tc:
        sb = tc.alloc_tile_pool(name="sb", bufs=1)
        sb2 = tc.alloc_tile_pool(name="sb2", bufs=3)
        ps_g = tc.alloc_tile_pool(name="ps_g", bufs=2, space="PSUM")
        ps_v = tc.alloc_tile_pool(name="ps_v", bufs=2, space="PSUM")
        ps_s = tc.alloc_tile_pool(name="ps_s", bufs=2, space="PSUM")
        dr = tc.alloc_tile_pool(name="dr", bufs=4, space="DRAM")
        pools = [sb, sb2, ps_g, ps_v, ps_s, dr]

        # ---- persistent SBUF state & weights ----
        wfh = [sb.tile([128, VSP], BF16, name=f"wfh{q}") for q in range(4)]
        wfl = [sb.tile([128, VSP], BF16, name=f"wfl{q}") for q in range(4)]
        wgh = [sb.tile([128, 2048], BF16, name=f"wgh{r}") for r in range(8)]
        wgl = [sb.tile([128, 2048], BF16, name=f"wgl{r}") for r in range(8)]
        bias_g2 = sb.tile([2, 2048], BF16, name="bias_g2")
        bias_fc2 = sb.tile([2, VSP], BF16, name="bias_fc2")
        gkey = sb.tile([128, 2], FP32, name="gkey")   # BIG - (VS*j + 1024*g + 512*nt)
        ident = sb.tile([128, 128], FP32, name="ident")
        ones2 = sb.tile([2, 32], BF16, name="ones2")
        zeros = sb.tile([128, 512], BF16, name="zeros")
        identb = sb.tile([32, 32], BF16, name="identb")
        hT = sb.tile([128, 128], FP32, name="hT")
        xTh = sb.tile([128, 128], BF16, name="xTh")
        xTl = sb.tile([128, 128], BF16, name="xTl")
        hTh = sb.tile([128, 128], BF16, name="hTh")
        hTl = sb.tile([128, 128], BF16, name="hTl")
        c_t = sb.tile([128, 128], FP32, name="c_t")

        for q in range(4):
            nc.sync.dma_start(wfh[q][:], io["wfc_hi"][128 * q:128 * (q + 1), :])
            nc.sync.dma_start(wfl[q][:], io["wfc_lo"][128 * q:128 * (q + 1), :])
        for r in range(8):
            nc.sync.dma_start(wgh[r][:], io["wgates_hi"][128 * r:128 * (r + 1), :])
            nc.sync.dma_start(wgl[r][:], io["wgates_lo"][128 * r:128 * (r + 1), :])
        nc.sync.dma_start(bias_g2[:], io["bias_g2"][:])
        nc.sync.dma_start(bias_fc2[:], io["bias_fc2"][:])
        nc.sync.dma_start(gkey[:], io["gkey"][:])
        nc.sync.dma_start(ident[:], io["ident"][:])
        nc.sync.dma_start(ones2[:], io["ones2"][:])
        nc.sync.dma_start(identb[:], io["identb"][:])
        nc.sync.dma_start(hT[:], io["h0t"][:])
        nc.sync.dma_start(c_t[:], io["c0"][:])
        nc.vector.memset(zeros[:], 0.0)
        # initial hi/lo splits of the (identical) x0 = h0 state
        nc.vector.tensor_copy(hTh[:], hT[:])
        nc.vector.tensor_tensor(hTl[:], hT[:], hTh[:], op=Alu.subtract)
        nc.vector.tensor_copy(xTh[:], hTh[:])
        nc.vector.tensor_copy(xTl[:], hTl[:])

        emb = io["emb"]
        out_logits = io["logits"]  # [T, 128, 1024] bf16

        # ---- gates matmul emission helpers ----
        # psum layout: partition 32q+b, free = gate*128+hw (cols permuted on host)
        def emit_gates_bias_h(pg, with_bias):
            # with_bias: step-0 form (bias wave carries start=True). Steps >=1
            # get the bias from the precomputed G table, so start moves to the
            # first h-round matmul of each column group.
            if with_bias:
                for g in range(4):
                    nc.tensor.matmul(
                        pg[32 * g:32 * (g + 1), :], lhsT=ones2[:, :],
                        rhs=bias_g2[:, 512 * g:512 * (g + 1)],
                        start=True, stop=False, tile_position=(0, 32 * g),
                        skip_group_check=True,
                    )
            emit_gates_rounds(pg, [4, 5, 6, 7], stop=False, start=not with_bias)

        def emit_gates_rounds(pg, rounds, stop, start=False):
            for r in rounds:
                hi, lo = (xTh, xTl) if r < 4 else (hTh, hTl)
                q = r % 4
                cs = slice(32 * q, 32 * (q + 1))
                for g in range(4):
                    gs = slice(512 * g, 512 * (g + 1))
                    out = pg[32 * g:32 * (g + 1), :]
                    passes = ((hi[:, cs], wgh[r][:, gs]),
                              (lo[:, cs], wgh[r][:, gs]),
                              (hi[:, cs], wgl[r][:, gs]))
                    for pi, (lhsT, rhs) in enumerate(passes):
                        nc.tensor.matmul(
                            out, lhsT=lhsT, rhs=rhs,
                            start=(start and r == rounds[0] and pi == 0),
                            stop=(stop and r == rounds[-1] and pi == 2),
                            tile_position=(0, 32 * g),
                            skip_group_check=True,
                        )

        def emit_gates_x(pg, gx):
            # inject the gathered per-token gates rows (hi|lo bf16) into the
            # psum accumulation via identity matmuls: 2 waves total.
            for w in range(2):
                for g in range(4):
                    nc.tensor.matmul(
                        pg[32 * g:32 * (g + 1), :], lhsT=identb[:],
                        rhs=gx[:, 2048 * w + 512 * g:2048 * w + 512 * (g + 1)],
                        start=False, stop=(w == 1),
                        tile_position=(0, 32 * g), skip_group_check=True,
                    )

        def emit_fillers(n):
            # bf16 matmuls over zeros: 512 cycles each of PE activity with
            # minimal switching power; results never read.
            p_fil = ps_s.tile([128, 512], FP32, name="p_fil", tag="small")
            for _ in range(n):
                nc.tensor.matmul(
                    p_fil[:], lhsT=zeros[:, 0:128], rhs=zeros[:],
                    start=True, stop=True, skip_group_check=True,
                )

        # step-0 gates: bias + h-rounds up front (x == h0 so all 8 rounds)
        pg = ps_g.tile([128, 512], FP32, name="pg", tag="pg")
        emit_gates_bias_h(pg, with_bias=True)

        for t in range(n_steps):
            last = t == n_steps - 1
            # ================= gates matmul: x contribution =================
            if t == 0:
                emit_gates_rounds(pg, [0, 1, 2, 3], stop=True)
            else:
                emit_gates_x(pg, gx)
            emit_fillers(FILL_A)

            # ================= activations / state =================
            # gate slots after host permutation: [i | f | o | g(tanh)]
            acts = sb2.tile([128, 512], FP32, name="acts", tag="acts")
            nc.scalar.activation(acts[:, 0:256], pg[:, 0:256], Sigmoid)
            nc.scalar.activation(acts[:, 384:512], pg[:, 384:512], Tanh)
            nc.scalar.activation(acts[:, 256:384], pg[:, 256:384], Sigmoid)
            nc.vector.tensor_tensor(c_t[:], acts[:, 128:256], c_t[:], op=Alu.mult)
            t1 = sb2.tile([128, 128], FP32, name="t1", tag="t1")
            nc.vector.tensor_tensor(t1[:], acts[:, 0:128], acts[:, 384:512], op=Alu.mult)
            nc.vector.tensor_tensor(c_t[:], c_t[:], t1[:], op=Alu.add)
            tanh_c = sb2.tile([128, 128], FP32, name="tanh_c", tag="tanh_c")
            nc.scalar.activation(tanh_c[:], c_t[:], Tanh)
            h_new = sb2.tile([128, 128], FP32, name="h_new", tag="h_new")
            nc.vector.tensor_tensor(h_new[:], acts[:, 256:384], tanh_c[:], op=Alu.mult)

            # hT = transpose(h_new); hi cast on ACT in parallel with fp32 copy on DVE
            p_ht = ps_s.tile([128, 128], FP32, name="p_ht", tag="small")
            nc.tensor.transpose(p_ht[:], h_new[:], ident[:])
            nc.scalar.copy(hTh[:], p_ht[:])
            nc.vector.tensor_copy(hT[:], p_ht[:])
            nc.vector.tensor_tensor(hTl[:], hT[:], hTh[:], op=Alu.subtract)

            # ================= vocab matmul (bias folded in) =================
            # psum layout: partition 32g+b (g = vocab quarter of shard); two
            # separate psum tiles per half so half-0's argmax reads don't WAR-
            # block half-1's matmuls under tile-granular dep tracking.
            pvs = [ps_v.tile([128, 512], FP32, name=f"pv{nt}", tag=f"pv{nt}")
                   for nt in range(2)]
            staged = sb2.tile([128, 1024], BF16, name="staged", tag="staged")
            v8 = [None, None]
            i8 = [None, None]
            for nt in range(2):
                pv = pvs[nt]
                for g in range(4):
                    ws = slice(1024 * g + 512 * nt, 1024 * g + 512 * (nt + 1))
                    nc.tensor.matmul(
                        pv[32 * g:32 * (g + 1), :],
                        lhsT=ones2[:, :], rhs=bias_fc2[:, ws],
                        start=True, stop=False, tile_position=(0, 32 * g),
                        skip_group_check=True,
                    )
                for q in range(4):
                    cs = slice(32 * q, 32 * (q + 1))
                    for g in range(4):
                        ws = slice(1024 * g + 512 * nt, 1024 * g + 512 * (nt + 1))
                        out = pv[32 * g:32 * (g + 1), :]
                        passes = ((hTh[:, cs], wfh[q][:, ws]),
                                  (hTl[:, cs], wfh[q][:, ws]),
                                  (hTh[:, cs], wfl[q][:, ws]))
                        for pi, (lhsT, rhs) in enumerate(passes):
                            nc.tensor.matmul(
                                out, lhsT=lhsT, rhs=rhs,
                                start=False,
                                stop=(q == 3 and pi == 2),
                                tile_position=(0, 32 * g),
                                skip_group_check=True,
                            )
                # candidate first (critical path), then stage to DRAM (bf16);
                # the nt=0 chain runs on ACT/DVE under the nt=1 matmul.
                # pay rows (per psum partition 32g+b): [v_a, v_b, key_a, key_b]
                # where key = BIG - global_idx (so keys never collide with
                # logit values in the eq-fold, and max(key) = min global idx).
                half = slice(512 * nt, 512 * (nt + 1))
                if not last:
                    if nt == 0:
                        pay = sb2.tile([128, 4], FP32, name="pay", tag="pay")
                        iloc = sb2.tile([128, 2], FP32, name="iloc", tag="iloc")
                    v8[nt] = sb2.tile([128, 8], FP32, name=f"v8{nt}", tag=f"v8{nt}")
                    i8[nt] = sb2.tile([128, 8], U32, name=f"i8{nt}", tag=f"i8{nt}")
                    nc.vector.max(v8[nt][:], pv[:, :])
                    nc.vector.max_index(i8[nt][:], v8[nt][:], pv[:, :])
                    nc.vector.tensor_copy(pay[:, nt:nt + 1], v8[nt][:, 0:1])
                    nc.vector.tensor_copy(iloc[:, nt:nt + 1], i8[nt][:, 0:1])
                    nc.vector.tensor_scalar(
                        pay[:, 2 + nt:3 + nt], iloc[:, nt:nt + 1],
                        -1.0, gkey[:, nt:nt + 1], op0=Alu.mult, op1=Alu.add)
                if last:
                    nc.scalar.copy(staged[:, half], pv[:, :])
            if last:
                nc.scalar.dma_start(out_logits[t], staged[:])

            if not last:
                emit_fillers(FILL_P)
                # transpose candidates -> [4, 128] and ship [16, 32] to the AG
                p_pa = ps_s.tile([4, 128], FP32, name="p_pa", tag="small")
                nc.tensor.transpose(p_pa[:], pay[:], ident[:])
                payT = sb2.tile([4, 128], FP32, name="payT", tag="payT")
                nc.scalar.copy(payT[:], p_pa[:])
                cc_in = dr.tile([16, 32], FP32, name="cc_in", tag="cc_in")
                cc_out = dr.tile([128, 32], FP32, name="cc_out", tag="cc_out",
                                 addr_space="Shared")
                # cc_in row = g*4 + f  <-  payT row f, free g*32+b; issued on
                # the ACT ring right behind the payT copy (no cross-engine hop,
                # and ahead of the slack-rich staged/logits traffic).
                nc.scalar.dma_start(
                    cc_in[:].rearrange("(g f) b -> f g b", g=4, f=4),
                    payT[:].rearrange("f (g b) -> f g b", g=4))
                for nt in range(2):
                    nc.scalar.copy(staged[:, 512 * nt:512 * (nt + 1)], pvs[nt][:, :])
                nc.scalar.dma_start(out_logits[t], staged[:])
                nc.gpsimd.collective_compute(
                    "AllGather", Alu.bypass,
                    replica_groups=[list(range(N_CORES))],
                    ins=[cc_in[:]], outs=[cc_out[:]],
                )

            # ================= next-step gates: bias + h rounds =================
            if not last:
                pg = ps_g.tile([128, 512], FP32, name="pg", tag="pg")
                emit_gates_bias_h(pg, with_bias=False)
                emit_fillers(FILL_B)

                # ================= AG result: fold over 64 candidates ==========
                agb = sb2.tile([128, 32], FP32, name="agb", tag="agb")
                nc.sync.dma_start(agb[:], cc_out[:])
                p_ag = ps_s.tile([32, 128], FP32, name="p_ag", tag="small")
                nc.tensor.transpose(p_ag[:], agb[:], ident[:])
                emit_fillers(FILL_C)
                t32 = sb2.tile([32, 128], FP32, name="t32", tag="t32")
                nc.vector.tensor_copy(t32[:], p_ag[:])

                # col = r*16 + g*4 + f; candidate order (r, g, f) is global-idx
                # order, and key = BIG - gidx makes max pick the first occurrence.
                tv = t32[:].rearrange("p (r g f) -> p r g f", r=8, g=4, f=4)
                vals = tv[:, :, :, 0:2]
                keys = tv[:, :, :, 2:4]
                gv32 = sb2.tile([32, 1], FP32, name="gv32", tag="gv32")
                nc.vector.tensor_reduce(gv32[:], vals, axis=mybir.AxisListType.XYZ,
                                        op=Alu.max)
                eqt = sb2.tile([32, 64], FP32, name="eqt", tag="eqt")
                eqv = eqt[:].rearrange("p (r g f) -> p r g f", r=8, g=4, f=2)
                nc.vector.tensor_scalar(eqv, vals, gv32[:, 0:1], None,
                                        op0=Alu.is_equal)
                mselt = sb2.tile([32, 64], FP32, name="mselt", tag="mselt")
                mselv = mselt[:].rearrange("p (r g f) -> p r g f", r=8, g=4, f=2)
                nc.vector.tensor_tensor(mselv, eqv, keys, op=Alu.mult)
                m2r = sb2.tile([32, 1], FP32, name="m2r", tag="m2r")
                nc.vector.tensor_reduce(m2r[:], mselv, axis=mybir.AxisListType.XYZ,
                                        op=Alu.max)
                idxf = sb2.tile([32, 1], FP32, name="idxf", tag="idxf")
                nc.vector.tensor_scalar(idxf[:], m2r[:], -1.0, BIG,
                                        op0=Alu.mult, op1=Alu.add)
                idx32 = sb2.tile([32, 1], I32, name="idx32", tag="idx32")
                nc.vector.tensor_copy(idx32[:], idxf[:])

                # ================= G-table gather =================
                # G[v] = emb[v] @ Wih.T + b_gates, permuted to the psum gate
                # layout and split [hi | lo] bf16 on the host: the whole x-side
                # of the next step's gates matmul is one row gather.
                gx = sb2.tile([32, 4096], BF16, name="gx", tag="gx")
                nc.gpsimd.indirect_dma_start(
                    out=gx[:], out_offset=None, in_=emb[:],
                    in_offset=bass.IndirectOffsetOnAxis(ap=idx32[:, 0:1], axis=0),
                )

        for p in reversed(pools):
            p.release()


def host_prep(inputs):
    """Build per-core in_maps from the full problem inputs."""
    z = np.asarray(inputs["z"], np.float32)
    embedding = np.ascontiguousarray(np.asarray(inputs["embedding"], np.float32))
    Wh = np.asarray(inputs["Wh"], np.float32)
    bh = np.asarray(inputs["bh"], np.float32)
    Wc = np.asarray(inputs["Wc"], np.float32)
    bc = np.asarray(inputs["bc"], np.float32)
    Wih = np.asarray(inputs["Wih"], np.float32)
    Whh = np.asarray(inputs["Whh"], np.float32)
    bih = np.asarray(inputs["bih"], np.float32)
    bhh = np.asarray(inputs["bhh"], np.float32)
    Wfc = np.asarray(inputs["Wfc"], np.float32)
    bfc = np.asarray(inputs["bfc"], np.float32)

    h0 = (z @ Wh.T + bh).astype(np.float32)   # [B, H]
    c0 = (z @ Wc.T + bc).astype(np.float32)
    b_gates = (bih + bhh).astype(np.float32)  # [4H]

    # gate column permutation: c' = q*512 + slot*128 + hw with slot order
    # [i, f, o, g] so the sigmoid gates are one contiguous 384-wide range.
    cp = np.arange(2048)
    qq, rem = cp // 512, cp % 512
    slot, hw = rem // 128, rem % 128
    gate = np.array([0, 1, 3, 2])[slot]        # slot -> original gate (i,f,o,g)
    perm = gate * 512 + qq * 128 + hw          # original col index for permuted col c'
    Wall = np.concatenate([Wih, Whh], axis=1)  # [2048, 1024] (k = [x | h])
    Wperm = Wall[perm]                         # [2048, 1024]
    wgates = np.ascontiguousarray(Wperm.T)     # [1024, 2048]

    import ml_dtypes

    def split_bf16(w):
        hi = w.astype(ml_dtypes.bfloat16)
        lo = (w - hi.astype(np.float32)).astype(ml_dtypes.bfloat16)
        return np.ascontiguousarray(hi), np.ascontiguousarray(lo)

    wgates_hi, wgates_lo = split_bf16(wgates)
    bg_hi, bg_lo = split_bf16(b_gates[perm][None, :])
    bias_g2 = np.ascontiguousarray(np.concatenate([bg_hi, bg_lo], axis=0))  # [2, 2048]

    # state layout tiles
    h0t = np.zeros((128, 128), np.float32)     # h0t[p, q*32+b] = h0[b, 128q+p]
    c0t = np.zeros((128, 128), np.float32)     # c0t[32q+b, hw] = c0[b, 128q+hw]
    for q in range(4):
        h0t[:, 32 * q:32 * (q + 1)] = h0[:, 128 * q:128 * (q + 1)].T
        c0t[32 * q:32 * (q + 1), :] = c0[:, 128 * q:128 * (q + 1)]

    ident = np.eye(128, dtype=np.float32)
    identb = np.eye(32, dtype=ml_dtypes.bfloat16)
    ones2 = np.ones((2, 32), ml_dtypes.bfloat16)
    # G[v] = emb[v] @ Wih.T + b_gates in the permuted psum gate layout
    G = embedding @ wgates[0:512, :] + b_gates[perm][None, :]
    G_hi, G_lo = split_bf16(G.astype(np.float32))
    emb2 = np.ascontiguousarray(np.concatenate([G_hi, G_lo], axis=1))

    in_maps = []
    for j in range(N_CORES):
        shard = Wfc[VS * j:VS * (j + 1)]                    # [4000, 512]
        shard_p = np.zeros((VSP, H), np.float32)
        shard_p[:VS] = shard
        wfc_in = np.ascontiguousarray(shard_p.T)            # [512, 4096]
        wfc_hi, wfc_lo = split_bf16(wfc_in)
        bfc_p = np.full(VSP, -1e30, np.float32)
        bfc_p[:VS] = bfc[VS * j:VS * (j + 1)]
        bf_hi, bf_lo = split_bf16(bfc_p[None, :])
        bias_fc2 = np.ascontiguousarray(np.concatenate([bf_hi, bf_lo], axis=0))
        # gkey[p, nt] = BIG - (VS*j + 1024*(p//32) + 512*nt)
        gbase = VS * j + (np.arange(128) // 32) * 1024
        gkey = np.stack([BIG - gbase, BIG - gbase - 512], axis=1).astype(np.float32)
        in_maps.append({
            "wfc_hi": wfc_hi,
            "wfc_lo": wfc_lo,
            "wgates_hi": wgates_hi,
            "wgates_lo": wgates_lo,
            "bias_g2": bias_g2,
            "bias_fc2": bias_fc2,
            "gkey": np.ascontiguousarray(gkey),
            "ident": ident,
            "ones2": ones2,
            "identb": identb,
            "h0t": h0t,
            "c0": c0t,
            "emb": emb2,
        })
    return in_maps


def declare_io(nc, n_steps):
    io = {}
    io["wfc_hi"] = nc.dram_tensor("wfc_hi", [512, VSP], BF16, kind="ExternalInput").ap()
    io["wfc_lo"] = nc.dram_tensor("wfc_lo", [512, VSP], BF16, kind="ExternalInput").ap()
    io["wgates_hi"] = nc.dram_tensor("wgates_hi", [1024, 2048], BF16, kind="ExternalInput").ap()
    io["wgates_lo"] = nc.dram_tensor("wgates_lo", [1024, 2048], BF16, kind="ExternalInput").ap()
    io["bias_g2"] = nc.dram_tensor("bias_g2", [2, 2048], BF16, kind="ExternalInput").ap()
    io["bias_fc2"] = nc.dram_tensor("bias_fc2", [2, VSP], BF16, kind="ExternalInput").ap()
    io["gkey"] = nc.dram_tensor("gkey", [128, 2], FP32, kind="ExternalInput").ap()
    io["ident"] = nc.dram_tensor("ident", [128, 128], FP32, kind="ExternalInput").ap()
    io["ones2"] = nc.dram_tensor("ones2", [2, 32], BF16, kind="ExternalInput").ap()
    io["identb"] = nc.dram_tensor("identb", [32, 32], BF16, kind="ExternalInput").ap()
    io["h0t"] = nc.dram_tensor("h0t", [128, 128], FP32, kind="ExternalInput").ap()
    io["c0"] = nc.dram_tensor("c0", [128, 128], FP32, kind="ExternalInput").ap()
    io["emb"] = nc.dram_tensor("emb", [V, 4096], BF16, kind="ExternalInput").ap()
    io["logits"] = nc.dram_tensor("logits", [n_steps, 128, 1024], BF16,
                                  kind="ExternalOutput").ap()
    return io


_BUILT = {}


def build(n_steps=T):
    if n_steps in _BUILT:
        return _BUILT[n_steps]
    nc = bacc.Bacc("TRN2", target_bir_lowering=False, debug=False,
                   num_devices=N_CORES)
    io = declare_io(nc, n_steps)
    build_decoder(nc, io, n_steps)
    nc.compile()
    _BUILT[n_steps] = nc
    return nc


def assemble(results, n_steps=T):
    """results: list of per-core out dicts -> full [B, T, V] fp32."""
    full = np.empty((B, n_steps, V), np.float32)
    for j in range(N_CORES):
        arr = results[j]["logits"].astype(np.float32)
        arr = arr.reshape(n_steps, 4, 32, 1024)
        arr = arr.transpose(2, 0, 1, 3).reshape(B, n_steps, VSP)[:, :, :VS]
        full[:, :, VS * j:VS * (j + 1)] = arr
    return full


def kernel(**inputs):
    n_steps = int(inputs.get("context_length", T))
    assert n_steps == T, f"kernel hardcodes T={T}, got {n_steps}"
    nc = build(T)
    in_maps = host_prep(inputs)
    res = run_bass_kernel_spmd(nc, in_maps, core_ids=list(range(N_CORES)))
    return assemble(res.results, T)


if __name__ == "__main__":
    import reference
    inputs = reference.setup_inputs()
    out = kernel(**{k: np.asarray(v) if hasattr(v, "shape") else v
                    for k, v in inputs.items()})
    print("output shape:", out.shape)
